# revision 51
# baseline (speedup 1.0000x reference)
"""Trainium2 Bass kernel for nn_MultiHeadAttention_37512244363503.

Sharding: 8 cores = 4 batches x 2 head-groups (8 heads each).
Per core (b, g): Wq/Wk/Wv column-sliced, Wo row-sliced; the host sums the
two partial outputs per batch (the row-parallel "all-reduce") and adds bo.

Key compaction: pad_mask is host-visible, so masked keys are dropped on
the host before upload. Keys compact to jt*128 columns (jt chosen from
the max per-batch unmasked count, 5 for the reference distribution),
cutting K/V projections, scores, softmax and AV by L_c/L. Padding slots
get a -30000 exp-bias so they contribute exactly 0.

All DRAM inputs are pre-tiled on host to the SBUF k-tile-major layout so
every load is a contiguous [128, cols] copy (no strided descriptors).

Per-core algorithm (matmuls bf16 in / fp32 PSUM accumulate):
  QT[d,i]   = Wq_g.T @ x_q[b].T        (d=512 cols of this group)
  KT[d,j]   = Wk_g.T @ x_kv_c[b].T     (j over compacted keys)
  V[j,c]    = x_kv_c[b] @ Wv_g         (per-head [ones|V_h] / [V_h|ones])
  ST[j,i]   = K_h Q_h.T per head       (2 heads packed via PE row groups)
  PT        = exp(ST/8 + mask_bias[j]) (ACT; bias rides the ACT bias input)
  po        = V_ext.T @ PT             (denom + O^T in one matmul)
  O_norm^T  = po_OT * recip(po_denom)  (DVE; written straight into ot_sb)
  partial   = O_norm @ Wo_g            (bf16 partial -> DRAM, host sums)

Norms write directly into ot_sb (no SBUF->SBUF DMA): head A (hh=0) keeps
V_ext = [ones|V] so denom sits at PSUM rows 0:64 and O^T at 64:128; the
DVE mul shifts the PSUM operand down to write ot rows 0:64. Head B flips
V_ext = [V|ones] so its mul writes ot rows 64:128 with the SBUF operands
(recip, out) partition-aligned at 64. The fast reciprocal always reads
the full [128,*] PSUM tile base-aligned (unused rows are garbage but
never read; the custom DVE op corrupts on shifted APs, so base-aligned
full-height is the only safe form).
"""

import numpy as np
import ml_dtypes

import concourse.bass as bass
import concourse.mybir as mybir
from concourse import bacc
from concourse.tile import TileContext
from concourse.bass_utils import run_bass_kernel_spmd

BF16 = ml_dtypes.bfloat16

B, N, L, D, H = 4, 1024, 1024, 1024, 16
DH = D // H           # 64 channels per head
HG = 8                # heads per core
DG = HG * DH          # 512 channels per core
NCORES = 8
DP_SCALE = DH ** -0.5
MASK_NEG = -30000.0   # exp(x + MASK_NEG) underflows to exactly 0.0

f32 = mybir.dt.float32
bf16 = mybir.dt.bfloat16

KT = D // 128         # 8 k-tiles in the contraction dim of projections
IT = N // 128         # 8 query tiles
IC = N // 512         # 2 query chunks (PSUM free dim)
PAIRS = HG // 2       # 4 head pairs (2 heads packed per 128 partitions)
VW = 2 * DH           # 128 cols per (j, head) V_ext block


def build_nc(jt=5, debug=False, num_devices=NCORES, repeat=1):
    lc = jt * 128
    nc = bacc.Bacc("TRN2", target_bir_lowering=False, debug=False,
                   num_devices=num_devices)

    xq = nc.dram_tensor("xq", [128, KT * N], bf16, kind="ExternalInput")
    xkv = nc.dram_tensor("xkv", [128, KT * lc], bf16, kind="ExternalInput")
    # wq/wk are PAIR-major ([128, pair*KT*128 + k*128 + c]) so one pair's
    # projection weights are a single contiguous 256KB slice loadable first
    wq = nc.dram_tensor("wq", [128, KT * DG], bf16, kind="ExternalInput")
    wk = nc.dram_tensor("wk", [128, KT * DG], bf16, kind="ExternalInput")
    wv = nc.dram_tensor("wv", [128, KT * DG], bf16, kind="ExternalInput")
    wo = nc.dram_tensor("wo", [128, PAIRS * D], bf16, kind="ExternalInput")
    mb = nc.dram_tensor("mb", [128, jt], f32, kind="ExternalInput")
    out = nc.dram_tensor("out", [N, D], bf16, kind="ExternalOutput")

    with TileContext(nc) as tc:
        with (
            tc.tile_pool(name="persist", bufs=1) as persist,
            tc.tile_pool(name="pt", bufs=2) as pt_pool,
            tc.tile_pool(name="recp", bufs=4) as recp,
            tc.tile_pool(name="scp", bufs=4) as scp,
            tc.tile_pool(name="stps", bufs=2, space="PSUM") as stps,
            tc.tile_pool(name="smps", bufs=2, space="PSUM") as smps,
        ):
            env = dict(jt=jt, lc=lc)
            for nm, shape in [
                ("xq_sb", [128, KT * N]), ("xkv_sb", [128, KT * lc]),
                ("wq_sb", [128, KT * DG]), ("wk_sb", [128, KT * DG]),
                ("wv_sb", [128, KT * DG]), ("wo_sb", [128, PAIRS * D]),
                ("qT_sb", [128, PAIRS * N]), ("kT_sb", [128, PAIRS * lc]),
                ("v_sb", [128, jt * HG * VW]), ("ot_sb", [128, PAIRS * N]),
            ]:
                env[nm] = persist.tile(shape, bf16, name=nm)
            env["mb_sb"] = persist.tile([128, jt], f32, name="mb_sb")
            env["v_view"] = env["v_sb"][:].rearrange(
                "p (j h c) -> p j h c", j=jt, h=HG)
            # Per-head V_ext layout: head A (even h) = [ones | V_h] so the
            # softmax denominator lands at PSUM rows 0:64 (base-aligned for
            # the custom reciprocal) and O^T at 64:128 (the shiftable PSUM
            # mul operand, written to ot rows 0:64). Head B (odd h) =
            # [V_h | ones]: O^T at rows 0:64 shifts down into ot rows
            # 64:128 with recip/out SBUF operands aligned at 64.
            nc.vector.memset(env["v_view"][:, :, 0::2, 0:DH], 1.0)
            nc.vector.memset(env["v_view"][:, :, 1::2, DH:VW], 1.0)
            env.update(pt_pool=pt_pool, recp=recp, scp=scp, stps=stps,
                       smps=smps, xq=xq, xkv=xkv, wq=wq, wk=wk, wv=wv,
                       wo=wo, mb=mb, out=out)
            for _rep in range(repeat):
                _emit_body(nc, env)

    nc.compile()
    return nc


def _emit_body(nc, env):
    jt, lc = env["jt"], env["lc"]
    xq_sb, xkv_sb = env["xq_sb"], env["xkv_sb"]
    wq_sb, wk_sb, wv_sb, wo_sb = (env[k] for k in
                                  ["wq_sb", "wk_sb", "wv_sb", "wo_sb"])
    qT_sb, kT_sb, v_sb, ot_sb, mb_sb = (env[k] for k in
                                        ["qT_sb", "kT_sb", "v_sb", "ot_sb",
                                         "mb_sb"])
    v_view = env["v_view"]
    pt_pool, recp, scp = env["pt_pool"], env["recp"], env["scp"]
    stps, smps = env["stps"], env["smps"]
    xq, xkv, wq, wk, wv, wo, mb, out = (env[k] for k in
                                        ["xq", "xkv", "wq", "wk", "wv",
                                         "wo", "mb", "out"])

    # ---- input loads ----
    # Loads are ordered by first PE use: xq k0 + pair-0 Q weights, the
    # rest of xq (k-tile streamed under the q0 projection), pair-0 K
    # weights, xkv, then the filler-unit inputs (wq pairs 1-3, wv, wk
    # pairs 1-3) and finally wo. Three rules learned from the timeline:
    # (1) the DMA transfer queue drains in request order, so loads must
    # reach it strictly in consumption order -- a lower-priority load on
    # a faster-clearing queue cuts the line and delays ST p0;
    # (2) ALL loads go on sync: a dma_start holds its issuing engine's
    # SEQ through the serialized shared HWDGE descriptor-gen stage, so
    # scalar-issued loads would block the prologue ACT copies (which hold
    # the pj PSUM ring!) and the exp stream behind ~8us of queued issues;
    # (3) gpsimd's SWDGE path (1us serialized issue, separate queue that
    # would jump the HWDGE stream) only carries the tiny mb load.
    def ld_split(dst, src, c0, c1, nsplit):
        step = 128 // nsplit
        for s in range(nsplit):
            p0, p1 = s * step, (s + 1) * step
            nc.sync.dma_start(out=dst[p0:p1, c0:c1], in_=src[p0:p1, c0:c1])

    nc.gpsimd.dma_start(out=mb_sb[:], in_=mb[:, :])
    PW = KT * 128  # cols per pair in the pair-major wq/wk layouts
    ld_split(xq_sb, xq, 0, N, 1)
    ld_split(wq_sb, wq, 0, 128, 1)         # 32KB: q0's first matmul dep
    ld_split(wq_sb, wq, 128, PW, 1)
    for k in range(1, KT):
        ld_split(xq_sb, xq, k * N, (k + 1) * N, 1)
    ld_split(wk_sb, wk, 0, PW, 1)
    qx = KT * lc // 4
    for s in range(4):
        ld_split(xkv_sb, xkv, s * qx, (s + 1) * qx, 1)
    ld_split(wq_sb, wq, PW, 2 * PW, 1)     # q1 (early pair-0 filler)
    ld_split(wq_sb, wq, 2 * PW, 3 * PW, 1)  # q2
    ld_split(wv_sb, wv, 0, KT * DG, 4)      # v_proj fillers (pair 0)
    ld_split(wk_sb, wk, PW, 2 * PW, 1)      # k1 (pair-0 last filler)
    ld_split(wq_sb, wq, 3 * PW, 4 * PW, 1)  # q3 (pair-1 filler)
    ld_split(wk_sb, wk, 2 * PW, 4 * PW, 2)
    ld_split(wo_sb, wo, 0, PAIRS * D, 2)

    # k-proj output chunking (lc may exceed one PSUM bank)
    kch = [(0, min(512, lc))] + ([(512, lc)] if lc > 512 else [])

    def make_proj(dst_sb, w_sb, x_sb, p, cols, chunks, on_act=False):
        """Projection for pair p as two units sharing PSUM accumulators.

        chunks: list of (c0, c1) output-column ranges (<=512 wide each).
        on_act: do the PSUM->SBUF copies on the idle ACT engine (prologue
        projections), keeping the DVE stream free.
        """
        ps = [None] * len(chunks)

        def half(k0, k1):
            for ci, (c0, c1) in enumerate(chunks):
                if k0 == 0:
                    ps[ci] = smps.tile([128, c1 - c0], f32,
                                       name="pj%d" % ci, tag="pj")
            for k in range(k0, k1):
                w = w_sb[:, (p * KT + k) * 128: (p * KT + k + 1) * 128]
                for ci, (c0, c1) in enumerate(chunks):
                    nc.tensor.matmul(
                        ps[ci][:],
                        lhsT=w,
                        rhs=x_sb[:, k * cols + c0: k * cols + c1],
                        start=(k == 0), stop=(k == KT - 1))
            if k1 == KT:
                for ci, (c0, c1) in enumerate(chunks):
                    dst = dst_sb[:, p * cols + c0: p * cols + c1]
                    if on_act:
                        nc.scalar.activation(
                            out=dst, in_=ps[ci][:],
                            func=mybir.ActivationFunctionType.Copy)
                    else:
                        nc.vector.tensor_copy(out=dst, in_=ps[ci][:])

        return [lambda: half(0, KT // 2), lambda: half(KT // 2, KT)]

    def proj_q(p, on_act=False):
        return make_proj(qT_sb, wq_sb, xq_sb, p, N, [(0, 512), (512, 1024)],
                         on_act)

    def proj_k(p, on_act=False):
        return make_proj(kT_sb, wk_sb, xkv_sb, p, lc, kch, on_act)

    def v_proj(j):
        """V[j, c] = x_kv @ Wv_g for one j tile (interleaved dst halves)."""
        ps = smps.tile([128, 512], f32, tag="av", bufs=1)
        for k in range(KT):
            nc.tensor.matmul(
                ps[:],
                lhsT=xkv_sb[:, k * lc + j * 128: k * lc + (j + 1) * 128],
                rhs=wv_sb[:, k * DG:(k + 1) * DG],
                start=(k == 0), stop=(k == KT - 1))
        pv = ps[:].rearrange("p (h c) -> p h c", h=HG)
        # head A (even) V goes to cols DH:VW, head B (odd) to cols 0:DH
        nc.vector.tensor_copy(out=v_view[:, j, 0::2, DH:VW], in_=pv[:, 0::2])
        nc.vector.tensor_copy(out=v_view[:, j, 1::2, 0:DH], in_=pv[:, 1::2])

    def norm(po, p, hh, chunks=1, rec_act=False):
        """Normalize one head's AV straight into ot_sb (no DMA).

        hh=0 (head A, V_ext=[ones|V]): denom rows 0:64, O^T rows 64:128;
        mul shifts the PSUM operand down into ot rows 0:64.
        hh=1 (head B, V_ext=[V|ones]): O^T rows 0:64 shifts up into ot
        rows 64:128; recip/out SBUF operands aligned at partition 64.
        The reciprocal runs full-height base-aligned; the unused half is
        garbage (1/O^T values) that is never read. denom > ~1 always, so
        the fast approx reciprocal's denorm/inf edge cases can't occur
        on the rows that are consumed.
        """
        rec_t = recp.tile([128, 1024], f32)
        dn, ot_rows = ((slice(0, 64), slice(64, 128)) if hh == 0
                       else (slice(64, 128), slice(0, 64)))
        step = 1024 // chunks
        for c in range(chunks):
            cs = slice(c * step, (c + 1) * step)
            nc.vector.reciprocal_approx_fast(out=rec_t[:, cs],
                                             in_=po[:, cs])
            nc.vector.tensor_mul(
                out=ot_sb[dn, p * N + c * step: p * N + (c + 1) * step],
                in0=po[ot_rows, cs], in1=rec_t[dn, cs])

    def av_head(p, hh, pt, pool_tag="av", chunks=1, rec_act=False,
                skip_norm=False):
        """AV for head 2p+hh, both i chunks, one V weight load per j."""
        st = {}
        for u in av_head_units(p, hh, pt, pool_tag, chunks, rec_act, 1,
                               skip_norm, st):
            u()
        return st.get("po")

    def av_head_units(p, hh, pt, pool_tag="av", chunks=1, rec_act=False,
                      nunits=2, skip_norm=False, state=None):
        """AV for one head as nunits filler units (j-ranges + final norm)."""
        h = 2 * p + hh
        if state is None:
            state = {}

        def run(j0, j1):
            if j0 == 0:
                state["po"] = (
                    stps.tile([128, 1024], f32, name="po", tag="st")
                    if pool_tag == "st" else
                    smps.tile([128, 1024], f32, name="po", tag="av", bufs=1))
            po = state["po"]
            for j in range(j0, j1):
                vblk = v_sb[:, (j * HG + h) * VW: (j * HG + h + 1) * VW]
                nc.tensor.matmul(po[:, 0:512], lhsT=vblk,
                                 rhs=pt[:, j * N: j * N + 512],
                                 start=(j == 0), stop=(j == jt - 1))
                nc.tensor.matmul(po[:, 512:1024], lhsT=vblk,
                                 rhs=pt[:, j * N + 512: (j + 1) * N],
                                 start=(j == 0), stop=(j == jt - 1))
            if j1 == jt and not skip_norm:
                norm(po, p, hh, chunks=chunks, rec_act=rec_act)

        bounds = [jt * i // nunits for i in range(nunits + 1)]
        return [lambda a=a, b=b: run(a, b)
                for a, b in zip(bounds[:-1], bounds[1:])]

    def st_pair(p, pa, pb, slot_units):
        """ST + exp for pair p; filler units interleaved into HALF-j slots
        (2*jt of them) so the exp stream is fed a fresh ST half roughly
        every exp-duration instead of in bursts."""
        for j in range(jt):
            for half, (rb, pt) in enumerate(((0, pa), (64, pb))):
                ps = stps.tile([128, 1024], f32, name="st", tag="st")
                kk = kT_sb[rb:rb + 64, p * lc + j * 128: p * lc + (j + 1) * 128]
                for ic in range(IC):
                    cols = slice(ic * 512, ic * 512 + 512)
                    nc.tensor.matmul(
                        ps[:, cols], lhsT=kk,
                        rhs=qT_sb[rb:rb + 64,
                                  p * N + ic * 512: p * N + ic * 512 + 512],
                        start=True, stop=True)
                # EXP right after this half's matmuls: ACT starts earlier
                # and the pool slot frees a half-j sooner
                nc.scalar.activation(
                    out=pt[:, j * N:(j + 1) * N], in_=ps[:],
                    func=mybir.ActivationFunctionType.Exp,
                    bias=mb_sb[:, j:j + 1], scale=DP_SCALE)
                for u in slot_units.get(2 * j + half, []):
                    u()

    def op_mms(it, ps0, ps1, cts):
        for ct in cts:
            ot_blk = ot_sb[:, ct * N + it * 128: ct * N + (it + 1) * 128]
            nc.tensor.matmul(
                ps0[:], lhsT=ot_blk,
                rhs=wo_sb[:, ct * D: ct * D + 512],
                start=(ct == 0), stop=(ct == PAIRS - 1))
            nc.tensor.matmul(
                ps1[:], lhsT=ot_blk,
                rhs=wo_sb[:, ct * D + 512: ct * D + 1024],
                start=(ct == 0), stop=(ct == PAIRS - 1))

    def op_finish(it, ps0, ps1):
        # Copies split across DVE+ACT (both idle here). Early tiles store
        # whole on the idle Pool SWDGE path (separate issue queue, keeps
        # HWDGE clear); the last two tiles are latency-critical:
        # column-half stores on sync/scalar, each depending only on its
        # own engine's copy so neither store cross-waits the other engine.
        out_t = scp.tile([128, 1024], bf16, tag="outt")
        nc.vector.tensor_copy(out=out_t[:, 0:512], in_=ps0[:])
        nc.scalar.activation(out=out_t[:, 512:1024], in_=ps1[:],
                             func=mybir.ActivationFunctionType.Copy)
        r = slice(it * 128, (it + 1) * 128)
        if it < IT - 2:
            nc.gpsimd.dma_start(out=out[r, :], in_=out_t[:])
        else:
            nc.sync.dma_start(out=out[r, 0:512], in_=out_t[:, 0:512])
            nc.scalar.dma_start(out=out[r, 512:1024],
                                in_=out_t[:, 512:1024])

    # ---- prologue: q0 then k0 only (minimum work before the ST stream
    # starts); q1-q3 stream in as pair-0/1 fillers as their weights land
    for u in proj_q(0, on_act=True):
        u()
    for u in proj_k(0, on_act=True):
        u()

    # ---- pipelined pairs ----
    prev = None
    for p in range(PAIRS):
        pa = pt_pool.tile([128, jt * N], bf16, tag="pa")
        pb = pt_pool.tile([128, jt * N], bf16, tag="pb")

        if p == 0:
            # DMA-paced fillers in arrival order: q1, q2, v tiles (wv),
            # q3 waits for pair 1, k1 last (its weights land last)
            proj_units = (proj_q(1) + proj_q(2)
                          + [lambda j=j: v_proj(j) for j in range(jt)]
                          + proj_k(1))
        elif p + 1 < PAIRS:
            proj_units = list(proj_k(p + 1))
            if p == 1:
                proj_units = proj_q(3) + proj_units
        else:
            proj_units = []
        if prev is not None:
            # av heads use the single "av" PSUM buffer: keep them apart so
            # head B's alloc never stalls the PE on head A's norms
            pp, ppa, ppb = prev
            na = len(proj_units) // 2
            units = ([lambda: av_head(pp, 0, ppa)] + proj_units[:na]
                     + [lambda: av_head(pp, 1, ppb)] + proj_units[na:])
        else:
            units = proj_units

        slot_units = {}
        nslots = 2 * jt
        # monotonic slot assignment keeps each proj's k0-half before its
        # k1-half (they share PSUM accumulators); +2 phantom units lean
        # the distribution toward late slots so the pair tail (when the
        # exp stream still drains) keeps PE fed
        for i, u in enumerate(units):
            slot_units.setdefault(
                min(nslots - 1, (i + 6) * nslots // (len(units) + 6)),
                []).append(u)
        st_pair(p, pa, pb, slot_units)
        prev = (p, pa, pb)

    # last pair's AV: head B first (single 'av' buffer), then head A from
    # the ST banks. The norms are hand-interleaved in column halves so the
    # DVE chain delivers the first 512 normalized query columns (both head
    # rows) as early as possible for the O-projection's ct=3 matmuls.
    pp, ppa, ppb = prev
    poB = av_head(pp, 1, ppb, skip_norm=True)
    recB_t = recp.tile([128, 1024], f32)
    nc.vector.reciprocal_approx_fast(out=recB_t[:], in_=poB[:])
    poA = av_head(pp, 0, ppa, pool_tag="st", skip_norm=True)
    recA_t = recp.tile([128, 1024], f32)
    for c in range(2):
        cs = slice(c * 512, (c + 1) * 512)
        oc = slice(pp * N + c * 512, pp * N + (c + 1) * 512)
        nc.vector.reciprocal_approx_fast(out=recA_t[:, cs], in_=poA[:, cs])
        # head B (V_ext=[V|ones]): O^T rows 0:64 -> ot rows 64:128
        nc.vector.tensor_mul(out=ot_sb[64:128, oc], in0=poB[0:64, cs],
                             in1=recB_t[64:128, cs])
        # head A (V_ext=[ones|V]): O^T rows 64:128 -> ot rows 0:64
        nc.vector.tensor_mul(out=ot_sb[0:64, oc], in0=poA[64:128, cs],
                             in1=recA_t[0:64, cs])

    # ---- output projection: partial[i, d] in bf16 ----
    # Runway: i-tiles 1-2 accumulate pairs 0-2 while the last norms run on
    # DVE; PSUM plan fills all 8 banks: AVB po (av, 2) + AVA po (st, 2) +
    # pend1 (st, 2) + pend2 (pj, 1+1). The O-proj loop then rotates
    # st/pj/av so tile allocs never wait on a two-deep copy pipeline.
    def op_psum(which):
        if which == "pj":
            o0 = smps.tile([128, 512], f32, name="o0", tag="pj")
            o1 = smps.tile([128, 512], f32, name="o1", tag="pj")
            return o0[:], o1[:]
        if which == "av":
            pw = smps.tile([128, 1024], f32, name="po", tag="av", bufs=1)
        else:
            pw = stps.tile([128, 1024], f32, name="pw", tag="st")
        return pw[:, 0:512], pw[:, 512:1024]

    pend = {}
    for it, pool in ((1, "st"), (2, "pj")):
        pend[it] = op_psum(pool)
        op_mms(it, *pend[it], range(PAIRS - 1))
    rot = ["av", "st", "st", "pj", "av", "st"]
    for it in range(IT):
        if it in pend:
            ps0, ps1 = pend[it]
            op_mms(it, ps0, ps1, range(PAIRS - 1, PAIRS))
        else:
            ps0, ps1 = op_psum(rot.pop(0))
            op_mms(it, ps0, ps1, range(PAIRS))
        op_finish(it, ps0, ps1)


_NC_CACHE = {}


def _get_nc(jt):
    if jt not in _NC_CACHE:
        _NC_CACHE[jt] = build_nc(jt=jt)
    return _NC_CACHE[jt]


def _tile_k(a, cols):
    """[KT*128, cols] -> [128, KT*cols] k-tile-major, contiguous bf16."""
    return np.ascontiguousarray(
        a.reshape(KT, 128, cols).transpose(1, 0, 2).reshape(128, KT * cols)
    ).astype(BF16)


def _make_in_maps(x_q, x_kv, pad_mask, Wq, Wk, Wv, Wo, jt=None):
    pad_mask = np.asarray(pad_mask)
    cnts = (~pad_mask).sum(axis=1)
    if jt is None:
        jt = max(1, int(-(-int(cnts.max()) // 128)))
    lc = jt * 128

    def _tile_pair(w):
        # [D, DG] -> [128, pair*KT*128 + k*128 + c] pair-major
        return np.ascontiguousarray(
            w.reshape(KT, 128, PAIRS, 128).transpose(1, 2, 0, 3)
            .reshape(128, PAIRS * KT * 128)).astype(BF16)

    per_g = []
    for g in range(2):
        cols = slice(g * DG, (g + 1) * DG)
        per_g.append({
            "wq": _tile_pair(np.ascontiguousarray(Wq[:, cols])),
            "wk": _tile_pair(np.ascontiguousarray(Wk[:, cols])),
            "wv": _tile_k(np.ascontiguousarray(Wv[:, cols]), DG),
            "wo": np.ascontiguousarray(
                Wo[g * DG:(g + 1) * DG, :]
                .reshape(PAIRS, 128, D).transpose(1, 0, 2)
                .reshape(128, PAIRS * D)).astype(BF16),
        })
    per_b = []
    for b in range(B):
        idx = np.flatnonzero(~pad_mask[b])
        n = len(idx)
        xc = np.zeros((lc, D), dtype=np.float32)
        xc[:n] = x_kv[b][idx]
        mbias = np.full(lc, MASK_NEG, dtype=np.float32)
        mbias[:n] = 0.0
        per_b.append({
            "xq": _tile_k(np.ascontiguousarray(x_q[b].T), N),
            "xkv": _tile_k(np.ascontiguousarray(xc.T), lc),
            "mb": np.ascontiguousarray(mbias.reshape(jt, 128).T),
        })

    in_maps = []
    for c in range(NCORES):
        b, g = c // 2, c % 2
        in_maps.append({**per_b[b], **per_g[g]})
    return in_maps, jt


def kernel(x_q, x_kv, pad_mask, Wq, Wk, Wv, Wo, bo):
    in_maps, jt = _make_in_maps(x_q, x_kv, pad_mask, Wq, Wk, Wv, Wo)
    nc = _get_nc(jt)
    res = run_bass_kernel_spmd(nc, in_maps, core_ids=list(range(NCORES)))
    full = np.empty((B, N, D), dtype=np.float32)
    bo32 = bo.astype(np.float32)
    for b in range(B):
        full[b] = (res.results[2 * b]["out"].astype(np.float32)
                   + res.results[2 * b + 1]["out"].astype(np.float32))
        full[b] += bo32
    return full


# revision 58
# speedup vs baseline: 1.0033x; 1.0033x over previous
"""Trainium2 Bass kernel for nn_MultiHeadAttention_37512244363503.

Sharding: 8 cores = 4 batches x 2 head-groups (8 heads each).
Per core (b, g): Wq/Wk/Wv column-sliced, Wo row-sliced; the host sums the
two partial outputs per batch (the row-parallel "all-reduce") and adds bo.

Key compaction: pad_mask is host-visible, so masked keys are dropped on
the host before upload. Keys compact to jt*128 columns (jt chosen from
the max per-batch unmasked count, 5 for the reference distribution),
cutting K/V projections, scores, softmax and AV by L_c/L. Padding slots
get a -30000 exp-bias so they contribute exactly 0.

All DRAM inputs are pre-tiled on host to the SBUF k-tile-major layout so
every load is a contiguous [128, cols] copy (no strided descriptors).

Per-core algorithm (matmuls bf16 in / fp32 PSUM accumulate):
  QT[d,i]   = Wq_g.T @ x_q[b].T        (d=512 cols of this group)
  KT[d,j]   = Wk_g.T @ x_kv_c[b].T     (j over compacted keys)
  V[j,c]    = x_kv_c[b] @ Wv_g         (per-head [ones|V_h] / [V_h|ones])
  ST[j,i]   = K_h Q_h.T per head       (2 heads packed via PE row groups)
  PT        = exp(ST/8 + mask_bias[j]) (ACT; bias rides the ACT bias input)
  po        = V_ext.T @ PT             (denom + O^T in one matmul)
  O_norm^T  = po_OT * recip(po_denom)  (DVE; written straight into ot_sb)
  partial   = O_norm @ Wo_g            (bf16 partial -> DRAM, host sums)

Norms write directly into ot_sb (no SBUF->SBUF DMA): head A (hh=0) keeps
V_ext = [ones|V] so denom sits at PSUM rows 0:64 and O^T at 64:128; the
DVE mul shifts the PSUM operand down to write ot rows 0:64. Head B flips
V_ext = [V|ones] so its mul writes ot rows 64:128 with the SBUF operands
(recip, out) partition-aligned at 64. The fast reciprocal always reads
the full [128,*] PSUM tile base-aligned (unused rows are garbage but
never read; the custom DVE op corrupts on shifted APs, so base-aligned
full-height is the only safe form).
"""

import numpy as np
import ml_dtypes

import concourse.bass as bass
import concourse.mybir as mybir
from concourse import bacc
from concourse.tile import TileContext
from concourse.bass_utils import run_bass_kernel_spmd

BF16 = ml_dtypes.bfloat16

B, N, L, D, H = 4, 1024, 1024, 1024, 16
DH = D // H           # 64 channels per head
HG = 8                # heads per core
DG = HG * DH          # 512 channels per core
NCORES = 8
DP_SCALE = DH ** -0.5
MASK_NEG = -30000.0   # exp(x + MASK_NEG) underflows to exactly 0.0

f32 = mybir.dt.float32
bf16 = mybir.dt.bfloat16

KT = D // 128         # 8 k-tiles in the contraction dim of projections
IT = N // 128         # 8 query tiles
IC = N // 512         # 2 query chunks (PSUM free dim)
PAIRS = HG // 2       # 4 head pairs (2 heads packed per 128 partitions)
VW = 2 * DH           # 128 cols per (j, head) V_ext block


def build_nc(jt=5, debug=False, num_devices=NCORES, repeat=1):
    lc = jt * 128
    nc = bacc.Bacc("TRN2", target_bir_lowering=False, debug=False,
                   num_devices=num_devices)

    xq = nc.dram_tensor("xq", [128, KT * N], bf16, kind="ExternalInput")
    xkv = nc.dram_tensor("xkv", [128, KT * lc], bf16, kind="ExternalInput")
    # wq/wk are PAIR-major ([128, pair*KT*128 + k*128 + c]) so one pair's
    # projection weights are a single contiguous 256KB slice loadable first
    wq = nc.dram_tensor("wq", [128, KT * DG], bf16, kind="ExternalInput")
    wk = nc.dram_tensor("wk", [128, KT * DG], bf16, kind="ExternalInput")
    wv = nc.dram_tensor("wv", [128, KT * DG], bf16, kind="ExternalInput")
    wo = nc.dram_tensor("wo", [128, PAIRS * D], bf16, kind="ExternalInput")
    mb = nc.dram_tensor("mb", [128, jt], f32, kind="ExternalInput")
    out = nc.dram_tensor("out", [N, D], bf16, kind="ExternalOutput")

    with TileContext(nc) as tc:
        with (
            tc.tile_pool(name="persist", bufs=1) as persist,
            tc.tile_pool(name="pt", bufs=3) as pt_pool,
            tc.tile_pool(name="recp", bufs=4) as recp,
            tc.tile_pool(name="scp", bufs=4) as scp,
            tc.tile_pool(name="stps", bufs=2, space="PSUM") as stps,
            tc.tile_pool(name="smps", bufs=2, space="PSUM") as smps,
        ):
            env = dict(jt=jt, lc=lc)
            for nm, shape in [
                ("xq_sb", [128, KT * N]), ("xkv_sb", [128, KT * lc]),
                ("wq_sb", [128, KT * DG]), ("wk_sb", [128, KT * DG]),
                ("wv_sb", [128, KT * DG]), ("wo_sb", [128, PAIRS * D]),
                ("qT_sb", [128, PAIRS * N]), ("kT_sb", [128, PAIRS * lc]),
                ("v_sb", [128, jt * HG * VW]), ("ot_sb", [128, PAIRS * N]),
            ]:
                env[nm] = persist.tile(shape, bf16, name=nm)
            env["mb_sb"] = persist.tile([128, jt], f32, name="mb_sb")
            env["v_view"] = env["v_sb"][:].rearrange(
                "p (j h c) -> p j h c", j=jt, h=HG)
            # Per-head V_ext layout: head A (even h) = [ones | V_h] so the
            # softmax denominator lands at PSUM rows 0:64 (base-aligned for
            # the custom reciprocal) and O^T at 64:128 (the shiftable PSUM
            # mul operand, written to ot rows 0:64). Head B (odd h) =
            # [V_h | ones]: O^T at rows 0:64 shifts down into ot rows
            # 64:128 with recip/out SBUF operands aligned at 64.
            nc.vector.memset(env["v_view"][:, :, 0::2, 0:DH], 1.0)
            nc.vector.memset(env["v_view"][:, :, 1::2, DH:VW], 1.0)
            env.update(pt_pool=pt_pool, recp=recp, scp=scp, stps=stps,
                       smps=smps, xq=xq, xkv=xkv, wq=wq, wk=wk, wv=wv,
                       wo=wo, mb=mb, out=out)
            for _rep in range(repeat):
                _emit_body(nc, env)

    nc.compile()
    return nc


def _emit_body(nc, env):
    jt, lc = env["jt"], env["lc"]
    xq_sb, xkv_sb = env["xq_sb"], env["xkv_sb"]
    wq_sb, wk_sb, wv_sb, wo_sb = (env[k] for k in
                                  ["wq_sb", "wk_sb", "wv_sb", "wo_sb"])
    qT_sb, kT_sb, v_sb, ot_sb, mb_sb = (env[k] for k in
                                        ["qT_sb", "kT_sb", "v_sb", "ot_sb",
                                         "mb_sb"])
    v_view = env["v_view"]
    pt_pool, recp, scp = env["pt_pool"], env["recp"], env["scp"]
    stps, smps = env["stps"], env["smps"]
    xq, xkv, wq, wk, wv, wo, mb, out = (env[k] for k in
                                        ["xq", "xkv", "wq", "wk", "wv",
                                         "wo", "mb", "out"])

    # ---- input loads ----
    # Loads are ordered by first PE use: xq k0 + pair-0 Q weights, the
    # rest of xq (k-tile streamed under the q0 projection), pair-0 K
    # weights, xkv, then the filler-unit inputs (wq pairs 1-3, wv, wk
    # pairs 1-3) and finally wo. Three rules learned from the timeline:
    # (1) the DMA transfer queue drains in request order, so loads must
    # reach it strictly in consumption order -- a lower-priority load on
    # a faster-clearing queue cuts the line and delays ST p0;
    # (2) ALL loads go on sync: a dma_start holds its issuing engine's
    # SEQ through the serialized shared HWDGE descriptor-gen stage, so
    # scalar-issued loads would block the prologue ACT copies (which hold
    # the pj PSUM ring!) and the exp stream behind ~8us of queued issues;
    # (3) gpsimd's SWDGE path (1us serialized issue, separate queue that
    # would jump the HWDGE stream) only carries the tiny mb load.
    def ld_split(dst, src, c0, c1, nsplit):
        step = 128 // nsplit
        for s in range(nsplit):
            p0, p1 = s * step, (s + 1) * step
            nc.sync.dma_start(out=dst[p0:p1, c0:c1], in_=src[p0:p1, c0:c1])

    nc.gpsimd.dma_start(out=mb_sb[:], in_=mb[:, :])
    PW = KT * 128  # cols per pair in the pair-major wq/wk layouts
    ld_split(xq_sb, xq, 0, N, 1)
    ld_split(wq_sb, wq, 0, 256, 1)         # 64KB: q0 k0/k1 matmul deps
    ld_split(wq_sb, wq, 256, PW, 1)
    for k in range(1, KT):
        ld_split(xq_sb, xq, k * N, (k + 1) * N, 1)
    ld_split(wk_sb, wk, 0, PW, 1)
    qx = KT * lc // 4
    for s in range(4):
        ld_split(xkv_sb, xkv, s * qx, (s + 1) * qx, 1)
    ld_split(wq_sb, wq, PW, 2 * PW, 1)     # q1 (early pair-0 filler)
    ld_split(wq_sb, wq, 2 * PW, 3 * PW, 1)  # q2
    ld_split(wv_sb, wv, 0, KT * DG, 4)      # v_proj fillers (pair 0)
    ld_split(wk_sb, wk, PW, 2 * PW, 1)      # k1 (pair-0 last filler)
    ld_split(wq_sb, wq, 3 * PW, 4 * PW, 1)  # q3 (pair-1 filler)
    ld_split(wk_sb, wk, 2 * PW, 4 * PW, 2)
    ld_split(wo_sb, wo, 0, PAIRS * D, 2)

    # k-proj output chunking (lc may exceed one PSUM bank)
    kch = [(0, min(512, lc))] + ([(512, lc)] if lc > 512 else [])

    def make_proj(dst_sb, w_sb, x_sb, p, cols, chunks, on_act=False):
        """Projection for pair p as two units sharing PSUM accumulators.

        chunks: list of (c0, c1) output-column ranges (<=512 wide each).
        on_act: do the PSUM->SBUF copies on the idle ACT engine (prologue
        projections), keeping the DVE stream free.
        """
        ps = [None] * len(chunks)

        def half(k0, k1):
            for ci, (c0, c1) in enumerate(chunks):
                if k0 == 0:
                    ps[ci] = smps.tile([128, c1 - c0], f32,
                                       name="pj%d" % ci, tag="pj")
            for k in range(k0, k1):
                w = w_sb[:, (p * KT + k) * 128: (p * KT + k + 1) * 128]
                for ci, (c0, c1) in enumerate(chunks):
                    nc.tensor.matmul(
                        ps[ci][:],
                        lhsT=w,
                        rhs=x_sb[:, k * cols + c0: k * cols + c1],
                        start=(k == 0), stop=(k == KT - 1))
            if k1 == KT:
                for ci, (c0, c1) in enumerate(chunks):
                    dst = dst_sb[:, p * cols + c0: p * cols + c1]
                    if on_act:
                        nc.scalar.activation(
                            out=dst, in_=ps[ci][:],
                            func=mybir.ActivationFunctionType.Copy)
                    else:
                        nc.vector.tensor_copy(out=dst, in_=ps[ci][:])

        return [lambda: half(0, KT // 2), lambda: half(KT // 2, KT)]

    def proj_q(p, on_act=False):
        return make_proj(qT_sb, wq_sb, xq_sb, p, N, [(0, 512), (512, 1024)],
                         on_act)

    def proj_k(p, on_act=False):
        return make_proj(kT_sb, wk_sb, xkv_sb, p, lc, kch, on_act)

    def v_proj(j):
        """V[j, c] = x_kv @ Wv_g for one j tile (interleaved dst halves)."""
        ps = smps.tile([128, 512], f32, tag="av", bufs=1)
        for k in range(KT):
            nc.tensor.matmul(
                ps[:],
                lhsT=xkv_sb[:, k * lc + j * 128: k * lc + (j + 1) * 128],
                rhs=wv_sb[:, k * DG:(k + 1) * DG],
                start=(k == 0), stop=(k == KT - 1))
        pv = ps[:].rearrange("p (h c) -> p h c", h=HG)
        # head A (even) V goes to cols DH:VW, head B (odd) to cols 0:DH
        nc.vector.tensor_copy(out=v_view[:, j, 0::2, DH:VW], in_=pv[:, 0::2])
        nc.vector.tensor_copy(out=v_view[:, j, 1::2, 0:DH], in_=pv[:, 1::2])

    def norm(po, p, hh, chunks=1, rec_act=False):
        """Normalize one head's AV straight into ot_sb (no DMA).

        hh=0 (head A, V_ext=[ones|V]): denom rows 0:64, O^T rows 64:128;
        mul shifts the PSUM operand down into ot rows 0:64.
        hh=1 (head B, V_ext=[V|ones]): O^T rows 0:64 shifts up into ot
        rows 64:128; recip/out SBUF operands aligned at partition 64.
        The reciprocal runs full-height base-aligned; the unused half is
        garbage (1/O^T values) that is never read. denom > ~1 always, so
        the fast approx reciprocal's denorm/inf edge cases can't occur
        on the rows that are consumed.
        """
        rec_t = recp.tile([128, 1024], f32)
        dn, ot_rows = ((slice(0, 64), slice(64, 128)) if hh == 0
                       else (slice(64, 128), slice(0, 64)))
        step = 1024 // chunks
        for c in range(chunks):
            cs = slice(c * step, (c + 1) * step)
            nc.vector.reciprocal_approx_fast(out=rec_t[:, cs],
                                             in_=po[:, cs])
            nc.vector.tensor_mul(
                out=ot_sb[dn, p * N + c * step: p * N + (c + 1) * step],
                in0=po[ot_rows, cs], in1=rec_t[dn, cs])

    def av_head(p, hh, pt, pool_tag="av", chunks=1, rec_act=False,
                skip_norm=False):
        """AV for head 2p+hh, both i chunks, one V weight load per j."""
        st = {}
        for u in av_head_units(p, hh, pt, pool_tag, chunks, rec_act, 1,
                               skip_norm, st):
            u()
        return st.get("po")

    def av_head_units(p, hh, pt, pool_tag="av", chunks=1, rec_act=False,
                      nunits=2, skip_norm=False, state=None):
        """AV for one head as nunits filler units (j-ranges + final norm)."""
        h = 2 * p + hh
        if state is None:
            state = {}

        def run(j0, j1):
            if j0 == 0:
                state["po"] = (
                    stps.tile([128, 1024], f32, name="po", tag="st")
                    if pool_tag == "st" else
                    smps.tile([128, 1024], f32, name="po", tag="av", bufs=1))
            po = state["po"]
            for j in range(j0, j1):
                vblk = v_sb[:, (j * HG + h) * VW: (j * HG + h + 1) * VW]
                nc.tensor.matmul(po[:, 0:512], lhsT=vblk,
                                 rhs=pt[:, j * N: j * N + 512],
                                 start=(j == 0), stop=(j == jt - 1))
                nc.tensor.matmul(po[:, 512:1024], lhsT=vblk,
                                 rhs=pt[:, j * N + 512: (j + 1) * N],
                                 start=(j == 0), stop=(j == jt - 1))
            if j1 == jt and not skip_norm:
                norm(po, p, hh, chunks=chunks, rec_act=rec_act)

        bounds = [jt * i // nunits for i in range(nunits + 1)]
        return [lambda a=a, b=b: run(a, b)
                for a, b in zip(bounds[:-1], bounds[1:])]

    def st_pair(p, pa, pb, slot_units):
        """ST + exp for pair p; filler units interleaved into HALF-j slots
        (2*jt of them) so the exp stream is fed a fresh ST half roughly
        every exp-duration instead of in bursts."""
        for j in range(jt):
            for half, (rb, pt) in enumerate(((0, pa), (64, pb))):
                ps = stps.tile([128, 1024], f32, name="st", tag="st")
                kk = kT_sb[rb:rb + 64, p * lc + j * 128: p * lc + (j + 1) * 128]
                for ic in range(IC):
                    cols = slice(ic * 512, ic * 512 + 512)
                    nc.tensor.matmul(
                        ps[:, cols], lhsT=kk,
                        rhs=qT_sb[rb:rb + 64,
                                  p * N + ic * 512: p * N + ic * 512 + 512],
                        start=True, stop=True)
                # EXP right after this half's matmuls: ACT starts earlier
                # and the pool slot frees a half-j sooner
                nc.scalar.activation(
                    out=pt[:, j * N:(j + 1) * N], in_=ps[:],
                    func=mybir.ActivationFunctionType.Exp,
                    bias=mb_sb[:, j:j + 1], scale=DP_SCALE)
                for u in slot_units.get(2 * j + half, []):
                    u()

    def op_mms(it, ps0, ps1, cts):
        for ct in cts:
            ot_blk = ot_sb[:, ct * N + it * 128: ct * N + (it + 1) * 128]
            nc.tensor.matmul(
                ps0[:], lhsT=ot_blk,
                rhs=wo_sb[:, ct * D: ct * D + 512],
                start=(ct == 0), stop=(ct == PAIRS - 1))
            nc.tensor.matmul(
                ps1[:], lhsT=ot_blk,
                rhs=wo_sb[:, ct * D + 512: ct * D + 1024],
                start=(ct == 0), stop=(ct == PAIRS - 1))

    def op_finish(it, ps0, ps1):
        # Copies split across DVE+ACT (both idle here). Early tiles store
        # whole on the idle Pool SWDGE path (separate issue queue, keeps
        # HWDGE clear); the last two tiles are latency-critical:
        # column-half stores on sync/scalar, each depending only on its
        # own engine's copy so neither store cross-waits the other engine.
        out_t = scp.tile([128, 1024], bf16, tag="outt")
        nc.vector.tensor_copy(out=out_t[:, 0:512], in_=ps0[:])
        nc.scalar.activation(out=out_t[:, 512:1024], in_=ps1[:],
                             func=mybir.ActivationFunctionType.Copy)
        r = slice(it * 128, (it + 1) * 128)
        if it < IT - 2:
            nc.gpsimd.dma_start(out=out[r, :], in_=out_t[:])
        elif it < IT - 1:
            # both halves on sync: a scalar-issued store here would hold
            # Activation.SEQ through HWDGE gen and delay the LAST tile's
            # ACT copy
            nc.sync.dma_start(out=out[r, 0:512], in_=out_t[:, 0:512])
            nc.sync.dma_start(out=out[r, 512:1024], in_=out_t[:, 512:1024])
        else:
            nc.sync.dma_start(out=out[r, 0:512], in_=out_t[:, 0:512])
            nc.scalar.dma_start(out=out[r, 512:1024],
                                in_=out_t[:, 512:1024])

    # ---- prologue: q0 then k0 only (minimum work before the ST stream
    # starts); q1-q3 stream in as pair-0/1 fillers as their weights land
    for u in proj_q(0, on_act=True):
        u()
    for u in proj_k(0, on_act=True):
        u()

    # ---- pipelined pairs ----
    prev = None
    for p in range(PAIRS):
        pa = pt_pool.tile([128, jt * N], bf16, tag="pa")
        pb = pt_pool.tile([128, jt * N], bf16, tag="pb")

        if p == 0:
            # DMA-paced fillers in arrival order: q1, q2, v tiles (wv),
            # q3 waits for pair 1, k1 last (its weights land last)
            proj_units = (proj_q(1) + proj_q(2)
                          + [lambda j=j: v_proj(j) for j in range(jt)]
                          + proj_k(1))
        elif p + 1 < PAIRS:
            proj_units = list(proj_k(p + 1))
            if p == 1:
                proj_units = proj_q(3) + proj_units
        else:
            proj_units = []
        if prev is not None:
            # av heads use the single "av" PSUM buffer: keep them apart so
            # head B's alloc never stalls the PE on head A's norms
            pp, ppa, ppb = prev
            na = len(proj_units) // 2
            units = ([lambda: av_head(pp, 0, ppa)] + proj_units[:na]
                     + [lambda: av_head(pp, 1, ppb)] + proj_units[na:])
        else:
            units = proj_units

        slot_units = {}
        nslots = 2 * jt
        # monotonic slot assignment keeps each proj's k0-half before its
        # k1-half (they share PSUM accumulators); +2 phantom units lean
        # the distribution toward late slots so the pair tail (when the
        # exp stream still drains) keeps PE fed
        for i, u in enumerate(units):
            slot_units.setdefault(
                min(nslots - 1, (i + 6) * nslots // (len(units) + 6)),
                []).append(u)
        st_pair(p, pa, pb, slot_units)
        prev = (p, pa, pb)

    # last pair's AV: head B first (single 'av' buffer), then head A from
    # the ST banks. The norms are hand-interleaved in column halves so the
    # DVE chain delivers the first 512 normalized query columns (both head
    # rows) as early as possible for the O-projection's ct=3 matmuls.
    pp, ppa, ppb = prev
    poB = av_head(pp, 1, ppb, skip_norm=True)
    recB_t = recp.tile([128, 1024], f32)
    nc.vector.reciprocal_approx_fast(out=recB_t[:], in_=poB[:])
    poA = av_head(pp, 0, ppa, pool_tag="st", skip_norm=True)
    recA_t = recp.tile([128, 1024], f32)
    for c in range(2):
        cs = slice(c * 512, (c + 1) * 512)
        oc = slice(pp * N + c * 512, pp * N + (c + 1) * 512)
        nc.vector.reciprocal_approx_fast(out=recA_t[:, cs], in_=poA[:, cs])
        # head B (V_ext=[V|ones]): O^T rows 0:64 -> ot rows 64:128
        nc.vector.tensor_mul(out=ot_sb[64:128, oc], in0=poB[0:64, cs],
                             in1=recB_t[64:128, cs])
        # head A (V_ext=[ones|V]): O^T rows 64:128 -> ot rows 0:64
        nc.vector.tensor_mul(out=ot_sb[0:64, oc], in0=poA[64:128, cs],
                             in1=recA_t[0:64, cs])

    # ---- output projection: partial[i, d] in bf16 ----
    # Runway: i-tiles 1-2 accumulate pairs 0-2 while the last norms run on
    # DVE; PSUM plan fills all 8 banks: AVB po (av, 2) + AVA po (st, 2) +
    # pend1 (st, 2) + pend2 (pj, 1+1). The O-proj loop then rotates
    # st/pj/av so tile allocs never wait on a two-deep copy pipeline.
    def op_psum(which):
        if which == "pj":
            o0 = smps.tile([128, 512], f32, name="o0", tag="pj")
            o1 = smps.tile([128, 512], f32, name="o1", tag="pj")
            return o0[:], o1[:]
        if which == "av":
            pw = smps.tile([128, 1024], f32, name="po", tag="av", bufs=1)
        else:
            pw = stps.tile([128, 1024], f32, name="pw", tag="st")
        return pw[:, 0:512], pw[:, 512:1024]

    pend = {}
    for it, pool in ((1, "st"), (2, "pj")):
        pend[it] = op_psum(pool)
        op_mms(it, *pend[it], range(PAIRS - 1))
    rot = ["av", "st", "st", "pj", "av", "st"]
    for it in range(IT):
        if it in pend:
            ps0, ps1 = pend[it]
            op_mms(it, ps0, ps1, range(PAIRS - 1, PAIRS))
        else:
            ps0, ps1 = op_psum(rot.pop(0))
            op_mms(it, ps0, ps1, range(PAIRS))
        op_finish(it, ps0, ps1)


_NC_CACHE = {}


def _get_nc(jt):
    if jt not in _NC_CACHE:
        _NC_CACHE[jt] = build_nc(jt=jt)
    return _NC_CACHE[jt]


def _tile_k(a, cols):
    """[KT*128, cols] -> [128, KT*cols] k-tile-major, contiguous bf16."""
    return np.ascontiguousarray(
        a.reshape(KT, 128, cols).transpose(1, 0, 2).reshape(128, KT * cols)
    ).astype(BF16)


def _make_in_maps(x_q, x_kv, pad_mask, Wq, Wk, Wv, Wo, jt=None):
    pad_mask = np.asarray(pad_mask)
    cnts = (~pad_mask).sum(axis=1)
    if jt is None:
        jt = max(1, int(-(-int(cnts.max()) // 128)))
    lc = jt * 128

    def _tile_pair(w):
        # [D, DG] -> [128, pair*KT*128 + k*128 + c] pair-major
        return np.ascontiguousarray(
            w.reshape(KT, 128, PAIRS, 128).transpose(1, 2, 0, 3)
            .reshape(128, PAIRS * KT * 128)).astype(BF16)

    per_g = []
    for g in range(2):
        cols = slice(g * DG, (g + 1) * DG)
        per_g.append({
            "wq": _tile_pair(np.ascontiguousarray(Wq[:, cols])),
            "wk": _tile_pair(np.ascontiguousarray(Wk[:, cols])),
            "wv": _tile_k(np.ascontiguousarray(Wv[:, cols]), DG),
            "wo": np.ascontiguousarray(
                Wo[g * DG:(g + 1) * DG, :]
                .reshape(PAIRS, 128, D).transpose(1, 0, 2)
                .reshape(128, PAIRS * D)).astype(BF16),
        })
    per_b = []
    for b in range(B):
        idx = np.flatnonzero(~pad_mask[b])
        n = len(idx)
        xc = np.zeros((lc, D), dtype=np.float32)
        xc[:n] = x_kv[b][idx]
        mbias = np.full(lc, MASK_NEG, dtype=np.float32)
        mbias[:n] = 0.0
        per_b.append({
            "xq": _tile_k(np.ascontiguousarray(x_q[b].T), N),
            "xkv": _tile_k(np.ascontiguousarray(xc.T), lc),
            "mb": np.ascontiguousarray(mbias.reshape(jt, 128).T),
        })

    in_maps = []
    for c in range(NCORES):
        b, g = c // 2, c % 2
        in_maps.append({**per_b[b], **per_g[g]})
    return in_maps, jt


def kernel(x_q, x_kv, pad_mask, Wq, Wk, Wv, Wo, bo):
    in_maps, jt = _make_in_maps(x_q, x_kv, pad_mask, Wq, Wk, Wv, Wo)
    nc = _get_nc(jt)
    res = run_bass_kernel_spmd(nc, in_maps, core_ids=list(range(NCORES)))
    full = np.empty((B, N, D), dtype=np.float32)
    bo32 = bo.astype(np.float32)
    for b in range(B):
        full[b] = (res.results[2 * b]["out"].astype(np.float32)
                   + res.results[2 * b + 1]["out"].astype(np.float32))
        full[b] += bo32
    return full


# revision 62
# speedup vs baseline: 1.0128x; 1.0094x over previous
"""Trainium2 Bass kernel for nn_MultiHeadAttention_37512244363503.

Sharding: 8 cores = 4 batches x 2 head-groups (8 heads each).
Per core (b, g): Wq/Wk/Wv column-sliced, Wo row-sliced; the host sums the
two partial outputs per batch (the row-parallel "all-reduce") and adds bo.

Key compaction: pad_mask is host-visible, so masked keys are dropped on
the host before upload. Keys compact to jt*128 columns (jt chosen from
the max per-batch unmasked count, 5 for the reference distribution),
cutting K/V projections, scores, softmax and AV by L_c/L. Padding slots
get a -30000 exp-bias so they contribute exactly 0.

All DRAM inputs are pre-tiled on host to the SBUF k-tile-major layout so
every load is a contiguous [128, cols] copy (no strided descriptors).

Per-core algorithm (matmuls bf16 in / fp32 PSUM accumulate):
  QT[d,i]   = Wq_g.T @ x_q[b].T        (d=512 cols of this group)
  KT[d,j]   = Wk_g.T @ x_kv_c[b].T     (j over compacted keys)
  V[j,c]    = x_kv_c[b] @ Wv_g         (per-head [ones|V_h] / [V_h|ones])
  ST[j,i]   = K_h Q_h.T per head       (2 heads packed via PE row groups)
  PT        = exp(ST/8 + mask_bias[j]) (ACT; bias rides the ACT bias input)
  po        = V_ext.T @ PT             (denom + O^T in one matmul)
  O_norm^T  = po_OT * recip(po_denom)  (DVE; written straight into ot_sb)
  partial   = O_norm @ Wo_g            (bf16 partial -> DRAM, host sums)

Norms write directly into ot_sb (no SBUF->SBUF DMA): head A (hh=0) keeps
V_ext = [ones|V] so denom sits at PSUM rows 0:64 and O^T at 64:128; the
DVE mul shifts the PSUM operand down to write ot rows 0:64. Head B flips
V_ext = [V|ones] so its mul writes ot rows 64:128 with the SBUF operands
(recip, out) partition-aligned at 64. The fast reciprocal always reads
the full [128,*] PSUM tile base-aligned (unused rows are garbage but
never read; the custom DVE op corrupts on shifted APs, so base-aligned
full-height is the only safe form).
"""

import numpy as np
import ml_dtypes

import concourse.bass as bass
import concourse.mybir as mybir
from concourse import bacc
from concourse.tile import TileContext
from concourse.bass_utils import run_bass_kernel_spmd

BF16 = ml_dtypes.bfloat16

B, N, L, D, H = 4, 1024, 1024, 1024, 16
DH = D // H           # 64 channels per head
HG = 8                # heads per core
DG = HG * DH          # 512 channels per core
NCORES = 8
DP_SCALE = DH ** -0.5
MASK_NEG = -30000.0   # exp(x + MASK_NEG) underflows to exactly 0.0

f32 = mybir.dt.float32
bf16 = mybir.dt.bfloat16

KT = D // 128         # 8 k-tiles in the contraction dim of projections
IT = N // 128         # 8 query tiles
IC = N // 512         # 2 query chunks (PSUM free dim)
PAIRS = HG // 2       # 4 head pairs (2 heads packed per 128 partitions)
VW = 2 * DH           # 128 cols per (j, head) V_ext block


def build_nc(jt=5, debug=False, num_devices=NCORES, repeat=1):
    lc = jt * 128
    nc = bacc.Bacc("TRN2", target_bir_lowering=False, debug=False,
                   num_devices=num_devices)

    xq = nc.dram_tensor("xq", [128, KT * N], bf16, kind="ExternalInput")
    xkv = nc.dram_tensor("xkv", [128, KT * lc], bf16, kind="ExternalInput")
    # wq/wk are PAIR-major ([128, pair*KT*128 + k*128 + c]) so one pair's
    # projection weights are a single contiguous 256KB slice loadable first
    wq = nc.dram_tensor("wq", [128, KT * DG], bf16, kind="ExternalInput")
    wk = nc.dram_tensor("wk", [128, KT * DG], bf16, kind="ExternalInput")
    wv = nc.dram_tensor("wv", [128, KT * DG], bf16, kind="ExternalInput")
    wo = nc.dram_tensor("wo", [128, PAIRS * D], bf16, kind="ExternalInput")
    mb = nc.dram_tensor("mb", [128, jt], f32, kind="ExternalInput")
    out = nc.dram_tensor("out", [N, D], bf16, kind="ExternalOutput")

    with TileContext(nc) as tc:
        with (
            tc.tile_pool(name="persist", bufs=1) as persist,
            tc.tile_pool(name="pt", bufs=3) as pt_pool,
            tc.tile_pool(name="recp", bufs=4) as recp,
            tc.tile_pool(name="scp", bufs=4) as scp,
            tc.tile_pool(name="stps", bufs=2, space="PSUM") as stps,
            tc.tile_pool(name="smps", bufs=2, space="PSUM") as smps,
        ):
            env = dict(jt=jt, lc=lc)
            for nm, shape in [
                ("xq_sb", [128, KT * N]), ("xkv_sb", [128, KT * lc]),
                ("wq_sb", [128, KT * DG]), ("wk_sb", [128, KT * DG]),
                ("wv_sb", [128, KT * DG]), ("wo_sb", [128, PAIRS * D]),
                ("qT_sb", [128, PAIRS * N]), ("kT_sb", [128, PAIRS * lc]),
                ("v_sb", [128, jt * HG * VW]), ("ot_sb", [128, PAIRS * N]),
            ]:
                env[nm] = persist.tile(shape, bf16, name=nm)
            env["mb_sb"] = persist.tile([128, jt], f32, name="mb_sb")
            env["v_view"] = env["v_sb"][:].rearrange(
                "p (j h c) -> p j h c", j=jt, h=HG)
            # Per-head V_ext layout: head A (even h) = [ones | V_h] so the
            # softmax denominator lands at PSUM rows 0:64 (base-aligned for
            # the custom reciprocal) and O^T at 64:128 (the shiftable PSUM
            # mul operand, written to ot rows 0:64). Head B (odd h) =
            # [V_h | ones]: O^T at rows 0:64 shifts down into ot rows
            # 64:128 with recip/out SBUF operands aligned at 64.
            nc.vector.memset(env["v_view"][:, :, 0::2, 0:DH], 1.0)
            nc.vector.memset(env["v_view"][:, :, 1::2, DH:VW], 1.0)
            env.update(pt_pool=pt_pool, recp=recp, scp=scp, stps=stps,
                       smps=smps, xq=xq, xkv=xkv, wq=wq, wk=wk, wv=wv,
                       wo=wo, mb=mb, out=out)
            for _rep in range(repeat):
                _emit_body(nc, env)

    nc.compile()
    return nc


def _emit_body(nc, env):
    jt, lc = env["jt"], env["lc"]
    xq_sb, xkv_sb = env["xq_sb"], env["xkv_sb"]
    wq_sb, wk_sb, wv_sb, wo_sb = (env[k] for k in
                                  ["wq_sb", "wk_sb", "wv_sb", "wo_sb"])
    qT_sb, kT_sb, v_sb, ot_sb, mb_sb = (env[k] for k in
                                        ["qT_sb", "kT_sb", "v_sb", "ot_sb",
                                         "mb_sb"])
    v_view = env["v_view"]
    pt_pool, recp, scp = env["pt_pool"], env["recp"], env["scp"]
    stps, smps = env["stps"], env["smps"]
    xq, xkv, wq, wk, wv, wo, mb, out = (env[k] for k in
                                        ["xq", "xkv", "wq", "wk", "wv",
                                         "wo", "mb", "out"])

    # ---- input loads ----
    # Loads are ordered by first PE use: xq k0 + pair-0 Q weights, the
    # rest of xq (k-tile streamed under the q0 projection), pair-0 K
    # weights, xkv, then the filler-unit inputs (wq pairs 1-3, wv, wk
    # pairs 1-3) and finally wo. Three rules learned from the timeline:
    # (1) the DMA transfer queue drains in request order, so loads must
    # reach it strictly in consumption order -- a lower-priority load on
    # a faster-clearing queue cuts the line and delays ST p0;
    # (2) ALL loads go on sync: a dma_start holds its issuing engine's
    # SEQ through the serialized shared HWDGE descriptor-gen stage, so
    # scalar-issued loads would block the prologue ACT copies (which hold
    # the pj PSUM ring!) and the exp stream behind ~8us of queued issues;
    # (3) gpsimd's SWDGE path (1us serialized issue, separate queue that
    # would jump the HWDGE stream) only carries the tiny mb load.
    def ld_split(dst, src, c0, c1, nsplit):
        step = 128 // nsplit
        for s in range(nsplit):
            p0, p1 = s * step, (s + 1) * step
            nc.sync.dma_start(out=dst[p0:p1, c0:c1], in_=src[p0:p1, c0:c1])

    nc.gpsimd.dma_start(out=mb_sb[:], in_=mb[:, :])
    PW = KT * 128  # cols per pair in the pair-major wq/wk layouts
    ld_split(xq_sb, xq, 0, N, 1)
    ld_split(wq_sb, wq, 0, 256, 1)         # 64KB: q0 k0/k1 matmul deps
    ld_split(wq_sb, wq, 256, PW, 1)
    for k in range(1, KT):
        ld_split(xq_sb, xq, k * N, (k + 1) * N, 1)
    ld_split(wk_sb, wk, 0, PW, 1)
    qx = KT * lc // 4
    for s in range(4):
        ld_split(xkv_sb, xkv, s * qx, (s + 1) * qx, 1)
    ld_split(wq_sb, wq, PW, 2 * PW, 1)     # q1 (early pair-0 filler)
    ld_split(wq_sb, wq, 2 * PW, 3 * PW, 1)  # q2
    ld_split(wv_sb, wv, 0, KT * DG, 2)      # v_proj fillers (pair 0)
    ld_split(wk_sb, wk, PW, 2 * PW, 1)      # k1 (pair-0 last filler)
    ld_split(wq_sb, wq, 3 * PW, 4 * PW, 1)  # q3 (pair-1 filler)
    ld_split(wk_sb, wk, 2 * PW, 4 * PW, 2)
    ld_split(wo_sb, wo, 0, PAIRS * D, 2)

    # k-proj output chunking (lc may exceed one PSUM bank)
    kch = [(0, min(512, lc))] + ([(512, lc)] if lc > 512 else [])

    def make_proj(dst_sb, w_sb, x_sb, p, cols, chunks, on_act=False):
        """Projection for pair p as two units sharing PSUM accumulators.

        chunks: list of (c0, c1) output-column ranges (<=512 wide each).
        on_act: do the PSUM->SBUF copies on the idle ACT engine (prologue
        projections), keeping the DVE stream free.
        """
        ps = [None] * len(chunks)

        def half(k0, k1):
            for ci, (c0, c1) in enumerate(chunks):
                if k0 == 0:
                    ps[ci] = smps.tile([128, c1 - c0], f32,
                                       name="pj%d" % ci, tag="pj")
            for k in range(k0, k1):
                w = w_sb[:, (p * KT + k) * 128: (p * KT + k + 1) * 128]
                for ci, (c0, c1) in enumerate(chunks):
                    nc.tensor.matmul(
                        ps[ci][:],
                        lhsT=w,
                        rhs=x_sb[:, k * cols + c0: k * cols + c1],
                        start=(k == 0), stop=(k == KT - 1))
            if k1 == KT:
                for ci, (c0, c1) in enumerate(chunks):
                    dst = dst_sb[:, p * cols + c0: p * cols + c1]
                    if on_act:
                        nc.scalar.activation(
                            out=dst, in_=ps[ci][:],
                            func=mybir.ActivationFunctionType.Copy)
                    else:
                        nc.vector.tensor_copy(out=dst, in_=ps[ci][:])

        return [lambda: half(0, KT // 2), lambda: half(KT // 2, KT)]

    def proj_q(p, on_act=False):
        return make_proj(qT_sb, wq_sb, xq_sb, p, N, [(0, 512), (512, 1024)],
                         on_act)

    def proj_k(p, on_act=False):
        return make_proj(kT_sb, wk_sb, xkv_sb, p, lc, kch, on_act)

    def v_proj(j):
        """V[j, c] = x_kv @ Wv_g for one j tile (interleaved dst halves)."""
        ps = smps.tile([128, 512], f32, tag="av", bufs=1)
        for k in range(KT):
            nc.tensor.matmul(
                ps[:],
                lhsT=xkv_sb[:, k * lc + j * 128: k * lc + (j + 1) * 128],
                rhs=wv_sb[:, k * DG:(k + 1) * DG],
                start=(k == 0), stop=(k == KT - 1))
        pv = ps[:].rearrange("p (h c) -> p h c", h=HG)
        # head A (even) V goes to cols DH:VW, head B (odd) to cols 0:DH
        nc.vector.tensor_copy(out=v_view[:, j, 0::2, DH:VW], in_=pv[:, 0::2])
        nc.vector.tensor_copy(out=v_view[:, j, 1::2, 0:DH], in_=pv[:, 1::2])

    def norm(po, p, hh, chunks=1, rec_act=False):
        """Normalize one head's AV straight into ot_sb (no DMA).

        hh=0 (head A, V_ext=[ones|V]): denom rows 0:64, O^T rows 64:128;
        mul shifts the PSUM operand down into ot rows 0:64.
        hh=1 (head B, V_ext=[V|ones]): O^T rows 0:64 shifts up into ot
        rows 64:128; recip/out SBUF operands aligned at partition 64.
        The reciprocal runs full-height base-aligned; the unused half is
        garbage (1/O^T values) that is never read. denom > ~1 always, so
        the fast approx reciprocal's denorm/inf edge cases can't occur
        on the rows that are consumed.
        """
        rec_t = recp.tile([128, 1024], f32)
        dn, ot_rows = ((slice(0, 64), slice(64, 128)) if hh == 0
                       else (slice(64, 128), slice(0, 64)))
        step = 1024 // chunks
        for c in range(chunks):
            cs = slice(c * step, (c + 1) * step)
            nc.vector.reciprocal_approx_fast(out=rec_t[:, cs],
                                             in_=po[:, cs])
            nc.vector.tensor_mul(
                out=ot_sb[dn, p * N + c * step: p * N + (c + 1) * step],
                in0=po[ot_rows, cs], in1=rec_t[dn, cs])

    def av_head(p, hh, pt, pool_tag="av", chunks=1, rec_act=False,
                skip_norm=False):
        """AV for head 2p+hh, both i chunks, one V weight load per j."""
        st = {}
        for u in av_head_units(p, hh, pt, pool_tag, chunks, rec_act, 1,
                               skip_norm, st):
            u()
        return st.get("po")

    def av_head_units(p, hh, pt, pool_tag="av", chunks=1, rec_act=False,
                      nunits=2, skip_norm=False, state=None):
        """AV for one head as nunits filler units (j-ranges + final norm)."""
        h = 2 * p + hh
        if state is None:
            state = {}

        def run(j0, j1):
            if j0 == 0:
                state["po"] = (
                    stps.tile([128, 1024], f32, name="po", tag="st")
                    if pool_tag == "st" else
                    smps.tile([128, 1024], f32, name="po", tag="av", bufs=1))
            po = state["po"]
            for j in range(j0, j1):
                vblk = v_sb[:, (j * HG + h) * VW: (j * HG + h + 1) * VW]
                nc.tensor.matmul(po[:, 0:512], lhsT=vblk,
                                 rhs=pt[:, j * N: j * N + 512],
                                 start=(j == 0), stop=(j == jt - 1))
                nc.tensor.matmul(po[:, 512:1024], lhsT=vblk,
                                 rhs=pt[:, j * N + 512: (j + 1) * N],
                                 start=(j == 0), stop=(j == jt - 1))
            if j1 == jt and not skip_norm:
                norm(po, p, hh, chunks=chunks, rec_act=rec_act)

        bounds = [jt * i // nunits for i in range(nunits + 1)]
        return [lambda a=a, b=b: run(a, b)
                for a, b in zip(bounds[:-1], bounds[1:])]

    def st_pair(p, pa, pb, slot_units):
        """ST + exp for pair p; filler units interleaved into HALF-j slots
        (2*jt of them) so the exp stream is fed a fresh ST half roughly
        every exp-duration instead of in bursts."""
        for j in range(jt):
            for half, (rb, pt) in enumerate(((0, pa), (64, pb))):
                ps = stps.tile([128, 1024], f32, name="st", tag="st")
                kk = kT_sb[rb:rb + 64, p * lc + j * 128: p * lc + (j + 1) * 128]
                for ic in range(IC):
                    cols = slice(ic * 512, ic * 512 + 512)
                    nc.tensor.matmul(
                        ps[:, cols], lhsT=kk,
                        rhs=qT_sb[rb:rb + 64,
                                  p * N + ic * 512: p * N + ic * 512 + 512],
                        start=True, stop=True)
                # EXP right after this half's matmuls: ACT starts earlier
                # and the pool slot frees a half-j sooner
                nc.scalar.activation(
                    out=pt[:, j * N:(j + 1) * N], in_=ps[:],
                    func=mybir.ActivationFunctionType.Exp,
                    bias=mb_sb[:, j:j + 1], scale=DP_SCALE)
                for u in slot_units.get(2 * j + half, []):
                    u()

    def op_mms(it, ps0, ps1, cts):
        for ct in cts:
            ot_blk = ot_sb[:, ct * N + it * 128: ct * N + (it + 1) * 128]
            nc.tensor.matmul(
                ps0[:], lhsT=ot_blk,
                rhs=wo_sb[:, ct * D: ct * D + 512],
                start=(ct == 0), stop=(ct == PAIRS - 1))
            nc.tensor.matmul(
                ps1[:], lhsT=ot_blk,
                rhs=wo_sb[:, ct * D + 512: ct * D + 1024],
                start=(ct == 0), stop=(ct == PAIRS - 1))

    def op_finish(it, ps0, ps1):
        # Copies split across DVE+ACT (both idle here). Early tiles store
        # whole on the idle Pool SWDGE path (separate issue queue, keeps
        # HWDGE clear); the last two tiles are latency-critical:
        # column-half stores on sync/scalar, each depending only on its
        # own engine's copy so neither store cross-waits the other engine.
        out_t = scp.tile([128, 1024], bf16, tag="outt")
        nc.vector.tensor_copy(out=out_t[:, 0:512], in_=ps0[:])
        nc.scalar.activation(out=out_t[:, 512:1024], in_=ps1[:],
                             func=mybir.ActivationFunctionType.Copy)
        r = slice(it * 128, (it + 1) * 128)
        if it < IT - 2:
            nc.gpsimd.dma_start(out=out[r, :], in_=out_t[:])
        elif it < IT - 1:
            # both halves on sync: a scalar-issued store here would hold
            # Activation.SEQ through HWDGE gen and delay the LAST tile's
            # ACT copy
            nc.sync.dma_start(out=out[r, 0:512], in_=out_t[:, 0:512])
            nc.sync.dma_start(out=out[r, 512:1024], in_=out_t[:, 512:1024])
        else:
            nc.sync.dma_start(out=out[r, 0:512], in_=out_t[:, 0:512])
            nc.scalar.dma_start(out=out[r, 512:1024],
                                in_=out_t[:, 512:1024])

    # ---- prologue: q0 then k0 only (minimum work before the ST stream
    # starts); q1-q3 stream in as pair-0/1 fillers as their weights land
    for u in proj_q(0, on_act=True):
        u()
    for u in proj_k(0, on_act=True):
        u()

    # ---- pipelined pairs ----
    prev = None
    for p in range(PAIRS):
        pa = pt_pool.tile([128, jt * N], bf16, tag="pa")
        pb = pt_pool.tile([128, jt * N], bf16, tag="pb")

        if p == 0:
            # DMA-paced fillers in arrival order: q1, q2, v tiles (wv),
            # q3 waits for pair 1, k1 last (its weights land last)
            proj_units = (proj_q(1) + proj_q(2)
                          + [lambda j=j: v_proj(j) for j in range(jt)]
                          + proj_k(1))
        elif p + 1 < PAIRS:
            proj_units = list(proj_k(p + 1))
            if p == 1:
                proj_units = proj_q(3) + proj_units
        else:
            proj_units = []
        if prev is not None:
            # av heads use the single "av" PSUM buffer: keep them apart so
            # head B's alloc never stalls the PE on head A's norms
            pp, ppa, ppb = prev
            na = len(proj_units) // 2
            units = ([lambda: av_head(pp, 0, ppa)] + proj_units[:na]
                     + [lambda: av_head(pp, 1, ppb)] + proj_units[na:])
        else:
            units = proj_units

        slot_units = {}
        nslots = 2 * jt
        # monotonic slot assignment keeps each proj's k0-half before its
        # k1-half (they share PSUM accumulators); +2 phantom units lean
        # the distribution toward late slots so the pair tail (when the
        # exp stream still drains) keeps PE fed
        for i, u in enumerate(units):
            slot_units.setdefault(
                min(nslots - 1, (i + 6) * nslots // (len(units) + 6)),
                []).append(u)
        st_pair(p, pa, pb, slot_units)
        prev = (p, pa, pb)

    # last pair's AV: head B first (single 'av' buffer), then head A from
    # the ST banks. The norms are hand-interleaved in column halves so the
    # DVE chain delivers the first 512 normalized query columns (both head
    # rows) as early as possible for the O-projection's ct=3 matmuls.
    pp, ppa, ppb = prev
    poB = av_head(pp, 1, ppb, skip_norm=True)
    recB_t = recp.tile([128, 1024], f32)
    nc.vector.reciprocal_approx_fast(out=recB_t[:], in_=poB[:])
    poA = av_head(pp, 0, ppa, pool_tag="st", skip_norm=True)
    recA_t = recp.tile([128, 1024], f32)
    for c in range(2):
        cs = slice(c * 512, (c + 1) * 512)
        oc = slice(pp * N + c * 512, pp * N + (c + 1) * 512)
        nc.vector.reciprocal_approx_fast(out=recA_t[:, cs], in_=poA[:, cs])
        # head B (V_ext=[V|ones]): O^T rows 0:64 -> ot rows 64:128
        nc.vector.tensor_mul(out=ot_sb[64:128, oc], in0=poB[0:64, cs],
                             in1=recB_t[64:128, cs])
        # head A (V_ext=[ones|V]): O^T rows 64:128 -> ot rows 0:64
        nc.vector.tensor_mul(out=ot_sb[0:64, oc], in0=poA[64:128, cs],
                             in1=recA_t[0:64, cs])

    # ---- output projection: partial[i, d] in bf16 ----
    # Runway: i-tiles 1-2 accumulate pairs 0-2 while the last norms run on
    # DVE; PSUM plan fills all 8 banks: AVB po (av, 2) + AVA po (st, 2) +
    # pend1 (st, 2) + pend2 (pj, 1+1). The O-proj loop then rotates
    # st/pj/av so tile allocs never wait on a two-deep copy pipeline.
    def op_psum(which):
        if which == "pj":
            o0 = smps.tile([128, 512], f32, name="o0", tag="pj")
            o1 = smps.tile([128, 512], f32, name="o1", tag="pj")
            return o0[:], o1[:]
        if which == "av":
            pw = smps.tile([128, 1024], f32, name="po", tag="av", bufs=1)
        else:
            pw = stps.tile([128, 1024], f32, name="pw", tag="st")
        return pw[:, 0:512], pw[:, 512:1024]

    pend = {}
    for it, pool in ((1, "st"), (2, "pj")):
        pend[it] = op_psum(pool)
        op_mms(it, *pend[it], range(PAIRS - 1))
    rot = ["av", "st", "st", "pj", "av", "st"]
    for it in range(IT):
        if it in pend:
            ps0, ps1 = pend[it]
            op_mms(it, ps0, ps1, range(PAIRS - 1, PAIRS))
        else:
            ps0, ps1 = op_psum(rot.pop(0))
            op_mms(it, ps0, ps1, range(PAIRS))
        op_finish(it, ps0, ps1)


_NC_CACHE = {}


def _get_nc(jt):
    if jt not in _NC_CACHE:
        _NC_CACHE[jt] = build_nc(jt=jt)
    return _NC_CACHE[jt]


def _tile_k(a, cols):
    """[KT*128, cols] -> [128, KT*cols] k-tile-major, contiguous bf16."""
    return np.ascontiguousarray(
        a.reshape(KT, 128, cols).transpose(1, 0, 2).reshape(128, KT * cols)
    ).astype(BF16)


def _make_in_maps(x_q, x_kv, pad_mask, Wq, Wk, Wv, Wo, jt=None):
    pad_mask = np.asarray(pad_mask)
    cnts = (~pad_mask).sum(axis=1)
    if jt is None:
        jt = max(1, int(-(-int(cnts.max()) // 128)))
    lc = jt * 128

    def _tile_pair(w):
        # [D, DG] -> [128, pair*KT*128 + k*128 + c] pair-major
        return np.ascontiguousarray(
            w.reshape(KT, 128, PAIRS, 128).transpose(1, 2, 0, 3)
            .reshape(128, PAIRS * KT * 128)).astype(BF16)

    per_g = []
    for g in range(2):
        cols = slice(g * DG, (g + 1) * DG)
        per_g.append({
            "wq": _tile_pair(np.ascontiguousarray(Wq[:, cols])),
            "wk": _tile_pair(np.ascontiguousarray(Wk[:, cols])),
            "wv": _tile_k(np.ascontiguousarray(Wv[:, cols]), DG),
            "wo": np.ascontiguousarray(
                Wo[g * DG:(g + 1) * DG, :]
                .reshape(PAIRS, 128, D).transpose(1, 0, 2)
                .reshape(128, PAIRS * D)).astype(BF16),
        })
    per_b = []
    for b in range(B):
        idx = np.flatnonzero(~pad_mask[b])
        n = len(idx)
        xc = np.zeros((lc, D), dtype=np.float32)
        xc[:n] = x_kv[b][idx]
        mbias = np.full(lc, MASK_NEG, dtype=np.float32)
        mbias[:n] = 0.0
        per_b.append({
            "xq": _tile_k(np.ascontiguousarray(x_q[b].T), N),
            "xkv": _tile_k(np.ascontiguousarray(xc.T), lc),
            "mb": np.ascontiguousarray(mbias.reshape(jt, 128).T),
        })

    in_maps = []
    for c in range(NCORES):
        b, g = c // 2, c % 2
        in_maps.append({**per_b[b], **per_g[g]})
    return in_maps, jt


def kernel(x_q, x_kv, pad_mask, Wq, Wk, Wv, Wo, bo):
    in_maps, jt = _make_in_maps(x_q, x_kv, pad_mask, Wq, Wk, Wv, Wo)
    nc = _get_nc(jt)
    res = run_bass_kernel_spmd(nc, in_maps, core_ids=list(range(NCORES)))
    full = np.empty((B, N, D), dtype=np.float32)
    bo32 = bo.astype(np.float32)
    for b in range(B):
        full[b] = (res.results[2 * b]["out"].astype(np.float32)
                   + res.results[2 * b + 1]["out"].astype(np.float32))
        full[b] += bo32
    return full


# revision 75
# speedup vs baseline: 1.0186x; 1.0058x over previous
"""Trainium2 Bass kernel for nn_MultiHeadAttention_37512244363503.

Sharding: 8 cores = 4 batches x 2 head-groups (8 heads each).
Per core (b, g): Wq/Wk/Wv column-sliced, Wo row-sliced; the host sums the
two partial outputs per batch (the row-parallel "all-reduce") and adds bo.

Key compaction: pad_mask is host-visible, so masked keys are dropped on
the host before upload. Keys compact to jt*128 columns (jt chosen from
the max per-batch unmasked count, 5 for the reference distribution),
cutting K/V projections, scores, softmax and AV by L_c/L. Padding slots
get a -30000 exp-bias so they contribute exactly 0.

All DRAM inputs are pre-tiled on host to the SBUF k-tile-major layout so
every load is a contiguous [128, cols] copy (no strided descriptors).

Per-core algorithm (matmuls bf16 in / fp32 PSUM accumulate):
  QT[d,i]   = Wq_g.T @ x_q[b].T        (d=512 cols of this group)
  KT[d,j]   = Wk_g.T @ x_kv_c[b].T     (j over compacted keys)
  V[j,c]    = x_kv_c[b] @ Wv_g         (per-head [ones|V_h] / [V_h|ones])
  ST[j,i]   = K_h Q_h.T per head       (2 heads packed via PE row groups)
  PT        = exp(ST/8 + mask_bias[j]) (ACT; bias rides the ACT bias input)
  po        = V_ext.T @ PT             (denom + O^T in one matmul)
  O_norm^T  = po_OT * recip(po_denom)  (DVE; written straight into ot_sb)
  partial   = O_norm @ Wo_g            (bf16 partial -> DRAM, host sums)

Norms write directly into ot_sb (no SBUF->SBUF DMA): head A (hh=0) keeps
V_ext = [ones|V] so denom sits at PSUM rows 0:64 and O^T at 64:128; the
DVE mul shifts the PSUM operand down to write ot rows 0:64. Head B flips
V_ext = [V|ones] so its mul writes ot rows 64:128 with the SBUF operands
(recip, out) partition-aligned at 64. The fast reciprocal always reads
the full [128,*] PSUM tile base-aligned (unused rows are garbage but
never read; the custom DVE op corrupts on shifted APs, so base-aligned
full-height is the only safe form).
"""

import numpy as np
import ml_dtypes

import concourse.bass as bass
import concourse.mybir as mybir
from concourse import bacc
from concourse.tile import TileContext
from concourse.bass_utils import run_bass_kernel_spmd

BF16 = ml_dtypes.bfloat16

B, N, L, D, H = 4, 1024, 1024, 1024, 16
DH = D // H           # 64 channels per head
HG = 8                # heads per core
DG = HG * DH          # 512 channels per core
NCORES = 8
DP_SCALE = DH ** -0.5
MASK_NEG = -30000.0   # exp(x + MASK_NEG) underflows to exactly 0.0

f32 = mybir.dt.float32
bf16 = mybir.dt.bfloat16

KT = D // 128         # 8 k-tiles in the contraction dim of projections
IT = N // 128         # 8 query tiles
IC = N // 512         # 2 query chunks (PSUM free dim)
PAIRS = HG // 2       # 4 head pairs (2 heads packed per 128 partitions)
VW = 2 * DH           # 128 cols per (j, head) V_ext block


def build_nc(jt=5, debug=False, num_devices=NCORES, repeat=1):
    lc = jt * 128
    nc = bacc.Bacc("TRN2", target_bir_lowering=False, debug=False,
                   num_devices=num_devices)

    xq = nc.dram_tensor("xq", [128, KT * N], bf16, kind="ExternalInput")
    xkv = nc.dram_tensor("xkv", [128, KT * lc], bf16, kind="ExternalInput")
    # wq/wk are PAIR-major ([128, pair*KT*128 + k*128 + c]) so one pair's
    # projection weights are a single contiguous 256KB slice loadable first
    wq = nc.dram_tensor("wq", [128, KT * DG], bf16, kind="ExternalInput")
    wk = nc.dram_tensor("wk", [128, KT * DG], bf16, kind="ExternalInput")
    wv = nc.dram_tensor("wv", [128, KT * DG], bf16, kind="ExternalInput")
    wo = nc.dram_tensor("wo", [128, PAIRS * D], bf16, kind="ExternalInput")
    mb = nc.dram_tensor("mb", [128, jt], f32, kind="ExternalInput")
    out = nc.dram_tensor("out", [N, D], bf16, kind="ExternalOutput")

    with TileContext(nc) as tc:
        with (
            tc.tile_pool(name="persist", bufs=1) as persist,
            tc.tile_pool(name="pt", bufs=3) as pt_pool,
            tc.tile_pool(name="recp", bufs=4) as recp,
            tc.tile_pool(name="scp", bufs=5) as scp,
            tc.tile_pool(name="stps", bufs=2, space="PSUM") as stps,
            tc.tile_pool(name="smps", bufs=2, space="PSUM") as smps,
        ):
            env = dict(jt=jt, lc=lc)
            for nm, shape in [
                ("xq_sb", [128, KT * N]), ("xkv_sb", [128, KT * lc]),
                ("wq_sb", [128, KT * DG]), ("wk_sb", [128, KT * DG]),
                ("wv_sb", [128, KT * DG]), ("wo_sb", [128, PAIRS * D]),
                ("qT_sb", [128, PAIRS * N]), ("kT_sb", [128, PAIRS * lc]),
                ("v_sb", [128, jt * HG * VW]), ("ot_sb", [128, PAIRS * N]),
            ]:
                env[nm] = persist.tile(shape, bf16, name=nm)
            env["mb_sb"] = persist.tile([128, jt], f32, name="mb_sb")
            env["v_view"] = env["v_sb"][:].rearrange(
                "p (j h c) -> p j h c", j=jt, h=HG)
            # Per-head V_ext layout: head A (even h) = [ones | V_h] so the
            # softmax denominator lands at PSUM rows 0:64 (base-aligned for
            # the custom reciprocal) and O^T at 64:128 (the shiftable PSUM
            # mul operand, written to ot rows 0:64). Head B (odd h) =
            # [V_h | ones]: O^T at rows 0:64 shifts down into ot rows
            # 64:128 with recip/out SBUF operands aligned at 64.
            nc.vector.memset(env["v_view"][:, :, 0::2, 0:DH], 1.0)
            nc.vector.memset(env["v_view"][:, :, 1::2, DH:VW], 1.0)
            env.update(pt_pool=pt_pool, recp=recp, scp=scp, stps=stps,
                       smps=smps, xq=xq, xkv=xkv, wq=wq, wk=wk, wv=wv,
                       wo=wo, mb=mb, out=out)
            for _rep in range(repeat):
                _emit_body(nc, env)

    nc.compile()
    return nc


def _emit_body(nc, env):
    jt, lc = env["jt"], env["lc"]
    xq_sb, xkv_sb = env["xq_sb"], env["xkv_sb"]
    wq_sb, wk_sb, wv_sb, wo_sb = (env[k] for k in
                                  ["wq_sb", "wk_sb", "wv_sb", "wo_sb"])
    qT_sb, kT_sb, v_sb, ot_sb, mb_sb = (env[k] for k in
                                        ["qT_sb", "kT_sb", "v_sb", "ot_sb",
                                         "mb_sb"])
    v_view = env["v_view"]
    pt_pool, recp, scp = env["pt_pool"], env["recp"], env["scp"]
    stps, smps = env["stps"], env["smps"]
    xq, xkv, wq, wk, wv, wo, mb, out = (env[k] for k in
                                        ["xq", "xkv", "wq", "wk", "wv",
                                         "wo", "mb", "out"])

    # ---- input loads ----
    # Loads are ordered by first PE use: xq k0 + pair-0 Q weights, the
    # rest of xq (k-tile streamed under the q0 projection), pair-0 K
    # weights, xkv, then the filler-unit inputs (wq pairs 1-3, wv, wk
    # pairs 1-3) and finally wo. Three rules learned from the timeline:
    # (1) the DMA transfer queue drains in request order, so loads must
    # reach it strictly in consumption order -- a lower-priority load on
    # a faster-clearing queue cuts the line and delays ST p0;
    # (2) ALL loads go on sync: a dma_start holds its issuing engine's
    # SEQ through the serialized shared HWDGE descriptor-gen stage, so
    # scalar-issued loads would block the prologue ACT copies (which hold
    # the pj PSUM ring!) and the exp stream behind ~8us of queued issues;
    # (3) gpsimd's SWDGE path (1us serialized issue, separate queue that
    # would jump the HWDGE stream) only carries the tiny mb load.
    def ld_split(dst, src, c0, c1, nsplit):
        step = 128 // nsplit
        for s in range(nsplit):
            p0, p1 = s * step, (s + 1) * step
            nc.sync.dma_start(out=dst[p0:p1, c0:c1], in_=src[p0:p1, c0:c1])

    nc.gpsimd.dma_start(out=mb_sb[:], in_=mb[:, :])
    PW = KT * 128  # cols per pair in the pair-major wq/wk layouts
    ld_split(xq_sb, xq, 0, N, 1)
    ld_split(wq_sb, wq, 0, 256, 1)         # 64KB: q0 k0/k1 matmul deps
    ld_split(wq_sb, wq, 256, PW, 1)
    for k in range(1, KT):
        ld_split(xq_sb, xq, k * N, (k + 1) * N, 1)
    ld_split(wk_sb, wk, 0, PW, 1)
    qx = KT * lc // 4
    for s in range(4):
        ld_split(xkv_sb, xkv, s * qx, (s + 1) * qx, 1)
    ld_split(wq_sb, wq, PW, 2 * PW, 1)     # q1 (early pair-0 filler)
    ld_split(wv_sb, wv, 0, KT * DG, 2)      # v_proj fillers (pair 0)
    ld_split(wq_sb, wq, 2 * PW, 3 * PW, 1)  # q2
    ld_split(wk_sb, wk, PW, 2 * PW, 1)      # k1 (pair-0 last filler)
    ld_split(wq_sb, wq, 3 * PW, 4 * PW, 1)  # q3 (pair-1 filler)
    ld_split(wk_sb, wk, 2 * PW, 4 * PW, 2)
    ld_split(wo_sb, wo, 0, PAIRS * D, 2)

    # k-proj output chunking (lc may exceed one PSUM bank)
    kch = [(0, min(512, lc))] + ([(512, lc)] if lc > 512 else [])

    def make_proj(dst_sb, w_sb, x_sb, p, cols, chunks, on_act=False):
        """Projection for pair p as two units sharing PSUM accumulators.

        chunks: list of (c0, c1) output-column ranges (<=512 wide each).
        on_act: do the PSUM->SBUF copies on the idle ACT engine (prologue
        projections), keeping the DVE stream free.
        """
        ps = [None] * len(chunks)

        def half(k0, k1):
            for ci, (c0, c1) in enumerate(chunks):
                if k0 == 0:
                    ps[ci] = smps.tile([128, c1 - c0], f32,
                                       name="pj%d" % ci, tag="pj")
            for k in range(k0, k1):
                w = w_sb[:, (p * KT + k) * 128: (p * KT + k + 1) * 128]
                for ci, (c0, c1) in enumerate(chunks):
                    nc.tensor.matmul(
                        ps[ci][:],
                        lhsT=w,
                        rhs=x_sb[:, k * cols + c0: k * cols + c1],
                        start=(k == 0), stop=(k == KT - 1))
            if k1 == KT:
                for ci, (c0, c1) in enumerate(chunks):
                    dst = dst_sb[:, p * cols + c0: p * cols + c1]
                    if on_act:
                        nc.scalar.activation(
                            out=dst, in_=ps[ci][:],
                            func=mybir.ActivationFunctionType.Copy)
                    else:
                        nc.vector.tensor_copy(out=dst, in_=ps[ci][:])

        return [lambda: half(0, KT // 2), lambda: half(KT // 2, KT)]

    def proj_q(p, on_act=False):
        return make_proj(qT_sb, wq_sb, xq_sb, p, N, [(0, 512), (512, 1024)],
                         on_act)

    def proj_k(p, on_act=False):
        return make_proj(kT_sb, wk_sb, xkv_sb, p, lc, kch, on_act)

    def v_proj(j):
        """V[j, c] = x_kv @ Wv_g for one j tile (interleaved dst halves)."""
        ps = smps.tile([128, 512], f32, tag="av", bufs=1)
        for k in range(KT):
            nc.tensor.matmul(
                ps[:],
                lhsT=xkv_sb[:, k * lc + j * 128: k * lc + (j + 1) * 128],
                rhs=wv_sb[:, k * DG:(k + 1) * DG],
                start=(k == 0), stop=(k == KT - 1))
        pv = ps[:].rearrange("p (h c) -> p h c", h=HG)
        # head A (even) V goes to cols DH:VW, head B (odd) to cols 0:DH
        nc.vector.tensor_copy(out=v_view[:, j, 0::2, DH:VW], in_=pv[:, 0::2])
        nc.vector.tensor_copy(out=v_view[:, j, 1::2, 0:DH], in_=pv[:, 1::2])

    def norm(po, p, hh, chunks=1, rec_act=False):
        """Normalize one head's AV straight into ot_sb (no DMA).

        hh=0 (head A, V_ext=[ones|V]): denom rows 0:64, O^T rows 64:128;
        mul shifts the PSUM operand down into ot rows 0:64.
        hh=1 (head B, V_ext=[V|ones]): O^T rows 0:64 shifts up into ot
        rows 64:128; recip/out SBUF operands aligned at partition 64.
        The reciprocal runs full-height base-aligned; the unused half is
        garbage (1/O^T values) that is never read. denom > ~1 always, so
        the fast approx reciprocal's denorm/inf edge cases can't occur
        on the rows that are consumed.
        """
        rec_t = recp.tile([128, 1024], f32)
        dn, ot_rows = ((slice(0, 64), slice(64, 128)) if hh == 0
                       else (slice(64, 128), slice(0, 64)))
        step = 1024 // chunks
        for c in range(chunks):
            cs = slice(c * step, (c + 1) * step)
            nc.vector.reciprocal_approx_fast(out=rec_t[:, cs],
                                             in_=po[:, cs])
            nc.vector.tensor_mul(
                out=ot_sb[dn, p * N + c * step: p * N + (c + 1) * step],
                in0=po[ot_rows, cs], in1=rec_t[dn, cs])

    def av_head(p, hh, pt, pool_tag="av", chunks=1, rec_act=False,
                skip_norm=False):
        """AV for head 2p+hh, both i chunks, one V weight load per j."""
        st = {}
        for u in av_head_units(p, hh, pt, pool_tag, chunks, rec_act, 1,
                               skip_norm, st):
            u()
        return st.get("po")

    def av_head_units(p, hh, pt, pool_tag="av", chunks=1, rec_act=False,
                      nunits=2, skip_norm=False, state=None):
        """AV for one head as nunits filler units (j-ranges + final norm)."""
        h = 2 * p + hh
        if state is None:
            state = {}

        def run(j0, j1):
            if j0 == 0:
                state["po"] = (
                    stps.tile([128, 1024], f32, name="po", tag="st")
                    if pool_tag == "st" else
                    smps.tile([128, 1024], f32, name="po", tag="av", bufs=1))
            po = state["po"]
            for j in range(j0, j1):
                vblk = v_sb[:, (j * HG + h) * VW: (j * HG + h + 1) * VW]
                nc.tensor.matmul(po[:, 0:512], lhsT=vblk,
                                 rhs=pt[:, j * N: j * N + 512],
                                 start=(j == 0), stop=(j == jt - 1))
                nc.tensor.matmul(po[:, 512:1024], lhsT=vblk,
                                 rhs=pt[:, j * N + 512: (j + 1) * N],
                                 start=(j == 0), stop=(j == jt - 1))
            if j1 == jt and not skip_norm:
                norm(po, p, hh, chunks=chunks, rec_act=rec_act)

        bounds = [jt * i // nunits for i in range(nunits + 1)]
        return [lambda a=a, b=b: run(a, b)
                for a, b in zip(bounds[:-1], bounds[1:])]

    def st_pair(p, pa, pb, slot_units):
        """ST + exp for pair p; filler units interleaved into HALF-j slots
        (2*jt of them) so the exp stream is fed a fresh ST half roughly
        every exp-duration instead of in bursts."""
        for j in range(jt):
            for half, (rb, pt) in enumerate(((0, pa), (64, pb))):
                ps = stps.tile([128, 1024], f32, name="st", tag="st")
                kk = kT_sb[rb:rb + 64, p * lc + j * 128: p * lc + (j + 1) * 128]
                for ic in range(IC):
                    cols = slice(ic * 512, ic * 512 + 512)
                    nc.tensor.matmul(
                        ps[:, cols], lhsT=kk,
                        rhs=qT_sb[rb:rb + 64,
                                  p * N + ic * 512: p * N + ic * 512 + 512],
                        start=True, stop=True)
                # EXP right after this half's matmuls: ACT starts earlier
                # and the pool slot frees a half-j sooner
                nc.scalar.activation(
                    out=pt[:, j * N:(j + 1) * N], in_=ps[:],
                    func=mybir.ActivationFunctionType.Exp,
                    bias=mb_sb[:, j:j + 1], scale=DP_SCALE)
                for u in slot_units.get(2 * j + half, []):
                    u()

    def op_mms(it, ps0, ps1, cts):
        for ct in cts:
            ot_blk = ot_sb[:, ct * N + it * 128: ct * N + (it + 1) * 128]
            nc.tensor.matmul(
                ps0[:], lhsT=ot_blk,
                rhs=wo_sb[:, ct * D: ct * D + 512],
                start=(ct == 0), stop=(ct == PAIRS - 1))
            nc.tensor.matmul(
                ps1[:], lhsT=ot_blk,
                rhs=wo_sb[:, ct * D + 512: ct * D + 1024],
                start=(ct == 0), stop=(ct == PAIRS - 1))

    def op_finish(it, ps0, ps1):
        # Copies split across DVE+ACT (both idle here). Early tiles store
        # whole on the idle Pool SWDGE path (separate issue queue, keeps
        # HWDGE clear); the last two tiles are latency-critical:
        # column-half stores on sync/scalar, each depending only on its
        # own engine's copy so neither store cross-waits the other engine.
        out_t = scp.tile([128, 1024], bf16, tag="outt")
        nc.vector.tensor_copy(out=out_t[:, 0:512], in_=ps0[:])
        nc.scalar.activation(out=out_t[:, 512:1024], in_=ps1[:],
                             func=mybir.ActivationFunctionType.Copy)
        r = slice(it * 128, (it + 1) * 128)
        if it < IT - 2:
            nc.gpsimd.dma_start(out=out[r, :], in_=out_t[:])
        elif it < IT - 1:
            # both halves on sync: a scalar-issued store here would hold
            # Activation.SEQ through HWDGE gen and delay the LAST tile's
            # ACT copy
            nc.sync.dma_start(out=out[r, 0:512], in_=out_t[:, 0:512])
            nc.sync.dma_start(out=out[r, 512:1024], in_=out_t[:, 512:1024])
        else:
            nc.sync.dma_start(out=out[r, 0:512], in_=out_t[:, 0:512])
            nc.scalar.dma_start(out=out[r, 512:1024],
                                in_=out_t[:, 512:1024])

    # ---- prologue: q0 then k0 only (minimum work before the ST stream
    # starts); q1-q3 stream in as pair-0/1 fillers as their weights land
    for u in proj_q(0, on_act=True):
        u()
    for u in proj_k(0, on_act=True):
        u()

    # ---- pipelined pairs ----
    prev = None
    for p in range(PAIRS):
        pa = pt_pool.tile([128, jt * N], bf16, tag="pa")
        pb = pt_pool.tile([128, jt * N], bf16, tag="pb")

        if p == 0:
            # DMA-paced fillers in arrival order: q1, q2, v tiles (wv),
            # q3 waits for pair 1, k1 last (its weights land last)
            proj_units = (proj_q(1)
                          + [lambda j=j: v_proj(j) for j in range(jt)]
                          + proj_q(2) + proj_k(1))
        elif p + 1 < PAIRS:
            proj_units = list(proj_k(p + 1))
            if p == 1:
                proj_units = proj_q(3) + proj_units
        else:
            proj_units = []
        if prev is not None:
            # av heads use the single "av" PSUM buffer: keep them apart so
            # head B's alloc never stalls the PE on head A's norms
            pp, ppa, ppb = prev
            na = len(proj_units) // 2
            units = ([lambda: av_head(pp, 0, ppa)] + proj_units[:na]
                     + [lambda: av_head(pp, 1, ppb)] + proj_units[na:])
        else:
            units = proj_units

        slot_units = {}
        nslots = 2 * jt
        # monotonic slot assignment keeps each proj's k0-half before its
        # k1-half (they share PSUM accumulators); +2 phantom units lean
        # the distribution toward late slots so the pair tail (when the
        # exp stream still drains) keeps PE fed
        for i, u in enumerate(units):
            slot_units.setdefault(
                min(nslots - 1, (i + 6) * nslots // (len(units) + 6)),
                []).append(u)
        st_pair(p, pa, pb, slot_units)
        prev = (p, pa, pb)

    # last pair's AV: head B first (single 'av' buffer), then head A from
    # the ST banks. The norms are hand-interleaved in column halves so the
    # DVE chain delivers the first 512 normalized query columns (both head
    # rows) as early as possible for the O-projection's ct=3 matmuls.
    pp, ppa, ppb = prev
    poB = av_head(pp, 1, ppb, skip_norm=True)
    recB_t = recp.tile([128, 1024], f32)
    nc.vector.reciprocal_approx_fast(out=recB_t[:], in_=poB[:])
    poA = av_head(pp, 0, ppa, pool_tag="st", skip_norm=True)
    recA_t = recp.tile([128, 1024], f32)
    for c in range(2):
        cs = slice(c * 512, (c + 1) * 512)
        oc = slice(pp * N + c * 512, pp * N + (c + 1) * 512)
        nc.vector.reciprocal_approx_fast(out=recA_t[:, cs], in_=poA[:, cs])
        # head B (V_ext=[V|ones]): O^T rows 0:64 -> ot rows 64:128
        nc.vector.tensor_mul(out=ot_sb[64:128, oc], in0=poB[0:64, cs],
                             in1=recB_t[64:128, cs])
        # head A (V_ext=[ones|V]): O^T rows 64:128 -> ot rows 0:64
        nc.vector.tensor_mul(out=ot_sb[0:64, oc], in0=poA[64:128, cs],
                             in1=recA_t[0:64, cs])

    # ---- output projection: partial[i, d] in bf16 ----
    # Runway: i-tiles 1-2 accumulate pairs 0-2 while the last norms run on
    # DVE; PSUM plan fills all 8 banks: AVB po (av, 2) + AVA po (st, 2) +
    # pend1 (st, 2) + pend2 (pj, 1+1). The O-proj loop then rotates
    # st/pj/av so tile allocs never wait on a two-deep copy pipeline.
    def op_psum(which):
        if which == "pj":
            o0 = smps.tile([128, 512], f32, name="o0", tag="pj")
            o1 = smps.tile([128, 512], f32, name="o1", tag="pj")
            return o0[:], o1[:]
        if which == "av":
            pw = smps.tile([128, 1024], f32, name="po", tag="av", bufs=1)
        else:
            pw = stps.tile([128, 1024], f32, name="pw", tag="st")
        return pw[:, 0:512], pw[:, 512:1024]

    pend = {}
    for it, pool in ((1, "st"), (2, "pj")):
        pend[it] = op_psum(pool)
        op_mms(it, *pend[it], range(PAIRS - 1))
    rot = ["av", "st", "st", "pj", "av", "st"]
    for it in range(IT):
        if it in pend:
            ps0, ps1 = pend[it]
            op_mms(it, ps0, ps1, range(PAIRS - 1, PAIRS))
        else:
            ps0, ps1 = op_psum(rot.pop(0))
            op_mms(it, ps0, ps1, range(PAIRS))
        op_finish(it, ps0, ps1)


_NC_CACHE = {}


def _get_nc(jt):
    if jt not in _NC_CACHE:
        _NC_CACHE[jt] = build_nc(jt=jt)
    return _NC_CACHE[jt]


def _tile_k(a, cols):
    """[KT*128, cols] -> [128, KT*cols] k-tile-major, contiguous bf16."""
    return np.ascontiguousarray(
        a.reshape(KT, 128, cols).transpose(1, 0, 2).reshape(128, KT * cols)
    ).astype(BF16)


def _make_in_maps(x_q, x_kv, pad_mask, Wq, Wk, Wv, Wo, jt=None):
    pad_mask = np.asarray(pad_mask)
    cnts = (~pad_mask).sum(axis=1)
    if jt is None:
        jt = max(1, int(-(-int(cnts.max()) // 128)))
    lc = jt * 128

    def _tile_pair(w):
        # [D, DG] -> [128, pair*KT*128 + k*128 + c] pair-major
        return np.ascontiguousarray(
            w.reshape(KT, 128, PAIRS, 128).transpose(1, 2, 0, 3)
            .reshape(128, PAIRS * KT * 128)).astype(BF16)

    per_g = []
    for g in range(2):
        cols = slice(g * DG, (g + 1) * DG)
        per_g.append({
            "wq": _tile_pair(np.ascontiguousarray(Wq[:, cols])),
            "wk": _tile_pair(np.ascontiguousarray(Wk[:, cols])),
            "wv": _tile_k(np.ascontiguousarray(Wv[:, cols]), DG),
            "wo": np.ascontiguousarray(
                Wo[g * DG:(g + 1) * DG, :]
                .reshape(PAIRS, 128, D).transpose(1, 0, 2)
                .reshape(128, PAIRS * D)).astype(BF16),
        })
    per_b = []
    for b in range(B):
        idx = np.flatnonzero(~pad_mask[b])
        n = len(idx)
        xc = np.zeros((lc, D), dtype=np.float32)
        xc[:n] = x_kv[b][idx]
        mbias = np.full(lc, MASK_NEG, dtype=np.float32)
        mbias[:n] = 0.0
        per_b.append({
            "xq": _tile_k(np.ascontiguousarray(x_q[b].T), N),
            "xkv": _tile_k(np.ascontiguousarray(xc.T), lc),
            "mb": np.ascontiguousarray(mbias.reshape(jt, 128).T),
        })

    in_maps = []
    for c in range(NCORES):
        b, g = c // 2, c % 2
        in_maps.append({**per_b[b], **per_g[g]})
    return in_maps, jt


def kernel(x_q, x_kv, pad_mask, Wq, Wk, Wv, Wo, bo):
    in_maps, jt = _make_in_maps(x_q, x_kv, pad_mask, Wq, Wk, Wv, Wo)
    nc = _get_nc(jt)
    res = run_bass_kernel_spmd(nc, in_maps, core_ids=list(range(NCORES)))
    full = np.empty((B, N, D), dtype=np.float32)
    bo32 = bo.astype(np.float32)
    for b in range(B):
        full[b] = (res.results[2 * b]["out"].astype(np.float32)
                   + res.results[2 * b + 1]["out"].astype(np.float32))
        full[b] += bo32
    return full


# revision 78
# speedup vs baseline: 1.0203x; 1.0017x over previous
"""Trainium2 Bass kernel for nn_MultiHeadAttention_37512244363503.

Sharding: 8 cores = 4 batches x 2 head-groups (8 heads each).
Per core (b, g): Wq/Wk/Wv column-sliced, Wo row-sliced; the host sums the
two partial outputs per batch (the row-parallel "all-reduce") and adds bo.

Key compaction: pad_mask is host-visible, so masked keys are dropped on
the host before upload. Keys compact to jt*128 columns (jt chosen from
the max per-batch unmasked count, 5 for the reference distribution),
cutting K/V projections, scores, softmax and AV by L_c/L. Padding slots
get a -30000 exp-bias so they contribute exactly 0.

All DRAM inputs are pre-tiled on host to the SBUF k-tile-major layout so
every load is a contiguous [128, cols] copy (no strided descriptors).

Per-core algorithm (matmuls bf16 in / fp32 PSUM accumulate):
  QT[d,i]   = Wq_g.T @ x_q[b].T        (d=512 cols of this group)
  KT[d,j]   = Wk_g.T @ x_kv_c[b].T     (j over compacted keys)
  V[j,c]    = x_kv_c[b] @ Wv_g         (per-head [ones|V_h] / [V_h|ones])
  ST[j,i]   = K_h Q_h.T per head       (2 heads packed via PE row groups)
  PT        = exp(ST/8 + mask_bias[j]) (ACT; bias rides the ACT bias input)
  po        = V_ext.T @ PT             (denom + O^T in one matmul)
  O_norm^T  = po_OT * recip(po_denom)  (DVE; written straight into ot_sb)
  partial   = O_norm @ Wo_g            (bf16 partial -> DRAM, host sums)

Norms write directly into ot_sb (no SBUF->SBUF DMA): head A (hh=0) keeps
V_ext = [ones|V] so denom sits at PSUM rows 0:64 and O^T at 64:128; the
DVE mul shifts the PSUM operand down to write ot rows 0:64. Head B flips
V_ext = [V|ones] so its mul writes ot rows 64:128 with the SBUF operands
(recip, out) partition-aligned at 64. The fast reciprocal always reads
the full [128,*] PSUM tile base-aligned (unused rows are garbage but
never read; the custom DVE op corrupts on shifted APs, so base-aligned
full-height is the only safe form).

Schedule (TimelineSim 97.1us vs 116.4us for the previous build; PE busy
78.8us ~= the bf16 cycle floor of this dataflow):
  prologue  q0 + k0 only, DMA-paced (loads stream in consumption order)
  pair p    ST halves + exp, with fillers slotted between half-j's:
            p0: q1, v_proj x jt, q2, k1; p1: AV0, q3, k2; p2: AV1, k3;
            p3: AV2 (filler distribution back-loaded: the exp stream is
            the per-pair pacer, ~11.5us/pair vs ~10.7us of PE work)
  endgame   AV3-A (full-norm) -> AV3-B (column-chunked norm), runway
            i-tiles 1-2 (pairs 0-2) hide the norm latency, then the
            O-projection drains with a st/st/st/pj/st/pj PSUM rotation
"""

import numpy as np
import ml_dtypes

import concourse.bass as bass
import concourse.mybir as mybir
from concourse import bacc
from concourse.tile import TileContext
from concourse.bass_utils import run_bass_kernel_spmd

BF16 = ml_dtypes.bfloat16

B, N, L, D, H = 4, 1024, 1024, 1024, 16
DH = D // H           # 64 channels per head
HG = 8                # heads per core
DG = HG * DH          # 512 channels per core
NCORES = 8
DP_SCALE = DH ** -0.5
MASK_NEG = -30000.0   # exp(x + MASK_NEG) underflows to exactly 0.0

f32 = mybir.dt.float32
bf16 = mybir.dt.bfloat16

KT = D // 128         # 8 k-tiles in the contraction dim of projections
IT = N // 128         # 8 query tiles
IC = N // 512         # 2 query chunks (PSUM free dim)
PAIRS = HG // 2       # 4 head pairs (2 heads packed per 128 partitions)
VW = 2 * DH           # 128 cols per (j, head) V_ext block


def build_nc(jt=5, debug=False, num_devices=NCORES, repeat=1):
    lc = jt * 128
    nc = bacc.Bacc("TRN2", target_bir_lowering=False, debug=False,
                   num_devices=num_devices)

    xq = nc.dram_tensor("xq", [128, KT * N], bf16, kind="ExternalInput")
    xkv = nc.dram_tensor("xkv", [128, KT * lc], bf16, kind="ExternalInput")
    # wq/wk are PAIR-major ([128, pair*KT*128 + k*128 + c]) so one pair's
    # projection weights are a single contiguous 256KB slice loadable first
    wq = nc.dram_tensor("wq", [128, KT * DG], bf16, kind="ExternalInput")
    wk = nc.dram_tensor("wk", [128, KT * DG], bf16, kind="ExternalInput")
    wv = nc.dram_tensor("wv", [128, KT * DG], bf16, kind="ExternalInput")
    wo = nc.dram_tensor("wo", [128, PAIRS * D], bf16, kind="ExternalInput")
    mb = nc.dram_tensor("mb", [128, jt], f32, kind="ExternalInput")
    out = nc.dram_tensor("out", [N, D], bf16, kind="ExternalOutput")

    with TileContext(nc) as tc:
        with (
            tc.tile_pool(name="persist", bufs=1) as persist,
            tc.tile_pool(name="pt", bufs=3) as pt_pool,
            tc.tile_pool(name="recp", bufs=4) as recp,
            tc.tile_pool(name="scp", bufs=5) as scp,
            tc.tile_pool(name="stps", bufs=2, space="PSUM") as stps,
            tc.tile_pool(name="smps", bufs=2, space="PSUM") as smps,
        ):
            env = dict(jt=jt, lc=lc)
            for nm, shape in [
                ("xq_sb", [128, KT * N]), ("xkv_sb", [128, KT * lc]),
                ("wq_sb", [128, KT * DG]), ("wk_sb", [128, KT * DG]),
                ("wv_sb", [128, KT * DG]), ("wo_sb", [128, PAIRS * D]),
                ("qT_sb", [128, PAIRS * N]), ("kT_sb", [128, PAIRS * lc]),
                ("v_sb", [128, jt * HG * VW]), ("ot_sb", [128, PAIRS * N]),
            ]:
                env[nm] = persist.tile(shape, bf16, name=nm)
            env["mb_sb"] = persist.tile([128, jt], f32, name="mb_sb")
            env["v_view"] = env["v_sb"][:].rearrange(
                "p (j h c) -> p j h c", j=jt, h=HG)
            # Per-head V_ext layout: head A (even h) = [ones | V_h] so the
            # softmax denominator lands at PSUM rows 0:64 (base-aligned for
            # the custom reciprocal) and O^T at 64:128 (the shiftable PSUM
            # mul operand, written to ot rows 0:64). Head B (odd h) =
            # [V_h | ones]: O^T at rows 0:64 shifts down into ot rows
            # 64:128 with recip/out SBUF operands aligned at 64.
            nc.vector.memset(env["v_view"][:, :, 0::2, 0:DH], 1.0)
            nc.vector.memset(env["v_view"][:, :, 1::2, DH:VW], 1.0)
            env.update(pt_pool=pt_pool, recp=recp, scp=scp, stps=stps,
                       smps=smps, xq=xq, xkv=xkv, wq=wq, wk=wk, wv=wv,
                       wo=wo, mb=mb, out=out)
            for _rep in range(repeat):
                _emit_body(nc, env)

    nc.compile()
    return nc


def _emit_body(nc, env):
    jt, lc = env["jt"], env["lc"]
    xq_sb, xkv_sb = env["xq_sb"], env["xkv_sb"]
    wq_sb, wk_sb, wv_sb, wo_sb = (env[k] for k in
                                  ["wq_sb", "wk_sb", "wv_sb", "wo_sb"])
    qT_sb, kT_sb, v_sb, ot_sb, mb_sb = (env[k] for k in
                                        ["qT_sb", "kT_sb", "v_sb", "ot_sb",
                                         "mb_sb"])
    v_view = env["v_view"]
    pt_pool, recp, scp = env["pt_pool"], env["recp"], env["scp"]
    stps, smps = env["stps"], env["smps"]
    xq, xkv, wq, wk, wv, wo, mb, out = (env[k] for k in
                                        ["xq", "xkv", "wq", "wk", "wv",
                                         "wo", "mb", "out"])

    # ---- input loads ----
    # Loads are ordered by first PE use: xq k0 + pair-0 Q weights, the
    # rest of xq (k-tile streamed under the q0 projection), pair-0 K
    # weights, xkv, then the filler-unit inputs (wq pairs 1-3, wv, wk
    # pairs 1-3) and finally wo. Three rules learned from the timeline:
    # (1) the DMA transfer queue drains in request order, so loads must
    # reach it strictly in consumption order -- a lower-priority load on
    # a faster-clearing queue cuts the line and delays ST p0;
    # (2) ALL loads go on sync: a dma_start holds its issuing engine's
    # SEQ through the serialized shared HWDGE descriptor-gen stage, so
    # scalar-issued loads would block the prologue ACT copies (which hold
    # the pj PSUM ring!) and the exp stream behind ~8us of queued issues;
    # (3) gpsimd's SWDGE path (1us serialized issue, separate queue that
    # would jump the HWDGE stream) only carries the tiny mb load.
    def ld_split(dst, src, c0, c1, nsplit):
        step = 128 // nsplit
        for s in range(nsplit):
            p0, p1 = s * step, (s + 1) * step
            nc.sync.dma_start(out=dst[p0:p1, c0:c1], in_=src[p0:p1, c0:c1])

    PW = KT * 128  # cols per pair in the pair-major wq/wk layouts
    # xq k0 rides the gpsimd SWDGE path: its descriptor-gen overlaps the
    # sync HWDGE chain, so the first matmul's deps land ~0.7us earlier
    nc.gpsimd.dma_start(out=xq_sb[:, 0:N], in_=xq[:, 0:N])
    nc.gpsimd.dma_start(out=mb_sb[:], in_=mb[:, :])
    ld_split(wq_sb, wq, 0, 256, 1)         # 64KB: q0 k0/k1 matmul deps
    ld_split(wq_sb, wq, 256, PW, 1)
    for k in range(1, KT):
        ld_split(xq_sb, xq, k * N, (k + 1) * N, 1)
    ld_split(wk_sb, wk, 0, PW, 1)
    qx = KT * lc // 4
    for s in range(4):
        ld_split(xkv_sb, xkv, s * qx, (s + 1) * qx, 1)
    ld_split(wq_sb, wq, PW, 2 * PW, 1)     # q1 (early pair-0 filler)
    ld_split(wv_sb, wv, 0, KT * DG, 2)      # v_proj fillers (pair 0)
    ld_split(wq_sb, wq, 2 * PW, 3 * PW, 1)  # q2
    ld_split(wk_sb, wk, PW, 2 * PW, 1)      # k1 (pair-0 last filler)
    ld_split(wq_sb, wq, 3 * PW, 4 * PW, 1)  # q3 (pair-1 filler)
    ld_split(wk_sb, wk, 2 * PW, 4 * PW, 2)
    ld_split(wo_sb, wo, 0, PAIRS * D, 2)

    # k-proj output chunking (lc may exceed one PSUM bank)
    kch = [(0, min(512, lc))] + ([(512, lc)] if lc > 512 else [])

    def make_proj(dst_sb, w_sb, x_sb, p, cols, chunks, on_act=False):
        """Projection for pair p as two units sharing PSUM accumulators.

        chunks: list of (c0, c1) output-column ranges (<=512 wide each).
        on_act: do the PSUM->SBUF copies on the idle ACT engine (prologue
        projections), keeping the DVE stream free.
        """
        ps = [None] * len(chunks)

        def half(k0, k1):
            for ci, (c0, c1) in enumerate(chunks):
                if k0 == 0:
                    ps[ci] = smps.tile([128, c1 - c0], f32,
                                       name="pj%d" % ci, tag="pj")
            for k in range(k0, k1):
                w = w_sb[:, (p * KT + k) * 128: (p * KT + k + 1) * 128]
                for ci, (c0, c1) in enumerate(chunks):
                    nc.tensor.matmul(
                        ps[ci][:],
                        lhsT=w,
                        rhs=x_sb[:, k * cols + c0: k * cols + c1],
                        start=(k == 0), stop=(k == KT - 1))
            if k1 == KT:
                for ci, (c0, c1) in enumerate(chunks):
                    dst = dst_sb[:, p * cols + c0: p * cols + c1]
                    if on_act:
                        nc.scalar.activation(
                            out=dst, in_=ps[ci][:],
                            func=mybir.ActivationFunctionType.Copy)
                    else:
                        nc.vector.tensor_copy(out=dst, in_=ps[ci][:])

        return [lambda: half(0, KT // 2), lambda: half(KT // 2, KT)]

    def proj_q(p, on_act=False):
        return make_proj(qT_sb, wq_sb, xq_sb, p, N, [(0, 512), (512, 1024)],
                         on_act)

    def proj_k(p, on_act=False):
        return make_proj(kT_sb, wk_sb, xkv_sb, p, lc, kch, on_act)

    def v_proj(j):
        """V[j, c] = x_kv @ Wv_g for one j tile (interleaved dst halves)."""
        ps = smps.tile([128, 512], f32, tag="av", bufs=1)
        for k in range(KT):
            nc.tensor.matmul(
                ps[:],
                lhsT=xkv_sb[:, k * lc + j * 128: k * lc + (j + 1) * 128],
                rhs=wv_sb[:, k * DG:(k + 1) * DG],
                start=(k == 0), stop=(k == KT - 1))
        pv = ps[:].rearrange("p (h c) -> p h c", h=HG)
        # head A (even) V goes to cols DH:VW, head B (odd) to cols 0:DH
        nc.vector.tensor_copy(out=v_view[:, j, 0::2, DH:VW], in_=pv[:, 0::2])
        nc.vector.tensor_copy(out=v_view[:, j, 1::2, 0:DH], in_=pv[:, 1::2])

    def norm(po, p, hh, chunks=1, rec_act=False):
        """Normalize one head's AV straight into ot_sb (no DMA).

        hh=0 (head A, V_ext=[ones|V]): denom rows 0:64, O^T rows 64:128;
        mul shifts the PSUM operand down into ot rows 0:64.
        hh=1 (head B, V_ext=[V|ones]): O^T rows 0:64 shifts up into ot
        rows 64:128; recip/out SBUF operands aligned at partition 64.
        The reciprocal runs full-height base-aligned; the unused half is
        garbage (1/O^T values) that is never read. denom > ~1 always, so
        the fast approx reciprocal's denorm/inf edge cases can't occur
        on the rows that are consumed.
        """
        rec_t = recp.tile([128, 1024], f32)
        dn, ot_rows = ((slice(0, 64), slice(64, 128)) if hh == 0
                       else (slice(64, 128), slice(0, 64)))
        step = 1024 // chunks
        for c in range(chunks):
            cs = slice(c * step, (c + 1) * step)
            nc.vector.reciprocal_approx_fast(out=rec_t[:, cs],
                                             in_=po[:, cs])
            nc.vector.tensor_mul(
                out=ot_sb[dn, p * N + c * step: p * N + (c + 1) * step],
                in0=po[ot_rows, cs], in1=rec_t[dn, cs])

    def av_head(p, hh, pt, pool_tag="av", chunks=1, rec_act=False,
                skip_norm=False):
        """AV for head 2p+hh, both i chunks, one V weight load per j."""
        st = {}
        for u in av_head_units(p, hh, pt, pool_tag, chunks, rec_act, 1,
                               skip_norm, st):
            u()
        return st.get("po")

    def av_head_units(p, hh, pt, pool_tag="av", chunks=1, rec_act=False,
                      nunits=2, skip_norm=False, state=None):
        """AV for one head as nunits filler units (j-ranges + final norm)."""
        h = 2 * p + hh
        if state is None:
            state = {}

        def run(j0, j1):
            if j0 == 0:
                state["po"] = (
                    stps.tile([128, 1024], f32, name="po", tag="st")
                    if pool_tag == "st" else
                    smps.tile([128, 1024], f32, name="po", tag="av", bufs=1))
            po = state["po"]
            for j in range(j0, j1):
                vblk = v_sb[:, (j * HG + h) * VW: (j * HG + h + 1) * VW]
                nc.tensor.matmul(po[:, 0:512], lhsT=vblk,
                                 rhs=pt[:, j * N: j * N + 512],
                                 start=(j == 0), stop=(j == jt - 1))
                nc.tensor.matmul(po[:, 512:1024], lhsT=vblk,
                                 rhs=pt[:, j * N + 512: (j + 1) * N],
                                 start=(j == 0), stop=(j == jt - 1))
            if j1 == jt and not skip_norm:
                norm(po, p, hh, chunks=chunks, rec_act=rec_act)

        bounds = [jt * i // nunits for i in range(nunits + 1)]
        return [lambda a=a, b=b: run(a, b)
                for a, b in zip(bounds[:-1], bounds[1:])]

    def st_pair(p, pa, pb, slot_units):
        """ST + exp for pair p; filler units interleaved into HALF-j slots
        (2*jt of them) so the exp stream is fed a fresh ST half roughly
        every exp-duration instead of in bursts."""
        for j in range(jt):
            for half, (rb, pt) in enumerate(((0, pa), (64, pb))):
                ps = stps.tile([128, 1024], f32, name="st", tag="st")
                kk = kT_sb[rb:rb + 64, p * lc + j * 128: p * lc + (j + 1) * 128]
                for ic in range(IC):
                    cols = slice(ic * 512, ic * 512 + 512)
                    nc.tensor.matmul(
                        ps[:, cols], lhsT=kk,
                        rhs=qT_sb[rb:rb + 64,
                                  p * N + ic * 512: p * N + ic * 512 + 512],
                        start=True, stop=True)
                # EXP right after this half's matmuls: ACT starts earlier
                # and the pool slot frees a half-j sooner
                nc.scalar.activation(
                    out=pt[:, j * N:(j + 1) * N], in_=ps[:],
                    func=mybir.ActivationFunctionType.Exp,
                    bias=mb_sb[:, j:j + 1], scale=DP_SCALE)
                for u in slot_units.get(2 * j + half, []):
                    u()

    def op_mms(it, ps0, ps1, cts):
        for ct in cts:
            ot_blk = ot_sb[:, ct * N + it * 128: ct * N + (it + 1) * 128]
            nc.tensor.matmul(
                ps0[:], lhsT=ot_blk,
                rhs=wo_sb[:, ct * D: ct * D + 512],
                start=(ct == 0), stop=(ct == PAIRS - 1))
            nc.tensor.matmul(
                ps1[:], lhsT=ot_blk,
                rhs=wo_sb[:, ct * D + 512: ct * D + 1024],
                start=(ct == 0), stop=(ct == PAIRS - 1))

    def op_finish(it, ps0, ps1):
        # Copies split across DVE+ACT (both idle here). Early tiles store
        # whole on the idle Pool SWDGE path (separate issue queue, keeps
        # HWDGE clear); the last two tiles are latency-critical:
        # column-half stores on sync/scalar, each depending only on its
        # own engine's copy so neither store cross-waits the other engine.
        out_t = scp.tile([128, 1024], bf16, tag="outt")
        nc.vector.tensor_copy(out=out_t[:, 0:512], in_=ps0[:])
        nc.scalar.activation(out=out_t[:, 512:1024], in_=ps1[:],
                             func=mybir.ActivationFunctionType.Copy)
        r = slice(it * 128, (it + 1) * 128)
        if it < IT - 2:
            nc.gpsimd.dma_start(out=out[r, :], in_=out_t[:])
        elif it < IT - 1:
            # both halves on sync: a scalar-issued store here would hold
            # Activation.SEQ through HWDGE gen and delay the LAST tile's
            # ACT copy
            nc.sync.dma_start(out=out[r, 0:512], in_=out_t[:, 0:512])
            nc.sync.dma_start(out=out[r, 512:1024], in_=out_t[:, 512:1024])
        else:
            nc.sync.dma_start(out=out[r, 0:512], in_=out_t[:, 0:512])
            nc.scalar.dma_start(out=out[r, 512:1024],
                                in_=out_t[:, 512:1024])

    # ---- prologue: q0 then k0 only (minimum work before the ST stream
    # starts); q1-q3 stream in as pair-0/1 fillers as their weights land
    for u in proj_q(0, on_act=True):
        u()
    for u in proj_k(0, on_act=True):
        u()

    # ---- pipelined pairs ----
    prev = None
    for p in range(PAIRS):
        pa = pt_pool.tile([128, jt * N], bf16, tag="pa")
        pb = pt_pool.tile([128, jt * N], bf16, tag="pb")

        if p == 0:
            # DMA-paced fillers in arrival order: q1, q2, v tiles (wv),
            # q3 waits for pair 1, k1 last (its weights land last)
            proj_units = (proj_q(1)
                          + [lambda j=j: v_proj(j) for j in range(jt)]
                          + proj_q(2) + proj_k(1))
        elif p + 1 < PAIRS:
            proj_units = list(proj_k(p + 1))
            if p == 1:
                proj_units = proj_q(3) + proj_units
        else:
            proj_units = []
        if prev is not None:
            # av heads use the single "av" PSUM buffer: keep them apart so
            # head B's alloc never stalls the PE on head A's norms
            pp, ppa, ppb = prev
            na = len(proj_units) // 2
            units = ([lambda: av_head(pp, 0, ppa)] + proj_units[:na]
                     + [lambda: av_head(pp, 1, ppb)] + proj_units[na:])
        else:
            units = proj_units

        slot_units = {}
        nslots = 2 * jt
        # monotonic slot assignment keeps each proj's k0-half before its
        # k1-half (they share PSUM accumulators); +2 phantom units lean
        # the distribution toward late slots so the pair tail (when the
        # exp stream still drains) keeps PE fed
        for i, u in enumerate(units):
            slot_units.setdefault(
                min(nslots - 1, (i + 6) * nslots // (len(units) + 6)),
                []).append(u)
        st_pair(p, pa, pb, slot_units)
        prev = (p, pa, pb)

    # last pair's AV: head A first (its PT's exps finish ~1us before head
    # B's -- st_pair emits the a-half exp before the b-half per j), then
    # head B from the ST banks. The norms are hand-interleaved in column
    # halves so the DVE chain delivers the first 512 normalized query
    # columns (both head rows) as early as possible for ct=3.
    pp, ppa, ppb = prev
    poA = av_head(pp, 0, ppa, skip_norm=True)
    recA_t = recp.tile([128, 1024], f32)
    nc.vector.reciprocal_approx_fast(out=recA_t[:], in_=poA[:])
    poB = av_head(pp, 1, ppb, pool_tag="st", skip_norm=True)
    recB_t = recp.tile([128, 1024], f32)
    for c in range(2):
        cs = slice(c * 512, (c + 1) * 512)
        oc = slice(pp * N + c * 512, pp * N + (c + 1) * 512)
        nc.vector.reciprocal_approx_fast(out=recB_t[:, cs], in_=poB[:, cs])
        # head A (V_ext=[ones|V]): O^T rows 64:128 -> ot rows 0:64
        nc.vector.tensor_mul(out=ot_sb[0:64, oc], in0=poA[64:128, cs],
                             in1=recA_t[0:64, cs])
        # head B (V_ext=[V|ones]): O^T rows 0:64 -> ot rows 64:128
        nc.vector.tensor_mul(out=ot_sb[64:128, oc], in0=poB[0:64, cs],
                             in1=recB_t[64:128, cs])

    # ---- output projection: partial[i, d] in bf16 ----
    # Runway: i-tiles 1-2 accumulate pairs 0-2 while the last norms run on
    # DVE; PSUM plan fills all 8 banks: AVB po (av, 2) + AVA po (st, 2) +
    # pend1 (st, 2) + pend2 (pj, 1+1). The O-proj loop then rotates
    # st/pj/av so tile allocs never wait on a two-deep copy pipeline.
    def op_psum(which):
        if which == "pj":
            o0 = smps.tile([128, 512], f32, name="o0", tag="pj")
            o1 = smps.tile([128, 512], f32, name="o1", tag="pj")
            return o0[:], o1[:]
        if which == "av":
            pw = smps.tile([128, 1024], f32, name="po", tag="av", bufs=1)
        else:
            pw = stps.tile([128, 1024], f32, name="pw", tag="st")
        return pw[:, 0:512], pw[:, 512:1024]

    pend = {}
    for it, pool in ((1, "st"), (2, "pj")):
        pend[it] = op_psum(pool)
        op_mms(it, *pend[it], range(PAIRS - 1))
    rot = ["av", "st", "st", "pj", "av", "st"]
    for it in range(IT):
        if it in pend:
            ps0, ps1 = pend[it]
            op_mms(it, ps0, ps1, range(PAIRS - 1, PAIRS))
        else:
            ps0, ps1 = op_psum(rot.pop(0))
            op_mms(it, ps0, ps1, range(PAIRS))
        op_finish(it, ps0, ps1)


_NC_CACHE = {}


def _get_nc(jt):
    if jt not in _NC_CACHE:
        _NC_CACHE[jt] = build_nc(jt=jt)
    return _NC_CACHE[jt]


def _tile_k(a, cols):
    """[KT*128, cols] -> [128, KT*cols] k-tile-major, contiguous bf16."""
    return np.ascontiguousarray(
        a.reshape(KT, 128, cols).transpose(1, 0, 2).reshape(128, KT * cols)
    ).astype(BF16)


def _make_in_maps(x_q, x_kv, pad_mask, Wq, Wk, Wv, Wo, jt=None):
    pad_mask = np.asarray(pad_mask)
    cnts = (~pad_mask).sum(axis=1)
    if jt is None:
        jt = max(1, int(-(-int(cnts.max()) // 128)))
    lc = jt * 128

    def _tile_pair(w):
        # [D, DG] -> [128, pair*KT*128 + k*128 + c] pair-major
        return np.ascontiguousarray(
            w.reshape(KT, 128, PAIRS, 128).transpose(1, 2, 0, 3)
            .reshape(128, PAIRS * KT * 128)).astype(BF16)

    per_g = []
    for g in range(2):
        cols = slice(g * DG, (g + 1) * DG)
        per_g.append({
            "wq": _tile_pair(np.ascontiguousarray(Wq[:, cols])),
            "wk": _tile_pair(np.ascontiguousarray(Wk[:, cols])),
            "wv": _tile_k(np.ascontiguousarray(Wv[:, cols]), DG),
            "wo": np.ascontiguousarray(
                Wo[g * DG:(g + 1) * DG, :]
                .reshape(PAIRS, 128, D).transpose(1, 0, 2)
                .reshape(128, PAIRS * D)).astype(BF16),
        })
    per_b = []
    for b in range(B):
        idx = np.flatnonzero(~pad_mask[b])
        n = len(idx)
        xc = np.zeros((lc, D), dtype=np.float32)
        xc[:n] = x_kv[b][idx]
        mbias = np.full(lc, MASK_NEG, dtype=np.float32)
        mbias[:n] = 0.0
        per_b.append({
            "xq": _tile_k(np.ascontiguousarray(x_q[b].T), N),
            "xkv": _tile_k(np.ascontiguousarray(xc.T), lc),
            "mb": np.ascontiguousarray(mbias.reshape(jt, 128).T),
        })

    in_maps = []
    for c in range(NCORES):
        b, g = c // 2, c % 2
        in_maps.append({**per_b[b], **per_g[g]})
    return in_maps, jt


def kernel(x_q, x_kv, pad_mask, Wq, Wk, Wv, Wo, bo):
    in_maps, jt = _make_in_maps(x_q, x_kv, pad_mask, Wq, Wk, Wv, Wo)
    nc = _get_nc(jt)
    res = run_bass_kernel_spmd(nc, in_maps, core_ids=list(range(NCORES)))
    full = np.empty((B, N, D), dtype=np.float32)
    bo32 = bo.astype(np.float32)
    for b in range(B):
        full[b] = (res.results[2 * b]["out"].astype(np.float32)
                   + res.results[2 * b + 1]["out"].astype(np.float32))
        full[b] += bo32
    return full


# revision 81
# speedup vs baseline: 1.0262x; 1.0057x over previous
"""Trainium2 Bass kernel for nn_MultiHeadAttention_37512244363503.

Sharding: 8 cores = 4 batches x 2 head-groups (8 heads each).
Per core (b, g): Wq/Wk/Wv column-sliced, Wo row-sliced; the host sums the
two partial outputs per batch (the row-parallel "all-reduce") and adds bo.

Key compaction: pad_mask is host-visible, so masked keys are dropped on
the host before upload. Keys compact to jt*128 columns (jt chosen from
the max per-batch unmasked count, 5 for the reference distribution),
cutting K/V projections, scores, softmax and AV by L_c/L. Padding slots
get a -30000 exp-bias so they contribute exactly 0.

All DRAM inputs are pre-tiled on host to the SBUF k-tile-major layout so
every load is a contiguous [128, cols] copy (no strided descriptors).

Per-core algorithm (matmuls bf16 in / fp32 PSUM accumulate):
  QT[d,i]   = Wq_g.T @ x_q[b].T        (d=512 cols of this group)
  KT[d,j]   = Wk_g.T @ x_kv_c[b].T     (j over compacted keys)
  V[j,c]    = x_kv_c[b] @ Wv_g         (per-head [ones|V_h] / [V_h|ones])
  ST[j,i]   = K_h Q_h.T per head       (2 heads packed via PE row groups)
  PT        = exp(ST/8 + mask_bias[j]) (ACT; bias rides the ACT bias input)
  po        = V_ext.T @ PT             (denom + O^T in one matmul)
  O_norm^T  = po_OT * recip(po_denom)  (DVE; written straight into ot_sb)
  partial   = O_norm @ Wo_g            (bf16 partial -> DRAM, host sums)

Norms write directly into ot_sb (no SBUF->SBUF DMA): head A (hh=0) keeps
V_ext = [ones|V] so denom sits at PSUM rows 0:64 and O^T at 64:128; the
DVE mul shifts the PSUM operand down to write ot rows 0:64. Head B flips
V_ext = [V|ones] so its mul writes ot rows 64:128 with the SBUF operands
(recip, out) partition-aligned at 64. The fast reciprocal always reads
the full [128,*] PSUM tile base-aligned (unused rows are garbage but
never read; the custom DVE op corrupts on shifted APs, so base-aligned
full-height is the only safe form).

Schedule (TimelineSim 97.1us vs 116.4us for the previous build; PE busy
78.8us ~= the bf16 cycle floor of this dataflow):
  prologue  q0 + k0 only, DMA-paced (loads stream in consumption order)
  pair p    ST halves + exp, with fillers slotted between half-j's:
            p0: q1, v_proj x jt, q2, k1; p1: AV0, q3, k2; p2: AV1, k3;
            p3: AV2 (filler distribution back-loaded: the exp stream is
            the per-pair pacer, ~11.5us/pair vs ~10.7us of PE work)
  endgame   AV3-A (full-norm) -> AV3-B (column-chunked norm), runway
            i-tiles 1-2 (pairs 0-2) hide the norm latency, then the
            O-projection drains with a st/st/st/pj/st/pj PSUM rotation
"""

import numpy as np
import ml_dtypes

import concourse.bass as bass
import concourse.mybir as mybir
from concourse import bacc
from concourse.tile import TileContext
from concourse.bass_utils import run_bass_kernel_spmd

BF16 = ml_dtypes.bfloat16

B, N, L, D, H = 4, 1024, 1024, 1024, 16
DH = D // H           # 64 channels per head
HG = 8                # heads per core
DG = HG * DH          # 512 channels per core
NCORES = 8
DP_SCALE = DH ** -0.5
MASK_NEG = -30000.0   # exp(x + MASK_NEG) underflows to exactly 0.0

f32 = mybir.dt.float32
bf16 = mybir.dt.bfloat16

KT = D // 128         # 8 k-tiles in the contraction dim of projections
IT = N // 128         # 8 query tiles
IC = N // 512         # 2 query chunks (PSUM free dim)
PAIRS = HG // 2       # 4 head pairs (2 heads packed per 128 partitions)
VW = 2 * DH           # 128 cols per (j, head) V_ext block


def build_nc(jt=5, debug=False, num_devices=NCORES, repeat=1):
    lc = jt * 128
    nc = bacc.Bacc("TRN2", target_bir_lowering=False, debug=False,
                   num_devices=num_devices)

    xq = nc.dram_tensor("xq", [128, KT * N], bf16, kind="ExternalInput")
    xkv = nc.dram_tensor("xkv", [128, KT * lc], bf16, kind="ExternalInput")
    # wq/wk are PAIR-major ([128, pair*KT*128 + k*128 + c]) so one pair's
    # projection weights are a single contiguous 256KB slice loadable first
    wq = nc.dram_tensor("wq", [128, KT * DG], bf16, kind="ExternalInput")
    wk = nc.dram_tensor("wk", [128, KT * DG], bf16, kind="ExternalInput")
    wv = nc.dram_tensor("wv", [128, KT * DG], bf16, kind="ExternalInput")
    wo = nc.dram_tensor("wo", [128, PAIRS * D], bf16, kind="ExternalInput")
    mb = nc.dram_tensor("mb", [128, jt], f32, kind="ExternalInput")
    out = nc.dram_tensor("out", [N, D], bf16, kind="ExternalOutput")

    with TileContext(nc) as tc:
        with (
            tc.tile_pool(name="persist", bufs=1) as persist,
            tc.tile_pool(name="pt", bufs=3) as pt_pool,
            tc.tile_pool(name="recp", bufs=4) as recp,
            tc.tile_pool(name="scp", bufs=5) as scp,
            tc.tile_pool(name="stps", bufs=2, space="PSUM") as stps,
            tc.tile_pool(name="smps", bufs=2, space="PSUM") as smps,
        ):
            env = dict(jt=jt, lc=lc)
            for nm, shape in [
                ("xq_sb", [128, KT * N]), ("xkv_sb", [128, KT * lc]),
                ("wq_sb", [128, KT * DG]), ("wk_sb", [128, KT * DG]),
                ("wv_sb", [128, KT * DG]), ("wo_sb", [128, PAIRS * D]),
                ("qT_sb", [128, PAIRS * N]), ("kT_sb", [128, PAIRS * lc]),
                ("v_sb", [128, jt * HG * VW]), ("ot_sb", [128, PAIRS * N]),
            ]:
                env[nm] = persist.tile(shape, bf16, name=nm)
            env["mb_sb"] = persist.tile([128, jt], f32, name="mb_sb")
            env["v_view"] = env["v_sb"][:].rearrange(
                "p (j h c) -> p j h c", j=jt, h=HG)
            # Per-head V_ext layout: head A (even h) = [ones | V_h] so the
            # softmax denominator lands at PSUM rows 0:64 (base-aligned for
            # the custom reciprocal) and O^T at 64:128 (the shiftable PSUM
            # mul operand, written to ot rows 0:64). Head B (odd h) =
            # [V_h | ones]: O^T at rows 0:64 shifts down into ot rows
            # 64:128 with recip/out SBUF operands aligned at 64.
            nc.vector.memset(env["v_view"][:, :, 0::2, 0:DH], 1.0)
            nc.vector.memset(env["v_view"][:, :, 1::2, DH:VW], 1.0)
            env.update(pt_pool=pt_pool, recp=recp, scp=scp, stps=stps,
                       smps=smps, xq=xq, xkv=xkv, wq=wq, wk=wk, wv=wv,
                       wo=wo, mb=mb, out=out)
            for _rep in range(repeat):
                _emit_body(nc, env)

    nc.compile()
    return nc


def _emit_body(nc, env):
    jt, lc = env["jt"], env["lc"]
    xq_sb, xkv_sb = env["xq_sb"], env["xkv_sb"]
    wq_sb, wk_sb, wv_sb, wo_sb = (env[k] for k in
                                  ["wq_sb", "wk_sb", "wv_sb", "wo_sb"])
    qT_sb, kT_sb, v_sb, ot_sb, mb_sb = (env[k] for k in
                                        ["qT_sb", "kT_sb", "v_sb", "ot_sb",
                                         "mb_sb"])
    v_view = env["v_view"]
    pt_pool, recp, scp = env["pt_pool"], env["recp"], env["scp"]
    stps, smps = env["stps"], env["smps"]
    xq, xkv, wq, wk, wv, wo, mb, out = (env[k] for k in
                                        ["xq", "xkv", "wq", "wk", "wv",
                                         "wo", "mb", "out"])

    # ---- input loads ----
    # Loads are ordered by first PE use: xq k0 + pair-0 Q weights, the
    # rest of xq (k-tile streamed under the q0 projection), pair-0 K
    # weights, xkv, then the filler-unit inputs (wq pairs 1-3, wv, wk
    # pairs 1-3) and finally wo. Three rules learned from the timeline:
    # (1) the DMA transfer queue drains in request order, so loads must
    # reach it strictly in consumption order -- a lower-priority load on
    # a faster-clearing queue cuts the line and delays ST p0;
    # (2) ALL loads go on sync: a dma_start holds its issuing engine's
    # SEQ through the serialized shared HWDGE descriptor-gen stage, so
    # scalar-issued loads would block the prologue ACT copies (which hold
    # the pj PSUM ring!) and the exp stream behind ~8us of queued issues;
    # (3) gpsimd's SWDGE path (1us serialized issue, separate queue that
    # would jump the HWDGE stream) only carries the tiny mb load.
    def ld_split(dst, src, c0, c1, nsplit):
        step = 128 // nsplit
        for s in range(nsplit):
            p0, p1 = s * step, (s + 1) * step
            nc.sync.dma_start(out=dst[p0:p1, c0:c1], in_=src[p0:p1, c0:c1])

    PW = KT * 128  # cols per pair in the pair-major wq/wk layouts
    # xq k0 rides the gpsimd SWDGE path: its descriptor-gen overlaps the
    # sync HWDGE chain, so the first matmul's deps land ~0.7us earlier
    nc.gpsimd.dma_start(out=xq_sb[:, 0:N], in_=xq[:, 0:N])
    nc.gpsimd.dma_start(out=mb_sb[:], in_=mb[:, :])
    ld_split(wq_sb, wq, 0, 256, 1)         # 64KB: q0 k0/k1 matmul deps
    ld_split(wq_sb, wq, 256, PW, 1)
    for k in range(1, KT):
        ld_split(xq_sb, xq, k * N, (k + 1) * N, 1)
    ld_split(wk_sb, wk, 0, PW, 1)
    qx = KT * lc // 4
    for s in range(4):
        ld_split(xkv_sb, xkv, s * qx, (s + 1) * qx, 1)
    ld_split(wq_sb, wq, PW, 2 * PW, 1)     # q1 (early pair-0 filler)
    ld_split(wv_sb, wv, 0, KT * DG, 2)      # v_proj fillers (pair 0)
    ld_split(wq_sb, wq, 2 * PW, 3 * PW, 1)  # q2
    ld_split(wk_sb, wk, PW, 2 * PW, 1)      # k1 (pair-0 last filler)
    ld_split(wq_sb, wq, 3 * PW, 4 * PW, 1)  # q3 (pair-1 filler)
    ld_split(wk_sb, wk, 2 * PW, 4 * PW, 2)
    ld_split(wo_sb, wo, 0, PAIRS * D, 2)

    # k-proj output chunking (lc may exceed one PSUM bank)
    kch = [(0, min(512, lc))] + ([(512, lc)] if lc > 512 else [])

    def make_proj(dst_sb, w_sb, x_sb, p, cols, chunks, on_act=False):
        """Projection for pair p as two units sharing PSUM accumulators.

        chunks: list of (c0, c1) output-column ranges (<=512 wide each).
        on_act: do the PSUM->SBUF copies on the idle ACT engine (prologue
        projections), keeping the DVE stream free.
        """
        ps = [None] * len(chunks)

        def half(k0, k1):
            for ci, (c0, c1) in enumerate(chunks):
                if k0 == 0:
                    ps[ci] = smps.tile([128, c1 - c0], f32,
                                       name="pj%d" % ci, tag="pj")
            for k in range(k0, k1):
                w = w_sb[:, (p * KT + k) * 128: (p * KT + k + 1) * 128]
                for ci, (c0, c1) in enumerate(chunks):
                    nc.tensor.matmul(
                        ps[ci][:],
                        lhsT=w,
                        rhs=x_sb[:, k * cols + c0: k * cols + c1],
                        start=(k == 0), stop=(k == KT - 1))
            if k1 == KT:
                for ci, (c0, c1) in enumerate(chunks):
                    dst = dst_sb[:, p * cols + c0: p * cols + c1]
                    if on_act:
                        nc.scalar.activation(
                            out=dst, in_=ps[ci][:],
                            func=mybir.ActivationFunctionType.Copy)
                    else:
                        nc.vector.tensor_copy(out=dst, in_=ps[ci][:])

        return [lambda: half(0, KT // 2), lambda: half(KT // 2, KT)]

    def proj_q(p, on_act=False):
        return make_proj(qT_sb, wq_sb, xq_sb, p, N, [(0, 512), (512, 1024)],
                         on_act)

    def proj_k(p, on_act=False):
        return make_proj(kT_sb, wk_sb, xkv_sb, p, lc, kch, on_act)

    def v_proj(j):
        """V[j, c] = x_kv @ Wv_g for one j tile (interleaved dst halves)."""
        ps = smps.tile([128, 512], f32, tag="av", bufs=1)
        for k in range(KT):
            nc.tensor.matmul(
                ps[:],
                lhsT=xkv_sb[:, k * lc + j * 128: k * lc + (j + 1) * 128],
                rhs=wv_sb[:, k * DG:(k + 1) * DG],
                start=(k == 0), stop=(k == KT - 1))
        pv = ps[:].rearrange("p (h c) -> p h c", h=HG)
        # head A (even) V goes to cols DH:VW, head B (odd) to cols 0:DH
        nc.vector.tensor_copy(out=v_view[:, j, 0::2, DH:VW], in_=pv[:, 0::2])
        nc.vector.tensor_copy(out=v_view[:, j, 1::2, 0:DH], in_=pv[:, 1::2])

    def norm(po, p, hh, chunks=1, rec_act=False):
        """Normalize one head's AV straight into ot_sb (no DMA).

        hh=0 (head A, V_ext=[ones|V]): denom rows 0:64, O^T rows 64:128;
        mul shifts the PSUM operand down into ot rows 0:64.
        hh=1 (head B, V_ext=[V|ones]): O^T rows 0:64 shifts up into ot
        rows 64:128; recip/out SBUF operands aligned at partition 64.
        The reciprocal runs full-height base-aligned; the unused half is
        garbage (1/O^T values) that is never read. denom > ~1 always, so
        the fast approx reciprocal's denorm/inf edge cases can't occur
        on the rows that are consumed.
        """
        rec_t = recp.tile([128, 1024], f32)
        dn, ot_rows = ((slice(0, 64), slice(64, 128)) if hh == 0
                       else (slice(64, 128), slice(0, 64)))
        step = 1024 // chunks
        for c in range(chunks):
            cs = slice(c * step, (c + 1) * step)
            nc.vector.reciprocal_approx_fast(out=rec_t[:, cs],
                                             in_=po[:, cs])
            nc.vector.tensor_mul(
                out=ot_sb[dn, p * N + c * step: p * N + (c + 1) * step],
                in0=po[ot_rows, cs], in1=rec_t[dn, cs])

    def av_head(p, hh, pt, pool_tag="av", chunks=1, rec_act=False,
                skip_norm=False):
        """AV for head 2p+hh, both i chunks, one V weight load per j."""
        st = {}
        for u in av_head_units(p, hh, pt, pool_tag, chunks, rec_act, 1,
                               skip_norm, st):
            u()
        return st.get("po")

    def av_head_units(p, hh, pt, pool_tag="av", chunks=1, rec_act=False,
                      nunits=2, skip_norm=False, state=None):
        """AV for one head as nunits filler units (j-ranges + final norm)."""
        h = 2 * p + hh
        if state is None:
            state = {}

        def run(j0, j1):
            if j0 == 0:
                state["po"] = (
                    stps.tile([128, 1024], f32, name="po", tag="st")
                    if pool_tag == "st" else
                    smps.tile([128, 1024], f32, name="po", tag="av", bufs=1))
            po = state["po"]
            for j in range(j0, j1):
                vblk = v_sb[:, (j * HG + h) * VW: (j * HG + h + 1) * VW]
                nc.tensor.matmul(po[:, 0:512], lhsT=vblk,
                                 rhs=pt[:, j * N: j * N + 512],
                                 start=(j == 0), stop=(j == jt - 1))
                nc.tensor.matmul(po[:, 512:1024], lhsT=vblk,
                                 rhs=pt[:, j * N + 512: (j + 1) * N],
                                 start=(j == 0), stop=(j == jt - 1))
            if j1 == jt and not skip_norm:
                norm(po, p, hh, chunks=chunks, rec_act=rec_act)

        bounds = [jt * i // nunits for i in range(nunits + 1)]
        return [lambda a=a, b=b: run(a, b)
                for a, b in zip(bounds[:-1], bounds[1:])]

    def st_pair(p, pa, pb, slot_units):
        """ST + exp for pair p; filler units interleaved into HALF-j slots
        (2*jt of them) so the exp stream is fed a fresh ST half roughly
        every exp-duration instead of in bursts."""
        for j in range(jt):
            for half, (rb, pt) in enumerate(((0, pa), (64, pb))):
                ps = stps.tile([128, 1024], f32, name="st", tag="st")
                kk = kT_sb[rb:rb + 64, p * lc + j * 128: p * lc + (j + 1) * 128]
                for ic in range(IC):
                    cols = slice(ic * 512, ic * 512 + 512)
                    nc.tensor.matmul(
                        ps[:, cols], lhsT=kk,
                        rhs=qT_sb[rb:rb + 64,
                                  p * N + ic * 512: p * N + ic * 512 + 512],
                        start=True, stop=True)
                # EXP right after this half's matmuls: ACT starts earlier
                # and the pool slot frees a half-j sooner
                nc.scalar.activation(
                    out=pt[:, j * N:(j + 1) * N], in_=ps[:],
                    func=mybir.ActivationFunctionType.Exp,
                    bias=mb_sb[:, j:j + 1], scale=DP_SCALE)
                for u in slot_units.get(2 * j + half, []):
                    u()

    def op_mms(it, ps0, ps1, cts):
        for ct in cts:
            ot_blk = ot_sb[:, ct * N + it * 128: ct * N + (it + 1) * 128]
            nc.tensor.matmul(
                ps0[:], lhsT=ot_blk,
                rhs=wo_sb[:, ct * D: ct * D + 512],
                start=(ct == 0), stop=(ct == PAIRS - 1))
            nc.tensor.matmul(
                ps1[:], lhsT=ot_blk,
                rhs=wo_sb[:, ct * D + 512: ct * D + 1024],
                start=(ct == 0), stop=(ct == PAIRS - 1))

    def op_finish(it, ps0, ps1):
        # Copies split across DVE+ACT (both idle here). Early tiles store
        # whole on the idle Pool SWDGE path (separate issue queue, keeps
        # HWDGE clear); the last two tiles are latency-critical:
        # column-half stores on sync/scalar, each depending only on its
        # own engine's copy so neither store cross-waits the other engine.
        out_t = scp.tile([128, 1024], bf16, tag="outt")
        nc.vector.tensor_copy(out=out_t[:, 0:512], in_=ps0[:])
        nc.scalar.activation(out=out_t[:, 512:1024], in_=ps1[:],
                             func=mybir.ActivationFunctionType.Copy)
        r = slice(it * 128, (it + 1) * 128)
        if it < IT - 2:
            nc.gpsimd.dma_start(out=out[r, :], in_=out_t[:])
        elif it < IT - 1:
            # both halves on sync: a scalar-issued store here would hold
            # Activation.SEQ through HWDGE gen and delay the LAST tile's
            # ACT copy
            nc.sync.dma_start(out=out[r, 0:512], in_=out_t[:, 0:512])
            nc.sync.dma_start(out=out[r, 512:1024], in_=out_t[:, 512:1024])
        else:
            nc.sync.dma_start(out=out[r, 0:512], in_=out_t[:, 0:512])
            nc.scalar.dma_start(out=out[r, 512:1024],
                                in_=out_t[:, 512:1024])

    # ---- prologue: q0 then k0 only (minimum work before the ST stream
    # starts); q1-q3 stream in as pair-0/1 fillers as their weights land
    for u in proj_q(0):
        u()
    for u in proj_k(0):
        u()

    # ---- pipelined pairs ----
    prev = None
    for p in range(PAIRS):
        pa = pt_pool.tile([128, jt * N], bf16, tag="pa")
        pb = pt_pool.tile([128, jt * N], bf16, tag="pb")

        if p == 0:
            # DMA-paced fillers in arrival order: q1, q2, v tiles (wv),
            # q3 waits for pair 1, k1 last (its weights land last)
            proj_units = (proj_q(1)
                          + [lambda j=j: v_proj(j) for j in range(jt)]
                          + proj_q(2) + proj_k(1))
        elif p + 1 < PAIRS:
            proj_units = list(proj_k(p + 1))
            if p == 1:
                proj_units = proj_q(3) + proj_units
        else:
            proj_units = []
        if prev is not None:
            # av heads use the single "av" PSUM buffer: keep them apart so
            # head B's alloc never stalls the PE on head A's norms
            pp, ppa, ppb = prev
            na = len(proj_units) // 2
            units = ([lambda: av_head(pp, 0, ppa)] + proj_units[:na]
                     + [lambda: av_head(pp, 1, ppb)] + proj_units[na:])
        else:
            units = proj_units

        slot_units = {}
        nslots = 2 * jt
        # monotonic slot assignment keeps each proj's k0-half before its
        # k1-half (they share PSUM accumulators); +2 phantom units lean
        # the distribution toward late slots so the pair tail (when the
        # exp stream still drains) keeps PE fed
        for i, u in enumerate(units):
            slot_units.setdefault(
                min(nslots - 1, (i + 5) * nslots // (len(units) + 5)),
                []).append(u)
        st_pair(p, pa, pb, slot_units)
        prev = (p, pa, pb)

    # last pair's AV: head A first (its PT's exps finish ~1us before head
    # B's -- st_pair emits the a-half exp before the b-half per j), then
    # head B from the ST banks. The norms are hand-interleaved in column
    # halves so the DVE chain delivers the first 512 normalized query
    # columns (both head rows) as early as possible for ct=3.
    pp, ppa, ppb = prev
    poA = av_head(pp, 0, ppa, skip_norm=True)
    recA_t = recp.tile([128, 1024], f32)
    nc.vector.reciprocal_approx_fast(out=recA_t[:], in_=poA[:])
    poB = av_head(pp, 1, ppb, pool_tag="st", skip_norm=True)
    recB_t = recp.tile([128, 1024], f32)
    for c in range(2):
        cs = slice(c * 512, (c + 1) * 512)
        oc = slice(pp * N + c * 512, pp * N + (c + 1) * 512)
        nc.vector.reciprocal_approx_fast(out=recB_t[:, cs], in_=poB[:, cs])
        # head A (V_ext=[ones|V]): O^T rows 64:128 -> ot rows 0:64
        nc.vector.tensor_mul(out=ot_sb[0:64, oc], in0=poA[64:128, cs],
                             in1=recA_t[0:64, cs])
        # head B (V_ext=[V|ones]): O^T rows 0:64 -> ot rows 64:128
        nc.vector.tensor_mul(out=ot_sb[64:128, oc], in0=poB[0:64, cs],
                             in1=recB_t[64:128, cs])

    # ---- output projection: partial[i, d] in bf16 ----
    # Runway: i-tiles 1-2 accumulate pairs 0-2 while the last norms run on
    # DVE; PSUM plan fills all 8 banks: AVB po (av, 2) + AVA po (st, 2) +
    # pend1 (st, 2) + pend2 (pj, 1+1). The O-proj loop then rotates
    # st/pj/av so tile allocs never wait on a two-deep copy pipeline.
    def op_psum(which):
        if which == "pj":
            o0 = smps.tile([128, 512], f32, name="o0", tag="pj")
            o1 = smps.tile([128, 512], f32, name="o1", tag="pj")
            return o0[:], o1[:]
        if which == "av":
            pw = smps.tile([128, 1024], f32, name="po", tag="av", bufs=1)
        else:
            pw = stps.tile([128, 1024], f32, name="pw", tag="st")
        return pw[:, 0:512], pw[:, 512:1024]

    pend = {}
    for it, pool in ((1, "st"), (2, "pj")):
        pend[it] = op_psum(pool)
        op_mms(it, *pend[it], range(PAIRS - 1))
    rot = ["av", "st", "st", "pj", "av", "st"]
    for it in range(IT):
        if it in pend:
            ps0, ps1 = pend[it]
            op_mms(it, ps0, ps1, range(PAIRS - 1, PAIRS))
        else:
            ps0, ps1 = op_psum(rot.pop(0))
            op_mms(it, ps0, ps1, range(PAIRS))
        op_finish(it, ps0, ps1)


_NC_CACHE = {}


def _get_nc(jt):
    if jt not in _NC_CACHE:
        _NC_CACHE[jt] = build_nc(jt=jt)
    return _NC_CACHE[jt]


def _tile_k(a, cols):
    """[KT*128, cols] -> [128, KT*cols] k-tile-major, contiguous bf16."""
    return np.ascontiguousarray(
        a.reshape(KT, 128, cols).transpose(1, 0, 2).reshape(128, KT * cols)
    ).astype(BF16)


def _make_in_maps(x_q, x_kv, pad_mask, Wq, Wk, Wv, Wo, jt=None):
    pad_mask = np.asarray(pad_mask)
    cnts = (~pad_mask).sum(axis=1)
    if jt is None:
        jt = max(1, int(-(-int(cnts.max()) // 128)))
    lc = jt * 128

    def _tile_pair(w):
        # [D, DG] -> [128, pair*KT*128 + k*128 + c] pair-major
        return np.ascontiguousarray(
            w.reshape(KT, 128, PAIRS, 128).transpose(1, 2, 0, 3)
            .reshape(128, PAIRS * KT * 128)).astype(BF16)

    per_g = []
    for g in range(2):
        cols = slice(g * DG, (g + 1) * DG)
        per_g.append({
            "wq": _tile_pair(np.ascontiguousarray(Wq[:, cols])),
            "wk": _tile_pair(np.ascontiguousarray(Wk[:, cols])),
            "wv": _tile_k(np.ascontiguousarray(Wv[:, cols]), DG),
            "wo": np.ascontiguousarray(
                Wo[g * DG:(g + 1) * DG, :]
                .reshape(PAIRS, 128, D).transpose(1, 0, 2)
                .reshape(128, PAIRS * D)).astype(BF16),
        })
    per_b = []
    for b in range(B):
        idx = np.flatnonzero(~pad_mask[b])
        n = len(idx)
        xc = np.zeros((lc, D), dtype=np.float32)
        xc[:n] = x_kv[b][idx]
        mbias = np.full(lc, MASK_NEG, dtype=np.float32)
        mbias[:n] = 0.0
        per_b.append({
            "xq": _tile_k(np.ascontiguousarray(x_q[b].T), N),
            "xkv": _tile_k(np.ascontiguousarray(xc.T), lc),
            "mb": np.ascontiguousarray(mbias.reshape(jt, 128).T),
        })

    in_maps = []
    for c in range(NCORES):
        b, g = c // 2, c % 2
        in_maps.append({**per_b[b], **per_g[g]})
    return in_maps, jt


def kernel(x_q, x_kv, pad_mask, Wq, Wk, Wv, Wo, bo):
    in_maps, jt = _make_in_maps(x_q, x_kv, pad_mask, Wq, Wk, Wv, Wo)
    nc = _get_nc(jt)
    res = run_bass_kernel_spmd(nc, in_maps, core_ids=list(range(NCORES)))
    full = np.empty((B, N, D), dtype=np.float32)
    bo32 = bo.astype(np.float32)
    for b in range(B):
        full[b] = (res.results[2 * b]["out"].astype(np.float32)
                   + res.results[2 * b + 1]["out"].astype(np.float32))
        full[b] += bo32
    return full


# revision 105
# speedup vs baseline: 1.0435x; 1.0169x over previous
"""Trainium2 Bass kernel for nn_MultiHeadAttention_37512244363503.

Sharding: 8 cores = 4 batches x 2 head-groups (8 heads each).
Per core (b, g): Wq/Wk/Wv column-sliced, Wo row-sliced; the host sums the
two partial outputs per batch (the row-parallel "all-reduce") and adds bo.

Key compaction: pad_mask is host-visible, so masked keys are dropped on
the host before upload. Keys compact to jt*128 columns (jt chosen from
the max per-batch unmasked count, 5 for the reference distribution),
cutting K/V projections, scores, softmax and AV by L_c/L. Padding slots
get a -30000 exp-bias so they contribute exactly 0.

All DRAM inputs are pre-tiled on host to the SBUF k-tile-major layout so
every load is a contiguous [128, cols] copy (no strided descriptors).

Per-core algorithm (matmuls bf16 in / fp32 PSUM accumulate):
  QT[d,i]   = Wq_g.T @ x_q[b].T        (d=512 cols of this group)
  KT[d,j]   = Wk_g.T @ x_kv_c[b].T     (j over compacted keys)
  V[j,c]    = x_kv_c[b] @ Wv_g         (per-head [ones|V_h] / [V_h|ones])
  ST[j,i]   = K_h Q_h.T per head       (2 heads packed via PE row groups)
  PT        = exp(ST/8 + mask_bias[j]) (ACT; bias rides the ACT bias input)
  po        = V_ext.T @ PT             (denom + O^T in one matmul)
  O_norm^T  = po_OT * recip(po_denom)  (DVE; written straight into ot_sb)
  partial   = O_norm @ Wo_g            (bf16 partial -> DRAM, host sums)

Norms write directly into ot_sb (no SBUF->SBUF DMA): head A (hh=0) keeps
V_ext = [ones|V] so denom sits at PSUM rows 0:64 and O^T at 64:128; the
DVE mul shifts the PSUM operand down to write ot rows 0:64. Head B flips
V_ext = [V|ones] so its mul writes ot rows 64:128 with the SBUF operands
(recip, out) partition-aligned at 64. The fast reciprocal always reads
the full [128,*] PSUM tile base-aligned (unused rows are garbage but
never read; the custom DVE op corrupts on shifted APs, so base-aligned
full-height is the only safe form).

Schedule (TimelineSim 96.0us vs 116.4us for the previous build; PE busy
78.8us ~= the bf16 cycle floor of this dataflow):
  prologue  q0 + k0 only, DMA-paced (loads stream in consumption order)
  pair p    ST halves + exp, with fillers slotted between half-j's:
            p0: q1, v_proj x jt, q2, k1; p1: AV0, q3, k2; p2: AV1, k3;
            p3: AV2 (filler distribution back-loaded: the exp stream is
            the per-pair pacer, ~11.5us/pair vs ~10.7us of PE work)
  endgame   AV3-A (full-norm) -> AV3-B (column-chunked norm), runway
            i-tiles 1-2 (pairs 0-2) hide the norm latency, then the
            O-projection drains with an av/st/st/av/pj/st PSUM rotation
"""

import numpy as np
import ml_dtypes

import concourse.bass as bass
import concourse.mybir as mybir
from concourse import bacc
from concourse.tile import TileContext
from concourse.bass_utils import run_bass_kernel_spmd

BF16 = ml_dtypes.bfloat16

B, N, L, D, H = 4, 1024, 1024, 1024, 16
DH = D // H           # 64 channels per head
HG = 8                # heads per core
DG = HG * DH          # 512 channels per core
NCORES = 8
DP_SCALE = DH ** -0.5
MASK_NEG = -30000.0   # exp(x + MASK_NEG) underflows to exactly 0.0

f32 = mybir.dt.float32
bf16 = mybir.dt.bfloat16

KT = D // 128         # 8 k-tiles in the contraction dim of projections
IT = N // 128         # 8 query tiles
IC = N // 512         # 2 query chunks (PSUM free dim)
PAIRS = HG // 2       # 4 head pairs (2 heads packed per 128 partitions)
VW = 2 * DH           # 128 cols per (j, head) V_ext block


def build_nc(jt=5, lc=None, debug=False, num_devices=NCORES, repeat=1):
    # lc may be below jt*128: the last j-tile holds lc-(jt-1)*128 keys
    # (rounded to 32 on the host). Key-tile costs that scale with tile
    # WIDTH (K-proj columns, xkv bytes) shrink; query-dim-driven costs
    # (ST/AV matmul cols, exp free-size) do not.
    if lc is None:
        lc = jt * 128
    nc = bacc.Bacc("TRN2", target_bir_lowering=False, debug=False,
                   num_devices=num_devices)

    xq = nc.dram_tensor("xq", [128, KT * N], bf16, kind="ExternalInput")
    xkv = nc.dram_tensor("xkv", [128, KT * lc], bf16, kind="ExternalInput")
    # wq/wk are PAIR-major ([128, pair*KT*128 + k*128 + c]) so one pair's
    # projection weights are a single contiguous 256KB slice loadable first
    wq = nc.dram_tensor("wq", [128, KT * DG], bf16, kind="ExternalInput")
    wk = nc.dram_tensor("wk", [128, KT * DG], bf16, kind="ExternalInput")
    wv = nc.dram_tensor("wv", [128, KT * DG], bf16, kind="ExternalInput")
    wo = nc.dram_tensor("wo", [128, PAIRS * D], bf16, kind="ExternalInput")
    mb = nc.dram_tensor("mb", [128, jt], f32, kind="ExternalInput")
    out = nc.dram_tensor("out", [N, D], bf16, kind="ExternalOutput")

    with TileContext(nc) as tc:
        with (
            tc.tile_pool(name="persist", bufs=1) as persist,
            tc.tile_pool(name="pt", bufs=3) as pt_pool,
            tc.tile_pool(name="recp", bufs=4) as recp,
            tc.tile_pool(name="scp", bufs=5) as scp,
            tc.tile_pool(name="stps", bufs=2, space="PSUM") as stps,
            tc.tile_pool(name="smps", bufs=2, space="PSUM") as smps,
        ):
            env = dict(jt=jt, lc=lc)
            for nm, shape in [
                ("xq_sb", [128, KT * N]), ("xkv_sb", [128, KT * lc]),
                ("wq_sb", [128, KT * DG]), ("wk_sb", [128, KT * DG]),
                ("wv_sb", [128, KT * DG]), ("wo_sb", [128, PAIRS * D]),
                ("qT_sb", [128, PAIRS * N]), ("kT_sb", [128, PAIRS * lc]),
                ("v_sb", [128, jt * HG * VW]), ("ot_sb", [128, PAIRS * N]),
            ]:
                env[nm] = persist.tile(shape, bf16, name=nm)
            env["mb_sb"] = persist.tile([128, jt], f32, name="mb_sb")
            env["v_view"] = env["v_sb"][:].rearrange(
                "p (j h c) -> p j h c", j=jt, h=HG)
            # Per-head V_ext layout: head A (even h) = [ones | V_h] so the
            # softmax denominator lands at PSUM rows 0:64 (base-aligned for
            # the custom reciprocal) and O^T at 64:128 (the shiftable PSUM
            # mul operand, written to ot rows 0:64). Head B (odd h) =
            # [V_h | ones]: O^T at rows 0:64 shifts down into ot rows
            # 64:128 with recip/out SBUF operands aligned at 64.
            nc.vector.memset(env["v_view"][:, :, 0::2, 0:DH], 1.0)
            nc.vector.memset(env["v_view"][:, :, 1::2, DH:VW], 1.0)
            env.update(pt_pool=pt_pool, recp=recp, scp=scp, stps=stps,
                       smps=smps, xq=xq, xkv=xkv, wq=wq, wk=wk, wv=wv,
                       wo=wo, mb=mb, out=out)
            for _rep in range(repeat):
                _emit_body(nc, env)

    nc.compile()
    return nc


PH0 = 5


def _emit_body(nc, env):
    jt, lc = env["jt"], env["lc"]
    xq_sb, xkv_sb = env["xq_sb"], env["xkv_sb"]
    wq_sb, wk_sb, wv_sb, wo_sb = (env[k] for k in
                                  ["wq_sb", "wk_sb", "wv_sb", "wo_sb"])
    qT_sb, kT_sb, v_sb, ot_sb, mb_sb = (env[k] for k in
                                        ["qT_sb", "kT_sb", "v_sb", "ot_sb",
                                         "mb_sb"])
    v_view = env["v_view"]
    pt_pool, recp, scp = env["pt_pool"], env["recp"], env["scp"]
    stps, smps = env["stps"], env["smps"]
    xq, xkv, wq, wk, wv, wo, mb, out = (env[k] for k in
                                        ["xq", "xkv", "wq", "wk", "wv",
                                         "wo", "mb", "out"])

    # ---- input loads ----
    # Loads are ordered by first PE use: xq k0 + pair-0 Q weights, the
    # rest of xq (k-tile streamed under the q0 projection), pair-0 K
    # weights, xkv, then the filler-unit inputs (wq pairs 1-3, wv, wk
    # pairs 1-3) and finally wo. Three rules learned from the timeline:
    # (1) the DMA transfer queue drains in request order, so loads must
    # reach it strictly in consumption order -- a lower-priority load on
    # a faster-clearing queue cuts the line and delays ST p0;
    # (2) ALL loads go on sync: a dma_start holds its issuing engine's
    # SEQ through the serialized shared HWDGE descriptor-gen stage, so
    # scalar-issued loads would block the prologue ACT copies (which hold
    # the pj PSUM ring!) and the exp stream behind ~8us of queued issues;
    # (3) gpsimd's SWDGE path (1us serialized issue, separate queue that
    # would jump the HWDGE stream) only carries the tiny mb load.
    def ld_split(dst, src, c0, c1, nsplit):
        step = 128 // nsplit
        for s in range(nsplit):
            p0, p1 = s * step, (s + 1) * step
            nc.sync.dma_start(out=dst[p0:p1, c0:c1], in_=src[p0:p1, c0:c1])

    PW = KT * 128  # cols per pair in the pair-major wq/wk layouts
    # xq k0 rides the gpsimd SWDGE path: its descriptor-gen overlaps the
    # sync HWDGE chain, so the first matmul's deps land ~0.7us earlier
    nc.gpsimd.dma_start(out=xq_sb[:, 0:N], in_=xq[:, 0:N])
    nc.gpsimd.dma_start(out=mb_sb[:], in_=mb[:, :])
    ld_split(wq_sb, wq, 0, 256, 1)         # 64KB: q0 k0/k1 matmul deps
    ld_split(wq_sb, wq, 256, PW, 1)
    for k in range(1, KT):
        ld_split(xq_sb, xq, k * N, (k + 1) * N, 1)
    ld_split(wk_sb, wk, 0, PW, 1)
    qx = KT * lc // 4
    for s in range(4):
        ld_split(xkv_sb, xkv, s * qx, (s + 1) * qx, 1)
    ld_split(wq_sb, wq, PW, 2 * PW, 1)     # q1 (early pair-0 filler)
    ld_split(wv_sb, wv, 0, KT * DG, 2)      # v_proj fillers (pair 0)
    ld_split(wq_sb, wq, 2 * PW, 3 * PW, 1)  # q2
    ld_split(wk_sb, wk, PW, 2 * PW, 1)      # k1 (pair-0 last filler)
    ld_split(wq_sb, wq, 3 * PW, 4 * PW, 1)  # q3 (pair-1 filler)
    ld_split(wk_sb, wk, 2 * PW, 4 * PW, 2)
    ld_split(wo_sb, wo, 0, PAIRS * D, 2)

    # k-proj output chunking (lc may exceed one PSUM bank)
    kch = [(0, min(512, lc))] + ([(512, lc)] if lc > 512 else [])

    def make_proj(dst_sb, w_sb, x_sb, p, cols, chunks, on_act=False,
                  on_st=False):
        """Projection for pair p as two units sharing PSUM accumulators.

        chunks: list of (c0, c1) output-column ranges (<=512 wide each).
        on_act: do the PSUM->SBUF copies on the idle ACT engine.
        on_st: take chunk accumulators as slices of one 'st'-pool tile
        (prologue k0 only: the pj ring is still held by q0, and the ST
        pool is idle before the pair loop starts).
        """
        ps = [None] * len(chunks)

        def half(k0, k1):
            if k0 == 0 and on_st:
                pw = stps.tile([128, 1024], f32, name="kpj", tag="st")
                off = 0
                for ci, (c0, c1) in enumerate(chunks):
                    ps[ci] = pw[:, off:off + (c1 - c0)]
                    off += c1 - c0
            for ci, (c0, c1) in enumerate(chunks):
                if k0 == 0 and not on_st:
                    ps[ci] = smps.tile([128, c1 - c0], f32,
                                       name="pj%d" % ci, tag="pj")
            for k in range(k0, k1):
                w = w_sb[:, (p * KT + k) * 128: (p * KT + k + 1) * 128]
                for ci, (c0, c1) in enumerate(chunks):
                    nc.tensor.matmul(
                        ps[ci][:],
                        lhsT=w,
                        rhs=x_sb[:, k * cols + c0: k * cols + c1],
                        start=(k == 0), stop=(k == KT - 1))
            if k1 == KT:
                for ci, (c0, c1) in enumerate(chunks):
                    dst = dst_sb[:, p * cols + c0: p * cols + c1]
                    if on_act:
                        nc.scalar.activation(
                            out=dst, in_=ps[ci][:],
                            func=mybir.ActivationFunctionType.Copy)
                    else:
                        nc.vector.tensor_copy(out=dst, in_=ps[ci][:])

        return [lambda: half(0, KT // 2), lambda: half(KT // 2, KT)]

    def proj_q(p, on_act=False):
        return make_proj(qT_sb, wq_sb, xq_sb, p, N, [(0, 512), (512, 1024)],
                         on_act)

    def proj_k(p, on_act=False, on_st=False):
        return make_proj(kT_sb, wk_sb, xkv_sb, p, lc, kch, on_act, on_st)

    def jw(j):
        # keys in j-tile j (the last tile may be narrower than 128)
        return min(128, lc - j * 128)

    def v_proj(j):
        """V[j, c] = x_kv @ Wv_g for one j tile (interleaved dst halves)."""
        w = jw(j)
        ps = smps.tile([128, 512], f32, tag="av", bufs=1)
        for k in range(KT):
            nc.tensor.matmul(
                ps[0:w, :],
                lhsT=xkv_sb[:, k * lc + j * 128: k * lc + j * 128 + w],
                rhs=wv_sb[:, k * DG:(k + 1) * DG],
                start=(k == 0), stop=(k == KT - 1))
        pv = ps[0:w].rearrange("p (h c) -> p h c", h=HG)
        # head A (even) V goes to cols DH:VW, head B (odd) to cols 0:DH
        nc.vector.tensor_copy(out=v_view[0:w, j, 0::2, DH:VW],
                              in_=pv[:, 0::2])
        nc.vector.tensor_copy(out=v_view[0:w, j, 1::2, 0:DH],
                              in_=pv[:, 1::2])

    def norm(po, p, hh, chunks=1, rec_act=False):
        """Normalize one head's AV straight into ot_sb (no DMA).

        hh=0 (head A, V_ext=[ones|V]): denom rows 0:64, O^T rows 64:128;
        mul shifts the PSUM operand down into ot rows 0:64.
        hh=1 (head B, V_ext=[V|ones]): O^T rows 0:64 shifts up into ot
        rows 64:128; recip/out SBUF operands aligned at partition 64.
        The reciprocal runs full-height base-aligned; the unused half is
        garbage (1/O^T values) that is never read. denom > ~1 always, so
        the fast approx reciprocal's denorm/inf edge cases can't occur
        on the rows that are consumed.
        """
        rec_t = recp.tile([128, 1024], f32)
        dn, ot_rows = ((slice(0, 64), slice(64, 128)) if hh == 0
                       else (slice(64, 128), slice(0, 64)))
        step = 1024 // chunks
        for c in range(chunks):
            cs = slice(c * step, (c + 1) * step)
            nc.vector.reciprocal_approx_fast(out=rec_t[:, cs],
                                             in_=po[:, cs])
            nc.vector.tensor_mul(
                out=ot_sb[dn, p * N + c * step: p * N + (c + 1) * step],
                in0=po[ot_rows, cs], in1=rec_t[dn, cs])

    def av_head(p, hh, pt, pool_tag="av", chunks=1, rec_act=False,
                skip_norm=False):
        """AV for head 2p+hh, both i chunks, one V weight load per j."""
        st = {}
        for u in av_head_units(p, hh, pt, pool_tag, chunks, rec_act, 1,
                               skip_norm, st):
            u()
        return st.get("po")

    def av_head_units(p, hh, pt, pool_tag="av", chunks=1, rec_act=False,
                      nunits=2, skip_norm=False, state=None):
        """AV for one head as nunits filler units (j-ranges + final norm)."""
        h = 2 * p + hh
        if state is None:
            state = {}

        def run(j0, j1):
            if j0 == 0:
                state["po"] = (
                    stps.tile([128, 1024], f32, name="po", tag="st")
                    if pool_tag == "st" else
                    smps.tile([128, 1024], f32, name="po", tag="av", bufs=1))
            po = state["po"]
            for j in range(j0, j1):
                w = jw(j)
                vblk = v_sb[0:w, (j * HG + h) * VW: (j * HG + h + 1) * VW]
                nc.tensor.matmul(po[:, 0:512], lhsT=vblk,
                                 rhs=pt[0:w, j * N: j * N + 512],
                                 start=(j == 0), stop=(j == jt - 1))
                nc.tensor.matmul(po[:, 512:1024], lhsT=vblk,
                                 rhs=pt[0:w, j * N + 512: (j + 1) * N],
                                 start=(j == 0), stop=(j == jt - 1))
            if j1 == jt and not skip_norm:
                norm(po, p, hh, chunks=chunks, rec_act=rec_act)

        bounds = [jt * i // nunits for i in range(nunits + 1)]
        return [lambda a=a, b=b: run(a, b)
                for a, b in zip(bounds[:-1], bounds[1:])]

    def st_pair(p, pa, pb, slot_units):
        """ST + exp for pair p; filler units interleaved into HALF-j slots
        (2*jt of them) so the exp stream is fed a fresh ST half roughly
        every exp-duration instead of in bursts."""
        for j in range(jt):
            for half, (rb, pt) in enumerate(((0, pa), (64, pb))):
                w = jw(j)
                ps = stps.tile([128, 1024], f32, name="st", tag="st")
                kk = kT_sb[rb:rb + 64, p * lc + j * 128: p * lc + j * 128 + w]
                for ic in range(IC):
                    cols = slice(ic * 512, ic * 512 + 512)
                    nc.tensor.matmul(
                        ps[0:w, cols], lhsT=kk,
                        rhs=qT_sb[rb:rb + 64,
                                  p * N + ic * 512: p * N + ic * 512 + 512],
                        start=True, stop=True)
                # EXP right after this half's matmuls: ACT starts earlier
                # and the pool slot frees a half-j sooner
                nc.scalar.activation(
                    out=pt[:, j * N:(j + 1) * N], in_=ps[:],
                    func=mybir.ActivationFunctionType.Exp,
                    bias=mb_sb[:, j:j + 1], scale=DP_SCALE)
                for u in slot_units.get(2 * j + half, []):
                    u()

    def op_mms(it, ps0, ps1, cts):
        for ct in cts:
            ot_blk = ot_sb[:, ct * N + it * 128: ct * N + (it + 1) * 128]
            nc.tensor.matmul(
                ps0[:], lhsT=ot_blk,
                rhs=wo_sb[:, ct * D: ct * D + 512],
                start=(ct == 0), stop=(ct == PAIRS - 1))
            nc.tensor.matmul(
                ps1[:], lhsT=ot_blk,
                rhs=wo_sb[:, ct * D + 512: ct * D + 1024],
                start=(ct == 0), stop=(ct == PAIRS - 1))

    def op_finish(it, ps0, ps1):
        # Copies split across DVE+ACT (both idle here). Early tiles store
        # whole on the idle Pool SWDGE path (separate issue queue, keeps
        # HWDGE clear); the last two tiles are latency-critical:
        # column-half stores on sync/scalar, each depending only on its
        # own engine's copy so neither store cross-waits the other engine.
        out_t = scp.tile([128, 1024], bf16, tag="outt")
        if it == IT - 1:
            # last tile: ps1's half finishes first (split matmul order) --
            # ACT takes it so its copy+store chain hides under ps0's mms
            nc.scalar.activation(out=out_t[:, 512:1024], in_=ps1[:],
                                 func=mybir.ActivationFunctionType.Copy)
            nc.vector.tensor_copy(out=out_t[:, 0:512], in_=ps0[:])
        else:
            nc.vector.tensor_copy(out=out_t[:, 0:512], in_=ps0[:])
            nc.scalar.activation(out=out_t[:, 512:1024], in_=ps1[:],
                                 func=mybir.ActivationFunctionType.Copy)
        r = slice(it * 128, (it + 1) * 128)
        if it < IT - 2:
            # split across the SWDGE and sync queues: on HW a single
            # dma_start rides one ~17GB/s queue, so a 256KB store would
            # take ~14us and outlive the latency-critical final stores
            r0 = it * 128
            nc.gpsimd.dma_start(out=out[r0:r0 + 64, :], in_=out_t[0:64, :])
            nc.sync.dma_start(out=out[r0 + 64:r0 + 128, :],
                              in_=out_t[64:128, :])
        elif it < IT - 1:
            # both halves on sync: a scalar-issued store here would hold
            # Activation.SEQ through HWDGE gen and delay the LAST tile's
            # ACT copy
            nc.sync.dma_start(out=out[r, 0:512], in_=out_t[:, 0:512])
            nc.sync.dma_start(out=out[r, 512:1024], in_=out_t[:, 512:1024])
        else:
            nc.scalar.dma_start(out=out[r, 512:1024],
                                in_=out_t[:, 512:1024])
            nc.sync.dma_start(out=out[r, 0:512], in_=out_t[:, 0:512])

    # ---- prologue: q0 then k0 only (minimum work before the ST stream
    # starts); q1-q3 stream in as pair-0/1 fillers as their weights land
    for u in proj_q(0):
        u()
    for u in proj_k(0, on_st=True):
        u()

    # ---- pipelined pairs ----
    prev = None
    for p in range(PAIRS):
        pa = pt_pool.tile([128, jt * N], bf16, tag="pa")
        pb = pt_pool.tile([128, jt * N], bf16, tag="pb")

        if p == 0:
            # DMA-paced fillers in arrival order: q1, q2, v tiles (wv),
            # q3 waits for pair 1, k1 last (its weights land last)
            proj_units = (proj_q(1)
                          + [lambda j=j: v_proj(j) for j in range(jt)]
                          + proj_q(2) + proj_k(1))
        elif p + 1 < PAIRS:
            proj_units = list(proj_k(p + 1))
            if p == 1:
                proj_units = proj_q(3) + proj_units
        else:
            proj_units = []
        if prev is not None:
            # av heads use the single "av" PSUM buffer: keep them apart so
            # head B's alloc never stalls the PE on head A's norms. The
            # last pair has no projection fillers, so its AV fillers split
            # into sub-units for a finer interleave with the exp stream.
            pp, ppa, ppb = prev
            nu = 2 if not proj_units else 1
            na = len(proj_units) // 2
            units = (av_head_units(pp, 0, ppa, nunits=nu)
                     + proj_units[:na]
                     + av_head_units(pp, 1, ppb, nunits=nu)
                     + proj_units[na:])
        else:
            units = proj_units

        slot_units = {}
        nslots = 2 * jt
        # monotonic slot assignment keeps each proj's k0-half before its
        # k1-half (they share PSUM accumulators); phantom units lean the
        # distribution toward late slots so the pair tail (when the exp
        # stream still drains) keeps PE fed. Pair 0's fillers are
        # DMA-arrival-paced rather than exp-paced, so it gets its own
        # weighting.
        ph = PH0 if p == 0 else 5
        for i, u in enumerate(units):
            slot_units.setdefault(
                min(nslots - 1, (i + ph) * nslots // (len(units) + ph)),
                []).append(u)
        st_pair(p, pa, pb, slot_units)
        prev = (p, pa, pb)

    # last pair's AV: head A first (its PT's exps finish ~1us before head
    # B's -- st_pair emits the a-half exp before the b-half per j), then
    # head B from the ST banks. The norms are hand-interleaved in column
    # halves so the DVE chain delivers the first 512 normalized query
    # columns (both head rows) as early as possible for ct=3.
    pp, ppa, ppb = prev
    poA = av_head(pp, 0, ppa, skip_norm=True)
    recA_t = recp.tile([128, 1024], f32)
    nc.vector.reciprocal_approx_fast(out=recA_t[:], in_=poA[:])
    poB = av_head(pp, 1, ppb, pool_tag="st", skip_norm=True)
    recB_t = recp.tile([128, 1024], f32)
    for c in range(2):
        cs = slice(c * 512, (c + 1) * 512)
        oc = slice(pp * N + c * 512, pp * N + (c + 1) * 512)
        nc.vector.reciprocal_approx_fast(out=recB_t[:, cs], in_=poB[:, cs])
        # head A (V_ext=[ones|V]): O^T rows 64:128 -> ot rows 0:64
        nc.vector.tensor_mul(out=ot_sb[0:64, oc], in0=poA[64:128, cs],
                             in1=recA_t[0:64, cs])
        # head B (V_ext=[V|ones]): O^T rows 0:64 -> ot rows 64:128
        nc.vector.tensor_mul(out=ot_sb[64:128, oc], in0=poB[0:64, cs],
                             in1=recB_t[64:128, cs])

    # ---- output projection: partial[i, d] in bf16 ----
    # Runway: i-tiles 1-2 accumulate pairs 0-2 while the last norms run on
    # DVE; PSUM plan fills all 8 banks: AVA po (av, 2) + AVB po (st, 2) +
    # pend1 (st, 2) + pend2 (pj, 1+1). The O-proj loop then rotates
    # av/st/pj so tile allocs never wait on a two-deep copy pipeline.
    def op_psum(which):
        if which == "pj":
            o0 = smps.tile([128, 512], f32, name="o0", tag="pj")
            o1 = smps.tile([128, 512], f32, name="o1", tag="pj")
            return o0[:], o1[:]
        if which == "av":
            pw = smps.tile([128, 1024], f32, name="po", tag="av", bufs=1)
        else:
            pw = stps.tile([128, 1024], f32, name="pw", tag="st")
        return pw[:, 0:512], pw[:, 512:1024]

    pend = {}
    for it, pool in ((1, "st"), (2, "pj")):
        pend[it] = op_psum(pool)
        op_mms(it, *pend[it], range(PAIRS - 1))
    rot = ["av", "st", "st", "av", "pj", "st"]
    # interleave the cheap pend tiles (2 matmuls each) between full tiles
    # so their finish copies don't burst-flood the DVE/ACT queues
    for it in (0, 1, 3, 2, 4, 5, 6, 7):
        if it in pend:
            ps0, ps1 = pend[it]
            op_mms(it, ps0, ps1, range(PAIRS - 1, PAIRS))
        else:
            ps0, ps1 = op_psum(rot.pop(0))
            op_mms(it, ps0, ps1, range(PAIRS))
        op_finish(it, ps0, ps1)


_NC_CACHE = {}


def _get_nc(jt, lc=None):
    key = (jt, lc)
    if key not in _NC_CACHE:
        _NC_CACHE[key] = build_nc(jt=jt, lc=lc)
    return _NC_CACHE[key]


def _tile_k(a, cols):
    """[KT*128, cols] -> [128, KT*cols] k-tile-major, contiguous bf16."""
    return np.ascontiguousarray(
        a.reshape(KT, 128, cols).transpose(1, 0, 2).reshape(128, KT * cols)
    ).astype(BF16)


def _make_in_maps(x_q, x_kv, pad_mask, Wq, Wk, Wv, Wo, jt=None):
    pad_mask = np.asarray(pad_mask)
    cnts = (~pad_mask).sum(axis=1)
    if jt is None:
        jt = max(1, int(-(-int(cnts.max()) // 128)))
    # last tile rounded to 8 keys: K-proj columns and xkv bytes shrink
    lc = min(jt * 128, max(8, int(-(-int(cnts.max()) // 8)) * 8))
    lc = max(lc, (jt - 1) * 128 + 8)

    def _tile_pair(w):
        # [D, DG] -> [128, pair*KT*128 + k*128 + c] pair-major
        return np.ascontiguousarray(
            w.reshape(KT, 128, PAIRS, 128).transpose(1, 2, 0, 3)
            .reshape(128, PAIRS * KT * 128)).astype(BF16)

    per_g = []
    for g in range(2):
        cols = slice(g * DG, (g + 1) * DG)
        per_g.append({
            "wq": _tile_pair(np.ascontiguousarray(Wq[:, cols])),
            "wk": _tile_pair(np.ascontiguousarray(Wk[:, cols])),
            "wv": _tile_k(np.ascontiguousarray(Wv[:, cols]), DG),
            "wo": np.ascontiguousarray(
                Wo[g * DG:(g + 1) * DG, :]
                .reshape(PAIRS, 128, D).transpose(1, 0, 2)
                .reshape(128, PAIRS * D)).astype(BF16),
        })
    per_b = []
    for b in range(B):
        idx = np.flatnonzero(~pad_mask[b])
        n = len(idx)
        xc = np.zeros((lc, D), dtype=np.float32)
        xc[:n] = x_kv[b][idx]
        mbias = np.full(jt * 128, MASK_NEG, dtype=np.float32)
        mbias[:n] = 0.0
        per_b.append({
            "xq": _tile_k(np.ascontiguousarray(x_q[b].T), N),
            "xkv": _tile_k(np.ascontiguousarray(xc.T), lc),
            "mb": np.ascontiguousarray(mbias.reshape(jt, 128).T),
        })

    in_maps = []
    for c in range(NCORES):
        b, g = c // 2, c % 2
        in_maps.append({**per_b[b], **per_g[g]})
    return in_maps, (jt, lc)


def kernel(x_q, x_kv, pad_mask, Wq, Wk, Wv, Wo, bo):
    in_maps, (jt, lc) = _make_in_maps(x_q, x_kv, pad_mask, Wq, Wk, Wv, Wo)
    nc = _get_nc(jt, lc)
    res = run_bass_kernel_spmd(nc, in_maps, core_ids=list(range(NCORES)))
    full = np.empty((B, N, D), dtype=np.float32)
    bo32 = bo.astype(np.float32)
    for b in range(B):
        full[b] = (res.results[2 * b]["out"].astype(np.float32)
                   + res.results[2 * b + 1]["out"].astype(np.float32))
        full[b] += bo32
    return full


# revision 108
# speedup vs baseline: 1.0456x; 1.0019x over previous
"""Trainium2 Bass kernel for nn_MultiHeadAttention_37512244363503.

Sharding: 8 cores = 4 batches x 2 head-groups (8 heads each).
Per core (b, g): Wq/Wk/Wv column-sliced, Wo row-sliced; the host sums the
two partial outputs per batch (the row-parallel "all-reduce") and adds bo.

Key compaction: pad_mask is host-visible, so masked keys are dropped on
the host before upload. Keys compact to jt*128 columns (jt chosen from
the max per-batch unmasked count, 5 for the reference distribution),
cutting K/V projections, scores, softmax and AV by L_c/L. Padding slots
get a -30000 exp-bias so they contribute exactly 0.

All DRAM inputs are pre-tiled on host to the SBUF k-tile-major layout so
every load is a contiguous [128, cols] copy (no strided descriptors).

Per-core algorithm (matmuls bf16 in / fp32 PSUM accumulate):
  QT[d,i]   = Wq_g.T @ x_q[b].T        (d=512 cols of this group)
  KT[d,j]   = Wk_g.T @ x_kv_c[b].T     (j over compacted keys)
  V[j,c]    = x_kv_c[b] @ Wv_g         (per-head [ones|V_h] / [V_h|ones])
  ST[j,i]   = K_h Q_h.T per head       (2 heads packed via PE row groups)
  PT        = exp(ST/8 + mask_bias[j]) (ACT; bias rides the ACT bias input)
  po        = V_ext.T @ PT             (denom + O^T in one matmul)
  O_norm^T  = po_OT * recip(po_denom)  (DVE; written straight into ot_sb)
  partial   = O_norm @ Wo_g            (bf16 partial -> DRAM, host sums)

Norms write directly into ot_sb (no SBUF->SBUF DMA): head A (hh=0) keeps
V_ext = [ones|V] so denom sits at PSUM rows 0:64 and O^T at 64:128; the
DVE mul shifts the PSUM operand down to write ot rows 0:64. Head B flips
V_ext = [V|ones] so its mul writes ot rows 64:128 with the SBUF operands
(recip, out) partition-aligned at 64. The fast reciprocal always reads
the full [128,*] PSUM tile base-aligned (unused rows are garbage but
never read; the custom DVE op corrupts on shifted APs, so base-aligned
full-height is the only safe form).

Schedule (TimelineSim 94.8us vs 116.4us for the previous build; PE busy
78.8us ~= the bf16 cycle floor of this dataflow):
  prologue  q0 + k0 only, DMA-paced (loads stream in consumption order)
  pair p    ST halves + exp, with fillers slotted between half-j's:
            p0: q1, v_proj x jt, q2, k1; p1: AV0, q3, k2; p2: AV1, k3;
            p3: AV2 (filler distribution back-loaded: the exp stream is
            the per-pair pacer, ~11.5us/pair vs ~10.7us of PE work)
  endgame   AV3-A (full-norm) -> AV3-B (column-chunked norm), runway
            i-tiles 1-2 (pairs 0-2) hide the norm latency, then the
            O-projection drains with an av/st/st/av/pj/st PSUM rotation
"""

import numpy as np
import ml_dtypes

import concourse.bass as bass
import concourse.mybir as mybir
from concourse import bacc
from concourse.tile import TileContext
from concourse.bass_utils import run_bass_kernel_spmd

BF16 = ml_dtypes.bfloat16

B, N, L, D, H = 4, 1024, 1024, 1024, 16
DH = D // H           # 64 channels per head
HG = 8                # heads per core
DG = HG * DH          # 512 channels per core
NCORES = 8
DP_SCALE = DH ** -0.5
MASK_NEG = -30000.0   # exp(x + MASK_NEG) underflows to exactly 0.0

f32 = mybir.dt.float32
bf16 = mybir.dt.bfloat16

KT = D // 128         # 8 k-tiles in the contraction dim of projections
IT = N // 128         # 8 query tiles
IC = N // 512         # 2 query chunks (PSUM free dim)
PAIRS = HG // 2       # 4 head pairs (2 heads packed per 128 partitions)
VW = 2 * DH           # 128 cols per (j, head) V_ext block


def build_nc(jt=5, lc=None, debug=False, num_devices=NCORES, repeat=1):
    # lc may be below jt*128: the last j-tile holds lc-(jt-1)*128 keys
    # (rounded to 32 on the host). Key-tile costs that scale with tile
    # WIDTH (K-proj columns, xkv bytes) shrink; query-dim-driven costs
    # (ST/AV matmul cols, exp free-size) do not.
    if lc is None:
        lc = jt * 128
    nc = bacc.Bacc("TRN2", target_bir_lowering=False, debug=False,
                   num_devices=num_devices)

    xq = nc.dram_tensor("xq", [128, KT * N], bf16, kind="ExternalInput")
    xkv = nc.dram_tensor("xkv", [128, KT * lc], bf16, kind="ExternalInput")
    # wq/wk are PAIR-major ([128, pair*KT*128 + k*128 + c]) so one pair's
    # projection weights are a single contiguous 256KB slice loadable first
    wq = nc.dram_tensor("wq", [128, KT * DG], bf16, kind="ExternalInput")
    wk = nc.dram_tensor("wk", [128, KT * DG], bf16, kind="ExternalInput")
    wv = nc.dram_tensor("wv", [128, KT * DG], bf16, kind="ExternalInput")
    wo = nc.dram_tensor("wo", [128, PAIRS * D], bf16, kind="ExternalInput")
    mb = nc.dram_tensor("mb", [128, jt], f32, kind="ExternalInput")
    out = nc.dram_tensor("out", [N, D], bf16, kind="ExternalOutput")

    with TileContext(nc) as tc:
        with (
            tc.tile_pool(name="persist", bufs=1) as persist,
            tc.tile_pool(name="pt", bufs=3) as pt_pool,
            tc.tile_pool(name="recp", bufs=4) as recp,
            tc.tile_pool(name="scp", bufs=5) as scp,
            tc.tile_pool(name="stps", bufs=2, space="PSUM") as stps,
            tc.tile_pool(name="smps", bufs=2, space="PSUM") as smps,
        ):
            env = dict(jt=jt, lc=lc)
            for nm, shape in [
                ("xq_sb", [128, KT * N]), ("xkv_sb", [128, KT * lc]),
                ("wq_sb", [128, KT * DG]), ("wk_sb", [128, KT * DG]),
                ("wv_sb", [128, KT * DG]), ("wo_sb", [128, PAIRS * D]),
                ("qT_sb", [128, PAIRS * N]), ("kT_sb", [128, PAIRS * lc]),
                ("v_sb", [128, jt * HG * VW]), ("ot_sb", [128, PAIRS * N]),
            ]:
                env[nm] = persist.tile(shape, bf16, name=nm)
            env["mb_sb"] = persist.tile([128, jt], f32, name="mb_sb")
            env["v_view"] = env["v_sb"][:].rearrange(
                "p (j h c) -> p j h c", j=jt, h=HG)
            # Per-head V_ext layout: head A (even h) = [ones | V_h] so the
            # softmax denominator lands at PSUM rows 0:64 (base-aligned for
            # the custom reciprocal) and O^T at 64:128 (the shiftable PSUM
            # mul operand, written to ot rows 0:64). Head B (odd h) =
            # [V_h | ones]: O^T at rows 0:64 shifts down into ot rows
            # 64:128 with recip/out SBUF operands aligned at 64.
            nc.vector.memset(env["v_view"][:, :, 0::2, 0:DH], 1.0)
            nc.vector.memset(env["v_view"][:, :, 1::2, DH:VW], 1.0)
            env.update(pt_pool=pt_pool, recp=recp, scp=scp, stps=stps,
                       smps=smps, xq=xq, xkv=xkv, wq=wq, wk=wk, wv=wv,
                       wo=wo, mb=mb, out=out)
            for _rep in range(repeat):
                _emit_body(nc, env)

    nc.compile()
    return nc


PH0 = 5


def _emit_body(nc, env):
    jt, lc = env["jt"], env["lc"]
    xq_sb, xkv_sb = env["xq_sb"], env["xkv_sb"]
    wq_sb, wk_sb, wv_sb, wo_sb = (env[k] for k in
                                  ["wq_sb", "wk_sb", "wv_sb", "wo_sb"])
    qT_sb, kT_sb, v_sb, ot_sb, mb_sb = (env[k] for k in
                                        ["qT_sb", "kT_sb", "v_sb", "ot_sb",
                                         "mb_sb"])
    v_view = env["v_view"]
    pt_pool, recp, scp = env["pt_pool"], env["recp"], env["scp"]
    stps, smps = env["stps"], env["smps"]
    xq, xkv, wq, wk, wv, wo, mb, out = (env[k] for k in
                                        ["xq", "xkv", "wq", "wk", "wv",
                                         "wo", "mb", "out"])

    # ---- input loads ----
    # Loads are ordered by first PE use: xq k0 + pair-0 Q weights, the
    # rest of xq (k-tile streamed under the q0 projection), pair-0 K
    # weights, xkv, then the filler-unit inputs (wq pairs 1-3, wv, wk
    # pairs 1-3) and finally wo. Three rules learned from the timeline:
    # (1) the DMA transfer queue drains in request order, so loads must
    # reach it strictly in consumption order -- a lower-priority load on
    # a faster-clearing queue cuts the line and delays ST p0;
    # (2) ALL loads go on sync: a dma_start holds its issuing engine's
    # SEQ through the serialized shared HWDGE descriptor-gen stage, so
    # scalar-issued loads would block the prologue ACT copies (which hold
    # the pj PSUM ring!) and the exp stream behind ~8us of queued issues;
    # (3) gpsimd's SWDGE path (1us serialized issue, separate queue that
    # would jump the HWDGE stream) only carries the tiny mb load.
    def ld_split(dst, src, c0, c1, nsplit):
        step = 128 // nsplit
        for s in range(nsplit):
            p0, p1 = s * step, (s + 1) * step
            nc.sync.dma_start(out=dst[p0:p1, c0:c1], in_=src[p0:p1, c0:c1])

    PW = KT * 128  # cols per pair in the pair-major wq/wk layouts
    # xq k0 rides the gpsimd SWDGE path: its descriptor-gen overlaps the
    # sync HWDGE chain, so the first matmul's deps land ~0.7us earlier
    nc.gpsimd.dma_start(out=xq_sb[:, 0:N], in_=xq[:, 0:N])
    nc.gpsimd.dma_start(out=mb_sb[:], in_=mb[:, :])
    ld_split(wq_sb, wq, 0, 256, 1)         # 64KB: q0 k0/k1 matmul deps
    ld_split(wq_sb, wq, 256, PW, 1)
    for k in range(1, KT):
        ld_split(xq_sb, xq, k * N, (k + 1) * N, 1)
    ld_split(wk_sb, wk, 0, PW, 1)
    qx = KT * lc // 4
    for s in range(4):
        ld_split(xkv_sb, xkv, s * qx, (s + 1) * qx, 1)
    ld_split(wq_sb, wq, PW, 2 * PW, 1)     # q1 (early pair-0 filler)
    ld_split(wv_sb, wv, 0, KT * DG, 2)      # v_proj fillers (pair 0)
    ld_split(wq_sb, wq, 2 * PW, 3 * PW, 1)  # q2
    ld_split(wk_sb, wk, PW, 2 * PW, 1)      # k1 (pair-0 last filler)
    ld_split(wq_sb, wq, 3 * PW, 4 * PW, 1)  # q3 (pair-1 filler)
    ld_split(wk_sb, wk, 2 * PW, 4 * PW, 2)
    ld_split(wo_sb, wo, 0, PAIRS * D, 2)

    # k-proj output chunking (lc may exceed one PSUM bank)
    kch = [(0, min(512, lc))] + ([(512, lc)] if lc > 512 else [])

    def make_proj(dst_sb, w_sb, x_sb, p, cols, chunks, on_act=False,
                  on_st=False):
        """Projection for pair p as two units sharing PSUM accumulators.

        chunks: list of (c0, c1) output-column ranges (<=512 wide each).
        on_act: do the PSUM->SBUF copies on the idle ACT engine.
        on_st: take chunk accumulators as slices of one 'st'-pool tile
        (prologue k0 only: the pj ring is still held by q0, and the ST
        pool is idle before the pair loop starts).
        """
        ps = [None] * len(chunks)

        def half(k0, k1):
            if k0 == 0 and on_st:
                pw = stps.tile([128, 1024], f32, name="kpj", tag="st")
                off = 0
                for ci, (c0, c1) in enumerate(chunks):
                    ps[ci] = pw[:, off:off + (c1 - c0)]
                    off += c1 - c0
            for ci, (c0, c1) in enumerate(chunks):
                if k0 == 0 and not on_st:
                    ps[ci] = smps.tile([128, c1 - c0], f32,
                                       name="pj%d" % ci, tag="pj")
            for k in range(k0, k1):
                w = w_sb[:, (p * KT + k) * 128: (p * KT + k + 1) * 128]
                for ci, (c0, c1) in enumerate(chunks):
                    nc.tensor.matmul(
                        ps[ci][:],
                        lhsT=w,
                        rhs=x_sb[:, k * cols + c0: k * cols + c1],
                        start=(k == 0), stop=(k == KT - 1))
            if k1 == KT:
                for ci, (c0, c1) in enumerate(chunks):
                    dst = dst_sb[:, p * cols + c0: p * cols + c1]
                    if on_act:
                        nc.scalar.activation(
                            out=dst, in_=ps[ci][:],
                            func=mybir.ActivationFunctionType.Copy)
                    else:
                        nc.vector.tensor_copy(out=dst, in_=ps[ci][:])

        return [lambda: half(0, KT // 2), lambda: half(KT // 2, KT)]

    def proj_q(p, on_act=False):
        return make_proj(qT_sb, wq_sb, xq_sb, p, N, [(0, 512), (512, 1024)],
                         on_act)

    def proj_k(p, on_act=False, on_st=False):
        return make_proj(kT_sb, wk_sb, xkv_sb, p, lc, kch, on_act, on_st)

    def jw(j):
        # keys in j-tile j (the last tile may be narrower than 128)
        return min(128, lc - j * 128)

    def v_proj(j):
        """V[j, c] = x_kv @ Wv_g for one j tile (interleaved dst halves)."""
        w = jw(j)
        ps = smps.tile([128, 512], f32, tag="av", bufs=1)
        for k in range(KT):
            nc.tensor.matmul(
                ps[0:w, :],
                lhsT=xkv_sb[:, k * lc + j * 128: k * lc + j * 128 + w],
                rhs=wv_sb[:, k * DG:(k + 1) * DG],
                start=(k == 0), stop=(k == KT - 1))
        pv = ps[0:w].rearrange("p (h c) -> p h c", h=HG)
        # head A (even) V goes to cols DH:VW, head B (odd) to cols 0:DH
        nc.vector.tensor_copy(out=v_view[0:w, j, 0::2, DH:VW],
                              in_=pv[:, 0::2])
        nc.vector.tensor_copy(out=v_view[0:w, j, 1::2, 0:DH],
                              in_=pv[:, 1::2])

    def norm(po, p, hh, chunks=1, rec_act=False):
        """Normalize one head's AV straight into ot_sb (no DMA).

        hh=0 (head A, V_ext=[ones|V]): denom rows 0:64, O^T rows 64:128;
        mul shifts the PSUM operand down into ot rows 0:64.
        hh=1 (head B, V_ext=[V|ones]): O^T rows 0:64 shifts up into ot
        rows 64:128; recip/out SBUF operands aligned at partition 64.
        The reciprocal runs full-height base-aligned; the unused half is
        garbage (1/O^T values) that is never read. denom > ~1 always, so
        the fast approx reciprocal's denorm/inf edge cases can't occur
        on the rows that are consumed.
        """
        rec_t = recp.tile([128, 1024], f32)
        dn, ot_rows = ((slice(0, 64), slice(64, 128)) if hh == 0
                       else (slice(64, 128), slice(0, 64)))
        step = 1024 // chunks
        for c in range(chunks):
            cs = slice(c * step, (c + 1) * step)
            nc.vector.reciprocal_approx_fast(out=rec_t[:, cs],
                                             in_=po[:, cs])
            nc.vector.tensor_mul(
                out=ot_sb[dn, p * N + c * step: p * N + (c + 1) * step],
                in0=po[ot_rows, cs], in1=rec_t[dn, cs])

    def av_head(p, hh, pt, pool_tag="av", chunks=1, rec_act=False,
                skip_norm=False):
        """AV for head 2p+hh, both i chunks, one V weight load per j."""
        st = {}
        for u in av_head_units(p, hh, pt, pool_tag, chunks, rec_act, 1,
                               skip_norm, st):
            u()
        return st.get("po")

    def av_head_units(p, hh, pt, pool_tag="av", chunks=1, rec_act=False,
                      nunits=2, skip_norm=False, state=None):
        """AV for one head as nunits filler units (j-ranges + final norm)."""
        h = 2 * p + hh
        if state is None:
            state = {}

        def run(j0, j1):
            if j0 == 0:
                state["po"] = (
                    stps.tile([128, 1024], f32, name="po", tag="st")
                    if pool_tag == "st" else
                    smps.tile([128, 1024], f32, name="po", tag="av", bufs=1))
            po = state["po"]
            for j in range(j0, j1):
                w = jw(j)
                vblk = v_sb[0:w, (j * HG + h) * VW: (j * HG + h + 1) * VW]
                nc.tensor.matmul(po[:, 0:512], lhsT=vblk,
                                 rhs=pt[0:w, j * N: j * N + 512],
                                 start=(j == 0), stop=(j == jt - 1))
                nc.tensor.matmul(po[:, 512:1024], lhsT=vblk,
                                 rhs=pt[0:w, j * N + 512: (j + 1) * N],
                                 start=(j == 0), stop=(j == jt - 1))
            if j1 == jt and not skip_norm:
                norm(po, p, hh, chunks=chunks, rec_act=rec_act)

        bounds = [jt * i // nunits for i in range(nunits + 1)]
        return [lambda a=a, b=b: run(a, b)
                for a, b in zip(bounds[:-1], bounds[1:])]

    def st_pair(p, pa, pb, slot_units):
        """ST + exp for pair p; filler units interleaved into HALF-j slots
        (2*jt of them) so the exp stream is fed a fresh ST half roughly
        every exp-duration instead of in bursts."""
        for j in range(jt):
            for half, (rb, pt) in enumerate(((0, pa), (64, pb))):
                w = jw(j)
                ps = stps.tile([128, 1024], f32, name="st", tag="st")
                kk = kT_sb[rb:rb + 64, p * lc + j * 128: p * lc + j * 128 + w]
                for ic in range(IC):
                    cols = slice(ic * 512, ic * 512 + 512)
                    nc.tensor.matmul(
                        ps[0:w, cols], lhsT=kk,
                        rhs=qT_sb[rb:rb + 64,
                                  p * N + ic * 512: p * N + ic * 512 + 512],
                        start=True, stop=True)
                # EXP right after this half's matmuls: ACT starts earlier
                # and the pool slot frees a half-j sooner
                nc.scalar.activation(
                    out=pt[:, j * N:(j + 1) * N], in_=ps[:],
                    func=mybir.ActivationFunctionType.Exp,
                    bias=mb_sb[:, j:j + 1], scale=DP_SCALE)
                for u in slot_units.get(2 * j + half, []):
                    u()

    def op_mms(it, ps0, ps1, cts):
        for ct in cts:
            ot_blk = ot_sb[:, ct * N + it * 128: ct * N + (it + 1) * 128]
            nc.tensor.matmul(
                ps0[:], lhsT=ot_blk,
                rhs=wo_sb[:, ct * D: ct * D + 512],
                start=(ct == 0), stop=(ct == PAIRS - 1))
            nc.tensor.matmul(
                ps1[:], lhsT=ot_blk,
                rhs=wo_sb[:, ct * D + 512: ct * D + 1024],
                start=(ct == 0), stop=(ct == PAIRS - 1))

    def op_finish(it, ps0, ps1):
        # Copies split across DVE+ACT (both idle here). Early tiles store
        # whole on the idle Pool SWDGE path (separate issue queue, keeps
        # HWDGE clear); the last two tiles are latency-critical:
        # column-half stores on sync/scalar, each depending only on its
        # own engine's copy so neither store cross-waits the other engine.
        out_t = scp.tile([128, 1024], bf16, tag="outt")
        if it == IT - 1:
            # last tile: ps1's half finishes first (split matmul order) --
            # ACT takes it so its copy+store chain hides under ps0's mms
            nc.scalar.activation(out=out_t[:, 512:1024], in_=ps1[:],
                                 func=mybir.ActivationFunctionType.Copy)
            nc.vector.tensor_copy(out=out_t[:, 0:512], in_=ps0[:])
        else:
            nc.vector.tensor_copy(out=out_t[:, 0:512], in_=ps0[:])
            nc.scalar.activation(out=out_t[:, 512:1024], in_=ps1[:],
                                 func=mybir.ActivationFunctionType.Copy)
        r = slice(it * 128, (it + 1) * 128)
        if it < IT - 2:
            # split across the SWDGE and sync queues: on HW a single
            # dma_start rides one ~17GB/s queue, so a 256KB store would
            # take ~14us and outlive the latency-critical final stores
            r0 = it * 128
            nc.gpsimd.dma_start(out=out[r0:r0 + 64, :], in_=out_t[0:64, :])
            nc.sync.dma_start(out=out[r0 + 64:r0 + 128, :],
                              in_=out_t[64:128, :])
        elif it < IT - 1:
            # both halves on sync: a scalar-issued store here would hold
            # Activation.SEQ through HWDGE gen and delay the LAST tile's
            # ACT copy
            nc.sync.dma_start(out=out[r, 0:512], in_=out_t[:, 0:512])
            nc.sync.dma_start(out=out[r, 512:1024], in_=out_t[:, 512:1024])
        else:
            nc.scalar.dma_start(out=out[r, 512:1024],
                                in_=out_t[:, 512:1024])
            nc.sync.dma_start(out=out[r, 0:512], in_=out_t[:, 0:512])

    # ---- prologue: q0 then k0 only (minimum work before the ST stream
    # starts); q1-q3 stream in as pair-0/1 fillers as their weights land
    for u in proj_q(0):
        u()
    for u in proj_k(0, on_st=True):
        u()

    # ---- pipelined pairs ----
    prev = None
    for p in range(PAIRS):
        pa = pt_pool.tile([128, jt * N], bf16, tag="pa")
        pb = pt_pool.tile([128, jt * N], bf16, tag="pb")

        if p == 0:
            # DMA-paced fillers in arrival order: q1, q2, v tiles (wv),
            # q3 waits for pair 1, k1 last (its weights land last)
            proj_units = (proj_q(1)
                          + [lambda j=j: v_proj(j) for j in range(jt)]
                          + proj_q(2) + proj_k(1))
        elif p + 1 < PAIRS:
            proj_units = list(proj_k(p + 1))
            if p == 1:
                proj_units = proj_q(3) + proj_units
        else:
            proj_units = []
        if prev is not None:
            # av heads use the single "av" PSUM buffer: keep them apart so
            # head B's alloc never stalls the PE on head A's norms. The
            # last pair has no projection fillers, so its AV fillers split
            # into sub-units for a finer interleave with the exp stream.
            pp, ppa, ppb = prev
            nu = 2 if not proj_units else 1
            na = len(proj_units) // 2
            units = (av_head_units(pp, 0, ppa, nunits=nu)
                     + proj_units[:na]
                     + av_head_units(pp, 1, ppb, nunits=nu)
                     + proj_units[na:])
        else:
            units = proj_units

        slot_units = {}
        nslots = 2 * jt
        # monotonic slot assignment keeps each proj's k0-half before its
        # k1-half (they share PSUM accumulators); phantom units lean the
        # distribution toward late slots so the pair tail (when the exp
        # stream still drains) keeps PE fed. Pair 0's fillers are
        # DMA-arrival-paced rather than exp-paced, so it gets its own
        # weighting.
        ph = PH0 if p == 0 else 3
        for i, u in enumerate(units):
            slot_units.setdefault(
                min(nslots - 1, (i + ph) * nslots // (len(units) + ph)),
                []).append(u)
        st_pair(p, pa, pb, slot_units)
        prev = (p, pa, pb)

    # last pair's AV: head A first (its PT's exps finish ~1us before head
    # B's -- st_pair emits the a-half exp before the b-half per j), then
    # head B from the ST banks. The norms are hand-interleaved in column
    # halves so the DVE chain delivers the first 512 normalized query
    # columns (both head rows) as early as possible for ct=3.
    pp, ppa, ppb = prev
    poA = av_head(pp, 0, ppa, skip_norm=True)
    recA_t = recp.tile([128, 1024], f32)
    nc.vector.reciprocal_approx_fast(out=recA_t[:], in_=poA[:])
    poB = av_head(pp, 1, ppb, pool_tag="st", skip_norm=True)
    recB_t = recp.tile([128, 1024], f32)
    for c in range(2):
        cs = slice(c * 512, (c + 1) * 512)
        oc = slice(pp * N + c * 512, pp * N + (c + 1) * 512)
        nc.vector.reciprocal_approx_fast(out=recB_t[:, cs], in_=poB[:, cs])
        # head A (V_ext=[ones|V]): O^T rows 64:128 -> ot rows 0:64
        nc.vector.tensor_mul(out=ot_sb[0:64, oc], in0=poA[64:128, cs],
                             in1=recA_t[0:64, cs])
        # head B (V_ext=[V|ones]): O^T rows 0:64 -> ot rows 64:128
        nc.vector.tensor_mul(out=ot_sb[64:128, oc], in0=poB[0:64, cs],
                             in1=recB_t[64:128, cs])

    # ---- output projection: partial[i, d] in bf16 ----
    # Runway: i-tiles 1-2 accumulate pairs 0-2 while the last norms run on
    # DVE; PSUM plan fills all 8 banks: AVA po (av, 2) + AVB po (st, 2) +
    # pend1 (st, 2) + pend2 (pj, 1+1). The O-proj loop then rotates
    # av/st/pj so tile allocs never wait on a two-deep copy pipeline.
    def op_psum(which):
        if which == "pj":
            o0 = smps.tile([128, 512], f32, name="o0", tag="pj")
            o1 = smps.tile([128, 512], f32, name="o1", tag="pj")
            return o0[:], o1[:]
        if which == "av":
            pw = smps.tile([128, 1024], f32, name="po", tag="av", bufs=1)
        else:
            pw = stps.tile([128, 1024], f32, name="pw", tag="st")
        return pw[:, 0:512], pw[:, 512:1024]

    pend = {}
    for it, pool in ((1, "st"), (2, "pj")):
        pend[it] = op_psum(pool)
        op_mms(it, *pend[it], range(PAIRS - 1))
    rot = ["av", "st", "st", "av", "pj", "st"]
    # interleave the cheap pend tiles (2 matmuls each) between full tiles
    # so their finish copies don't burst-flood the DVE/ACT queues
    for it in (0, 1, 3, 2, 4, 5, 6, 7):
        if it in pend:
            ps0, ps1 = pend[it]
            op_mms(it, ps0, ps1, range(PAIRS - 1, PAIRS))
        else:
            ps0, ps1 = op_psum(rot.pop(0))
            op_mms(it, ps0, ps1, range(PAIRS))
        op_finish(it, ps0, ps1)


_NC_CACHE = {}


def _get_nc(jt, lc=None):
    key = (jt, lc)
    if key not in _NC_CACHE:
        _NC_CACHE[key] = build_nc(jt=jt, lc=lc)
    return _NC_CACHE[key]


def _tile_k(a, cols):
    """[KT*128, cols] -> [128, KT*cols] k-tile-major, contiguous bf16."""
    return np.ascontiguousarray(
        a.reshape(KT, 128, cols).transpose(1, 0, 2).reshape(128, KT * cols)
    ).astype(BF16)


def _make_in_maps(x_q, x_kv, pad_mask, Wq, Wk, Wv, Wo, jt=None):
    pad_mask = np.asarray(pad_mask)
    cnts = (~pad_mask).sum(axis=1)
    if jt is None:
        jt = max(1, int(-(-int(cnts.max()) // 128)))
    # last tile rounded to 8 keys: K-proj columns and xkv bytes shrink
    lc = min(jt * 128, max(8, int(-(-int(cnts.max()) // 8)) * 8))
    lc = max(lc, (jt - 1) * 128 + 8)

    def _tile_pair(w):
        # [D, DG] -> [128, pair*KT*128 + k*128 + c] pair-major
        return np.ascontiguousarray(
            w.reshape(KT, 128, PAIRS, 128).transpose(1, 2, 0, 3)
            .reshape(128, PAIRS * KT * 128)).astype(BF16)

    per_g = []
    for g in range(2):
        cols = slice(g * DG, (g + 1) * DG)
        per_g.append({
            "wq": _tile_pair(np.ascontiguousarray(Wq[:, cols])),
            "wk": _tile_pair(np.ascontiguousarray(Wk[:, cols])),
            "wv": _tile_k(np.ascontiguousarray(Wv[:, cols]), DG),
            "wo": np.ascontiguousarray(
                Wo[g * DG:(g + 1) * DG, :]
                .reshape(PAIRS, 128, D).transpose(1, 0, 2)
                .reshape(128, PAIRS * D)).astype(BF16),
        })
    per_b = []
    for b in range(B):
        idx = np.flatnonzero(~pad_mask[b])
        n = len(idx)
        xc = np.zeros((lc, D), dtype=np.float32)
        xc[:n] = x_kv[b][idx]
        mbias = np.full(jt * 128, MASK_NEG, dtype=np.float32)
        mbias[:n] = 0.0
        per_b.append({
            "xq": _tile_k(np.ascontiguousarray(x_q[b].T), N),
            "xkv": _tile_k(np.ascontiguousarray(xc.T), lc),
            "mb": np.ascontiguousarray(mbias.reshape(jt, 128).T),
        })

    in_maps = []
    for c in range(NCORES):
        b, g = c // 2, c % 2
        in_maps.append({**per_b[b], **per_g[g]})
    return in_maps, (jt, lc)


def kernel(x_q, x_kv, pad_mask, Wq, Wk, Wv, Wo, bo):
    in_maps, (jt, lc) = _make_in_maps(x_q, x_kv, pad_mask, Wq, Wk, Wv, Wo)
    nc = _get_nc(jt, lc)
    res = run_bass_kernel_spmd(nc, in_maps, core_ids=list(range(NCORES)))
    full = np.empty((B, N, D), dtype=np.float32)
    bo32 = bo.astype(np.float32)
    for b in range(B):
        full[b] = (res.results[2 * b]["out"].astype(np.float32)
                   + res.results[2 * b + 1]["out"].astype(np.float32))
        full[b] += bo32
    return full
